# revision 1
# baseline (speedup 1.0000x reference)
"""DRIM layer (distorted Rytov inverse-scattering iteration) on Trainium2.

One Bass/Tile program per core (replicated SPMD on 8 cores):
  P1  Z-matrix build via large-branch Hankel evaluation (upper triangle only;
      Z is complex-symmetric), resident in SBUF as fp32r planes
  P2  block LDL^T elimination, Newton-iterated 128x128 block inverses,
      fp32r tensor-engine matmuls
  P3  back-substitution -> X = Z^-1 [-E_inc | -G]
  P4  total field, RSS power model, data vector
  P5  Rytov H^T rows (4608 x 1664 padded) + H^T d
  P7  Gram H H^T (upper blocks) + Jacobi scaling
  P8  scaled SPD block solve (same Newton machinery, real)
  P9  chi = H^T y, output dchi

Host does input packing / output reshape only.
"""
import math
import os
import numpy as np

import concourse.bass as bass
import concourse.bacc as bacc
import concourse.bass_isa as bass_isa
import concourse.mybir as mybir
import concourse.tile as tile
from concourse.bass_utils import run_bass_kernel_spmd

F32 = mybir.dt.float32
F32R = mybir.dt.float32r
U8 = mybir.dt.uint8
AF = mybir.ActivationFunctionType
ALU = mybir.AluOpType
AXX = mybir.AxisListType.X

M = 48
N = M * M
NB = N // 128               # 18
TX = RX = 40
NL = TX * (RX - 1)          # 1560
LPAD = 1664
LB = LPAD // 128            # 13
RW = 256                    # [0:128]=Re plane, [128:256]=Im plane
CW = 256                    # Z-build column chunk
DOI = 3.0
WL = 0.125
K0 = 2.0 * math.pi / WL
IMP = 120.0 * math.pi
GRID_LEN = DOI / M
GRID_RADIUS = math.sqrt(GRID_LEN ** 2 / math.pi)
NOISE = 1e-6

def _j1s(x):
    t2 = (x / 3.0) ** 2
    return x * (0.5 - 0.56249985*t2 + 0.21093573*t2**2 - 0.03954289*t2**3
                + 0.00443319*t2**4 - 0.00031761*t2**5 + 0.00001109*t2**6)

def _y1s(x):
    t2 = (x / 3.0) ** 2
    p = (-0.6366198 + 0.2212091*t2 + 2.1682709*t2**2 - 1.3164827*t2**3
         + 0.3123951*t2**4 - 0.0400976*t2**5 + 0.0027873*t2**6)
    return ((2.0/math.pi) * x * math.log(0.5*x) * _j1s(x) + p) / x

X0C = K0 * GRID_RADIUS
GRID_AREA = 4.0*math.pi*GRID_RADIUS/(2.0*K0) * _j1s(X0C)
C1 = -IMP * math.pi * GRID_RADIUS / 2.0
C2 = _j1s(X0C)
C3R, C3I = _j1s(X0C), _y1s(X0C)
C1C2 = C1 * C2
ZD_RE = C1 * C3R
ZD_IM_C = C1 * C3I
SA = GRID_AREA * K0 * K0
TWO_PI = 2.0 * math.pi
INV_2PI = 1.0 / TWO_PI
LOG10E20 = 20.0 * math.log10(math.e)
CADD = 10.0 * math.log10(WL * WL / (4.0 * math.pi * IMP) / 1e-3)
C20L = 20.0 / math.log(10.0)

F0C = [0.79788456, -0.00000077, -0.00552740, -0.00009512,
       0.00137237, -0.00072805, 0.00014476]
THC = [-0.78539816, -0.04166397, -0.00003954, 0.00262573,
       -0.00054125, -0.00029333, 0.00013558]
F0CS = [c * (3.0 ** k) * C1C2 for k, c in enumerate(F0C)]
THCS = [c * (3.0 ** k) for k, c in enumerate(THC)]

NEWTON_Z = 22
NEWTON_SPD = 22


def _horner(nc, out_ap, s_ap, coeffs):
    cs = coeffs[::-1]
    nc.vector.tensor_scalar(out=out_ap, in0=s_ap, scalar1=float(cs[0]),
                            scalar2=float(cs[1]), op0=ALU.mult, op1=ALU.add)
    for c in cs[2:]:
        nc.vector.tensor_tensor(out=out_ap, in0=out_ap, in1=s_ap, op=ALU.mult)
        nc.vector.tensor_scalar(out=out_ap, in0=out_ap, scalar1=float(c),
                                scalar2=None, op0=ALU.add)


def _cmm(nc, pool, lhsT, rhs, n=RW):
    P1 = pool.tile([128, n], F32, tag="cmmp1")
    P2 = pool.tile([128, n], F32, tag="cmmp2")
    nc.tensor.matmul(P1[:], lhsT[:, 0:128], rhs, start=True, stop=True)
    nc.tensor.matmul(P2[:], lhsT[:, 128:256], rhs, start=True, stop=True)
    return P1, P2


def _combine_sub(nc, dst, P1, P2):
    nc.vector.tensor_tensor(out=dst[:, 0:256], in0=dst[:, 0:256],
                            in1=P1[:, 0:256], op=ALU.subtract)
    nc.vector.tensor_tensor(out=dst[:, 0:128], in0=dst[:, 0:128],
                            in1=P2[:, 128:256], op=ALU.add)
    nc.vector.tensor_tensor(out=dst[:, 128:256], in0=dst[:, 128:256],
                            in1=P2[:, 0:128], op=ALU.subtract)


def _combine_set(nc, dst, P1, P2):
    nc.vector.tensor_copy(dst[:, 0:256], P1[:, 0:256])
    nc.vector.tensor_tensor(out=dst[:, 0:128], in0=dst[:, 0:128],
                            in1=P2[:, 128:256], op=ALU.subtract)
    nc.vector.tensor_tensor(out=dst[:, 128:256], in0=dst[:, 128:256],
                            in1=P2[:, 0:128], op=ALU.add)


def _newton_scale(nc, work, pmisc, m, tag):
    """1/(colmax * rowmax) of m [128,128] -> [128,1] fp32 AP."""
    ones = work.tile([128, 1], F32, tag=f"nwo_{tag}")
    nc.vector.memset(ones[:], 1.0)
    pc = pmisc.tile([128, 1], F32, tag=f"nwpc_{tag}")
    nc.tensor.matmul(pc[:], m[:], ones[:], start=True, stop=True)
    pr = pmisc.tile([1, 128], F32, tag=f"nwpr_{tag}")
    nc.tensor.matmul(pr[:], ones[:], m[:], start=True, stop=True)
    cs = work.tile([128, 1], F32, tag=f"nwcs_{tag}")
    nc.vector.tensor_copy(cs[:], pc[:])
    rs = work.tile([1, 128], F32, tag=f"nwrs_{tag}")
    nc.vector.tensor_copy(rs[:], pr[:])
    nc.gpsimd.partition_all_reduce(cs[:], cs[:], 128, bass_isa.ReduceOp.max)
    rmax = work.tile([1, 1], F32, tag=f"nwrm_{tag}")
    nc.vector.tensor_reduce(rmax[:], rs[:], axis=AXX, op=ALU.max)
    rmax_b = work.tile([128, 1], F32, tag=f"nwrb_{tag}")
    nc.gpsimd.partition_broadcast(rmax_b[:], rmax[:])
    a = work.tile([128, 1], F32, tag=f"nwa_{tag}")
    nc.vector.tensor_tensor(out=a[:], in0=cs[:], in1=rmax_b[:], op=ALU.mult)
    nc.vector.reciprocal(a[:], a[:])
    return a


def _newton_cplx(nc, work, pmm, pmisc, D, Xout, id_s, iters):
    m = work.tile([128, 128], F32, tag="nw_m")
    m2 = work.tile([128, 128], F32, tag="nw_m2")
    nc.scalar.activation(m[:], D[:, 0:128], AF.Abs)
    nc.scalar.activation(m2[:], D[:, 128:256], AF.Abs)
    nc.vector.tensor_tensor(out=m[:], in0=m[:], in1=m2[:], op=ALU.max)
    a = _newton_scale(nc, work, pmisc, m, "c")
    nc.vector.tensor_scalar(out=Xout[:, 0:128], in0=D[:, 0:128], scalar1=a[:],
                            scalar2=None, op0=ALU.mult)
    nc.vector.tensor_scalar(out=Xout[:, 128:256], in0=D[:, 128:256],
                            scalar1=a[:], scalar2=None, op0=ALU.mult)
    nc.vector.tensor_scalar(out=Xout[:, 128:256], in0=Xout[:, 128:256],
                            scalar1=-1.0, scalar2=None, op0=ALU.mult)
    R = work.tile([128, RW], F32R, tag="nw_R")
    for _ in range(iters):
        P1, P2 = _cmm(nc, pmm, D, Xout[:, 0:RW])
        nc.vector.tensor_tensor(out=R[:, 0:128], in0=id_s[:],
                                in1=P1[:, 0:128], op=ALU.subtract)
        nc.vector.tensor_tensor(out=R[:, 0:128], in0=R[:, 0:128],
                                in1=P2[:, 128:256], op=ALU.add)
        nc.vector.tensor_scalar(out=R[:, 128:256], in0=P1[:, 128:256],
                                scalar1=-1.0, scalar2=None, op0=ALU.mult)
        nc.vector.tensor_tensor(out=R[:, 128:256], in0=R[:, 128:256],
                                in1=P2[:, 0:128], op=ALU.subtract)
        Q1, Q2 = _cmm(nc, pmm, Xout, R[:, 0:RW])
        nc.vector.tensor_tensor(out=Xout[:, 0:256], in0=Xout[:, 0:256],
                                in1=Q1[:, 0:256], op=ALU.add)
        nc.vector.tensor_tensor(out=Xout[:, 0:128], in0=Xout[:, 0:128],
                                in1=Q2[:, 128:256], op=ALU.subtract)
        nc.vector.tensor_tensor(out=Xout[:, 128:256], in0=Xout[:, 128:256],
                                in1=Q2[:, 0:128], op=ALU.add)


def _newton_real(nc, work, pmm, pmisc, D, Xout, id_s, iters):
    m = work.tile([128, 128], F32, tag="nw_m")
    nc.scalar.activation(m[:], D[:], AF.Abs)
    a = _newton_scale(nc, work, pmisc, m, "r")
    nc.vector.tensor_scalar(out=Xout[:], in0=D[:], scalar1=a[:], scalar2=None,
                            op0=ALU.mult)
    R = work.tile([128, 128], F32R, tag="nw_R")
    for _ in range(iters):
        P1 = pmm.tile([128, 128], F32, tag="cmmp1")
        nc.tensor.matmul(P1[:], D[:], Xout[:], start=True, stop=True)
        nc.vector.tensor_tensor(out=R[:], in0=id_s[:], in1=P1[:],
                                op=ALU.subtract)
        Q1 = pmm.tile([128, 128], F32, tag="cmmp2")
        nc.tensor.matmul(Q1[:], Xout[:], R[:], start=True, stop=True)
        nc.vector.tensor_tensor(out=Xout[:], in0=Xout[:], in1=Q1[:], op=ALU.add)


def build_program(link_groups, alpha):
    nc = bacc.Bacc("TRN2", target_bir_lowering=False, num_devices=8)
    din = {}
    def inp(name, shape, dtype=F32):
        din[name] = nc.dram_tensor(name, shape, dtype, kind="ExternalInput")
    inp("geomS", [4, N]); inp("geomR", [4, N]); inp("scat_t", [128, NB])
    inp("bpack", [N, RW]); inp("gscT", [N, 80]); inp("dfpack", [40, 80])
    inp("tpT", [40, RX - 1]); inp("id128", [128, 128]); inp("idu8", [128, 128], U8)
    out_chi = nc.dram_tensor("out_chi", [2 * N], F32, kind="ExternalOutput")
    xdbg = nc.dram_tensor("xdbg", [N, RW], F32, kind="ExternalOutput")
    tfdbg = nc.dram_tensor("tfdbg", [40, 80], F32, kind="ExternalOutput")
    ddbg = nc.dram_tensor("ddbg", [40, RX - 1], F32, kind="ExternalOutput")
    scr = {}
    scr["vdram"] = nc.dram_tensor("vdram", [NB * 128, RW], F32R, kind="Internal")
    scr["utdram"] = nc.dram_tensor("utdram", [N, 2 * N], F32R, kind="Internal")
    scr["htdram"] = nc.dram_tensor("htdram", [2 * N, LPAD], F32, kind="Internal")
    scr["gramdram"] = nc.dram_tensor("gramdram", [LPAD, LPAD], F32, kind="Internal")
    scr["v2dram"] = nc.dram_tensor("v2dram", [LB * 128, 128], F32R, kind="Internal")
    scr["ut2dram"] = nc.dram_tensor("ut2dram", [LPAD, LPAD], F32R, kind="Internal")
    scr["sdram"] = nc.dram_tensor("sdram", [NL], F32, kind="Internal")
    scr["wdram"] = nc.dram_tensor("wdram", [2 * NL], F32, kind="Internal")
    scr["srowdram"] = nc.dram_tensor("srowdram", [LPAD], F32, kind="Internal")
    scr["yrowdram"] = nc.dram_tensor("yrowdram", [LPAD], F32, kind="Internal")

    with tile.TileContext(nc) as tc:
        _body(nc, tc, din, out_chi, xdbg, tfdbg, ddbg, scr, link_groups, alpha)
    nc.compile()
    return nc


def _body(nc, tc, din, out_chi, xdbg, tfdbg, ddbg, scr, link_groups, alpha):
    import contextlib
    ctx = contextlib.ExitStack()
    consts = ctx.enter_context(tc.tile_pool(name="consts", bufs=1))
    id_s = consts.tile([128, 128], F32)
    nc.sync.dma_start(id_s[:], din["id128"][:])
    idr_s = consts.tile([128, 128], F32R)
    nc.vector.tensor_copy(idr_s[:], id_s[:])
    idu_s = consts.tile([128, 128], U8)
    nc.sync.dma_start(idu_s[:], din["idu8"][:])
    scat_s = consts.tile([128, NB], F32)
    nc.sync.dma_start(scat_s[:], din["scat_t"][:])

    zdi_s = consts.tile([128, NB], F32)
    fsc_s = consts.tile([128, NB], F32)
    t0 = consts.tile([128, NB], F32)
    nc.vector.tensor_scalar(out=t0[:], in0=scat_s[:], scalar1=-1.0,
                            scalar2=None, op0=ALU.add)
    nc.vector.reciprocal(t0[:], t0[:])
    nc.vector.tensor_scalar(out=fsc_s[:], in0=t0[:], scalar1=(IMP / K0),
                            scalar2=None, op0=ALU.mult)
    nc.vector.tensor_tensor(out=t0[:], in0=t0[:], in1=scat_s[:], op=ALU.mult)
    nc.vector.tensor_scalar(out=zdi_s[:], in0=t0[:], scalar1=-(IMP / K0),
                            scalar2=ZD_IM_C, op0=ALU.mult, op1=ALU.add)
    zdr_c = consts.tile([128, 1], F32)
    nc.vector.memset(zdr_c[:], float(ZD_RE))

    bf_pool = ctx.enter_context(tc.tile_pool(name="bf", bufs=1))
    BF = [bf_pool.tile([128, RW], F32R, tag=f"bf{i}", name=f"bf{i}") for i in range(NB)]

    with tc.tile_pool(name="tri", bufs=1) as tri:
        ZT = {}
        for i in range(NB):
            for j in range(i, NB):
                ZT[(i, j)] = tri.tile([128, RW], F32R, tag=f"z{i}_{j}", name=f"z{i}_{j}")

        # ---------------- P1: Z build ----------------
        with (
            tc.tile_pool(name="zb_geom", bufs=2) as gpool,
            tc.tile_pool(name="zb_work", bufs=1) as work,
            tc.tile_pool(name="zb_psum", bufs=2, space="PSUM") as pz,
        ):
            for k in range(NB):
                r0 = 128 * k
                gS = gpool.tile([4, 128], F32, tag="gS", name="gS")
                nc.sync.dma_start(gS[:], din["geomS"][:, r0:r0+128])
                j = k
                while j < NB:
                    c0 = 128 * j
                    w = 256 if j + 1 < NB else 128
                    gR = work.tile([4, CW], F32, tag="gR", name="gR")
                    nc.sync.dma_start(gR[:, 0:w], din["geomR"][:, c0:c0+w])
                    # one 128-col block per chunk (CW=256 covers Re|Im writes)
                    pd = pz.tile([128, CW], F32, tag="zb_pd")
                    nc.tensor.matmul(pd[:, 0:w], gS[:], gR[:, 0:w],
                                     start=True, stop=True)
                    dsq = work.tile([128, CW], F32, tag="zb_dsq")
                    nc.vector.tensor_scalar(out=dsq[:, 0:w], in0=pd[:, 0:w],
                                            scalar1=0.002, scalar2=None,
                                            op0=ALU.max)
                    x = work.tile([128, CW], F32, tag="zb_x")
                    nc.scalar.activation(x[:, 0:w], dsq[:, 0:w], AF.Sqrt,
                                         scale=float(K0 * K0))
                    sp = work.tile([128, CW], F32, tag="zb_sp")
                    nc.vector.reciprocal(sp[:, 0:w], x[:, 0:w])
                    f0 = work.tile([128, CW], F32, tag="zb_f0")
                    _horner(nc, f0[:, 0:w], sp[:, 0:w], F0CS)
                    th = work.tile([128, CW], F32, tag="zb_th")
                    _horner(nc, th[:, 0:w], sp[:, 0:w], THCS)
                    nc.vector.tensor_tensor(out=th[:, 0:w], in0=th[:, 0:w],
                                            in1=x[:, 0:w], op=ALU.add)
                    nc.scalar.activation(x[:, 0:w], sp[:, 0:w], AF.Sqrt)
                    nc.vector.tensor_tensor(out=f0[:, 0:w], in0=f0[:, 0:w],
                                            in1=x[:, 0:w], op=ALU.mult)
                    u = work.tile([128, CW], F32, tag="zb_u")
                    nc.vector.tensor_scalar(out=u[:, 0:w], in0=th[:, 0:w],
                                            scalar1=INV_2PI, scalar2=None,
                                            op0=ALU.mult)
                    ki = work.tile([128, CW], mybir.dt.int32, tag="zb_ki")
                    nc.vector.tensor_copy(ki[:, 0:w], u[:, 0:w])
                    mf = work.tile([128, CW], F32, tag="zb_mf")
                    nc.vector.tensor_copy(mf[:, 0:w], ki[:, 0:w])
                    r1 = work.tile([128, CW], F32, tag="zb_r1")
                    nc.vector.tensor_scalar(out=r1[:, 0:w], in0=mf[:, 0:w],
                                            scalar1=-TWO_PI, scalar2=None,
                                            op0=ALU.mult)
                    nc.vector.tensor_tensor(out=r1[:, 0:w], in0=r1[:, 0:w],
                                            in1=th[:, 0:w], op=ALU.add)
                    sinr = work.tile([128, CW], F32, tag="zb_sin")
                    nc.scalar.activation(sinr[:, 0:w], r1[:, 0:w], AF.Sin)
                    nc.vector.tensor_scalar(out=u[:, 0:w], in0=u[:, 0:w],
                                            scalar1=0.25, scalar2=None, op0=ALU.add)
                    nc.vector.tensor_copy(ki[:, 0:w], u[:, 0:w])
                    nc.vector.tensor_copy(mf[:, 0:w], ki[:, 0:w])
                    nc.vector.tensor_scalar(out=mf[:, 0:w], in0=mf[:, 0:w],
                                            scalar1=-TWO_PI,
                                            scalar2=(math.pi / 2.0),
                                            op0=ALU.mult, op1=ALU.add)
                    nc.vector.tensor_tensor(out=mf[:, 0:w], in0=mf[:, 0:w],
                                            in1=th[:, 0:w], op=ALU.add)
                    cosr = work.tile([128, CW], F32, tag="zb_cos")
                    nc.scalar.activation(cosr[:, 0:w], mf[:, 0:w], AF.Sin)
                    nc.vector.tensor_tensor(out=cosr[:, 0:w], in0=cosr[:, 0:w],
                                            in1=f0[:, 0:w], op=ALU.mult)
                    nc.vector.tensor_tensor(out=sinr[:, 0:w], in0=sinr[:, 0:w],
                                            in1=f0[:, 0:w], op=ALU.mult)
                    if j == k:
                        nc.vector.copy_predicated(
                            cosr[:, 0:128], idu_s[:],
                            zdr_c[:].broadcast_to([128, 128]))
                        nc.vector.copy_predicated(
                            sinr[:, 0:128], idu_s[:],
                            zdi_s[:, k:k+1].broadcast_to([128, 128]))
                    for b in range(w // 128):
                        nc.vector.tensor_copy(ZT[(k, j + b)][:, 0:128],
                                              cosr[:, 128*b:128*b+128])
                        nc.vector.tensor_copy(ZT[(k, j + b)][:, 128:256],
                                              sinr[:, 128*b:128*b+128])
                    j += w // 128

        # ---------------- P2: block LDL^T ----------------
        with (
            tc.tile_pool(name="lu_work", bufs=2) as work,
            tc.tile_pool(name="lu_pmm", bufs=2, space="PSUM") as pmm,
            tc.tile_pool(name="lu_pmisc", bufs=1, space="PSUM") as pmisc,
        ):
            ldtmp0 = work.tile([128, RW], F32, tag="ldtmp")
            for i in range(NB):
                nc.sync.dma_start(ldtmp0[:], din["bpack"][128*i:128*(i+1), :])
                nc.vector.tensor_copy(BF[i][:], ldtmp0[:])
                ldtmp0 = work.tile([128, RW], F32, tag="ldtmp")
            for k in range(NB):
                V = work.tile([128, RW], F32R, tag="lu_V")
                _newton_cplx(nc, work, pmm, pmisc, ZT[(k, k)], V, id_s, NEWTON_Z)
                nc.sync.dma_start(scr["vdram"][128*k:128*(k+1), :], V[:])
                for i in range(k + 1, NB):
                    ptr = pmisc.tile([128, 128], F32R, tag="lu_ptr")
                    nc.tensor.transpose(ptr[:], ZT[(k, i)][:, 0:128], idr_s[:])
                    utt = work.tile([128, RW], F32R, tag="lu_utt")
                    nc.vector.tensor_copy(utt[:, 0:128], ptr[:])
                    pti = pmisc.tile([128, 128], F32R, tag="lu_pti")
                    nc.tensor.transpose(pti[:], ZT[(k, i)][:, 128:256], idr_s[:])
                    nc.vector.tensor_copy(utt[:, 128:256], pti[:])
                    nc.sync.dma_start(
                        scr["utdram"][128*i:128*(i+1), 256*k:256*(k+1)], utt[:])
                for i in range(k + 1, NB):
                    P1, P2 = _cmm(nc, pmm, V, ZT[(k, i)][:, 0:RW])
                    LT = work.tile([128, RW], F32R, tag="lu_LT")
                    _combine_set(nc, LT, P1, P2)
                    LTn = work.tile([128, 128], F32R, tag="lu_LTn")
                    nc.vector.tensor_scalar(out=LTn[:], in0=LT[:, 128:256],
                                            scalar1=-1.0, scalar2=None,
                                            op0=ALU.mult)

                    def upd(rhs, dst):
                        P = pmm.tile([128, RW], F32, tag="cmmp1", name="updP")
                        nc.tensor.matmul(P[:, 0:128], LT[:, 0:128],
                                         rhs[:, 0:128], start=True, stop=False)
                        nc.tensor.matmul(P[:, 0:128], LTn[:],
                                         rhs[:, 128:256], start=False, stop=True)
                        nc.tensor.matmul(P[:, 128:256], LT[:, 0:128],
                                         rhs[:, 128:256], start=True, stop=False)
                        nc.tensor.matmul(P[:, 128:256], LT[:, 128:256],
                                         rhs[:, 0:128], start=False, stop=True)
                        nc.vector.tensor_tensor(out=dst[:, 0:256],
                                                in0=dst[:, 0:256],
                                                in1=P[:, 0:256],
                                                op=ALU.subtract)

                    upd(BF[k], BF[i])
                    for j in range(i, NB):
                        upd(ZT[(k, j)], ZT[(i, j)])

    # ---------------- P3: back-substitution ----------------
    with (
        tc.tile_pool(name="bs_work", bufs=3) as work,
        tc.tile_pool(name="bs_pacc", bufs=1, space="PSUM") as pacc,
        tc.tile_pool(name="bs_pmm", bufs=2, space="PSUM") as pmm,
    ):
        for k in range(NB - 1, -1, -1):
            W = work.tile([128, RW], F32R, tag="bs_W")
            nc.vector.tensor_copy(W[:], BF[k][:])
            if k < NB - 1:
                P1 = pacc.tile([128, RW], F32, tag="bs_p1")
                P2 = pacc.tile([128, RW], F32, tag="bs_p2")
                for idx, j in enumerate(range(k + 1, NB)):
                    utt = work.tile([128, RW], F32R, tag="bs_utt")
                    nc.sync.dma_start(
                        utt[:], scr["utdram"][128*j:128*(j+1), 256*k:256*(k+1)])
                    st = (idx == 0); sp_ = (j == NB - 1)
                    nc.tensor.matmul(P1[:], utt[:, 0:128], BF[j][:, 0:RW],
                                     start=st, stop=sp_)
                    nc.tensor.matmul(P2[:], utt[:, 128:256], BF[j][:, 0:RW],
                                     start=st, stop=sp_)
                _combine_sub(nc, W, P1, P2)
            Vk = work.tile([128, RW], F32R, tag="bs_V")
            nc.sync.dma_start(Vk[:], scr["vdram"][128*k:128*(k+1), :])
            P1, P2 = _cmm(nc, pmm, Vk, W[:, 0:RW])
            _combine_set(nc, BF[k], P1, P2)
            nc.sync.dma_start(xdbg[128*k:128*(k+1), :], BF[k][:].bitcast(F32))

    # ---------------- P4: tf + data vector ----------------
    late = ctx.enter_context(tc.tile_pool(name="late", bufs=1))
    dvec = late.tile([128, LB], F32)
    drep = late.tile([128, LPAD], F32)
    wrep_r = late.tile([128, NL], F32)
    wrep_i = late.tile([128, NL], F32)
    with (
        tc.tile_pool(name="p4_work", bufs=2) as work,
        tc.tile_pool(name="p4_pacc", bufs=1, space="PSUM") as pacc,
        tc.tile_pool(name="p4_pmisc", bufs=1, space="PSUM") as pmisc,
    ):
        Ptf1 = pacc.tile([40, RW], F32, tag="tf_p1")
        Ptf2 = pacc.tile([40, RW], F32, tag="tf_p2")
        for i in range(NB):
            gt = work.tile([128, 80], F32, tag="tf_g")
            nc.sync.dma_start(gt[:], din["gscT"][128*i:128*(i+1), :])
            gtr = work.tile([128, 80], F32R, tag="tf_gr")
            nc.vector.tensor_copy(gtr[:], gt[:])
            st = (i == 0); sp_ = (i == NB - 1)
            nc.tensor.matmul(Ptf1[:], gtr[:, 0:40], BF[i][:, 0:RW],
                             start=st, stop=sp_)
            nc.tensor.matmul(Ptf2[:], gtr[:, 40:80], BF[i][:, 0:RW],
                             start=st, stop=sp_)
        df = work.tile([40, 80], F32, tag="tf_df")
        nc.sync.dma_start(df[:], din["dfpack"][:])
        tfr = work.tile([40, 40], F32, tag="tfr")
        tfi = work.tile([40, 40], F32, tag="tfi")
        nc.vector.tensor_tensor(out=tfr[:], in0=df[:, 0:40],
                                in1=Ptf1[:, 0:40], op=ALU.add)
        nc.vector.tensor_tensor(out=tfr[:], in0=tfr[:],
                                in1=Ptf2[:, 128:168], op=ALU.subtract)
        nc.vector.tensor_tensor(out=tfi[:], in0=df[:, 40:80],
                                in1=Ptf1[:, 128:168], op=ALU.add)
        nc.vector.tensor_tensor(out=tfi[:], in0=tfi[:],
                                in1=Ptf2[:, 0:40], op=ALU.add)
        tfd = work.tile([40, 80], F32, tag="tf_out")
        nc.vector.tensor_copy(tfd[:, 0:40], tfr[:])
        nc.vector.tensor_copy(tfd[:, 40:80], tfi[:])
        nc.sync.dma_start(tfdbg[:], tfd[:])

        pw = work.tile([40, 40], F32, tag="pw")
        nc.vector.tensor_tensor(out=pw[:], in0=tfr[:], in1=tfr[:], op=ALU.mult)
        t1 = work.tile([40, 40], F32, tag="pw_t")
        nc.vector.tensor_tensor(out=t1[:], in0=tfi[:], in1=tfi[:], op=ALU.mult)
        nc.vector.tensor_tensor(out=pw[:], in0=pw[:], in1=t1[:], op=ALU.add)
        amp = work.tile([40, 40], F32, tag="amp")
        nc.scalar.activation(amp[:], pw[:], AF.Sqrt)
        nc.vector.tensor_scalar(out=amp[:], in0=amp[:], scalar1=NOISE,
                                scalar2=None, op0=ALU.add)
        nc.scalar.activation(amp[:], amp[:], AF.Ln)
        tpi = work.tile([40, 40], F32, tag="tpi")
        nc.vector.tensor_scalar(out=tpi[:], in0=amp[:], scalar1=C20L,
                                scalar2=CADD, op0=ALU.mult, op1=ALU.add)
        rec = work.tile([40, 40], F32, tag="rec")
        nc.vector.reciprocal(rec[:], pw[:])
        wr = work.tile([40, 40], F32, tag="wr")
        nc.vector.tensor_tensor(out=wr[:], in0=tfr[:], in1=rec[:], op=ALU.mult)
        nc.vector.tensor_scalar(out=wr[:], in0=wr[:], scalar1=SA, scalar2=None,
                                op0=ALU.mult)
        wi = work.tile([40, 40], F32, tag="wi")
        nc.vector.tensor_tensor(out=wi[:], in0=tfi[:], in1=rec[:], op=ALU.mult)
        nc.vector.tensor_scalar(out=wi[:], in0=wi[:], scalar1=-SA, scalar2=None,
                                op0=ALU.mult)

        def t40(src, name):
            pt = pmisc.tile([40, 40], F32, tag=f"t40p_{name}")
            nc.tensor.matmul(pt[:], src[:], id_s[0:40, 0:40], start=True,
                             stop=True)
            d = work.tile([40, 40], F32, tag=f"t40_{name}")
            nc.vector.tensor_copy(d[:], pt[:])
            return d
        tpiT = t40(tpi, "tpi"); wrT = t40(wr, "wr"); wiT = t40(wi, "wi")

        pack = work.tile([40, 120], F32, tag="pack")
        nc.vector.tensor_copy(pack[:, 0:40], tpiT[:])
        nc.vector.tensor_copy(pack[:, 40:80], wrT[:])
        nc.vector.tensor_copy(pack[:, 80:120], wiT[:])
        kept3 = work.tile([1, 3 * NL], F32, tag="kept3")
        pack3d = pack[:].rearrange("p (a b) -> p a b", a=3)
        kept3d = kept3[:].rearrange("p (a b) -> p a b", a=3)
        for (t, rs_list) in link_groups:
            o = _GBASE[t]
            for (s0, ln) in _contig_segments(rs_list):
                nc.sync.dma_start(kept3d[0:1, :, o:o+ln],
                                  pack3d[t:t+1, :, s0:s0+ln])
                o += ln
        # data = (tpT - tpi_kept)/LOG10E20 on the packed row
        tprow = work.tile([1, NL], F32, tag="tprow")
        nc.sync.dma_start(tprow[:], bass.AP(din["tpT"], 0, [[1, NL]]))
        nc.vector.tensor_tensor(out=kept3[0:1, 0:NL], in0=tprow[:],
                                in1=kept3[0:1, 0:NL], op=ALU.subtract)
        nc.vector.tensor_scalar(out=kept3[0:1, 0:NL], in0=kept3[0:1, 0:NL],
                                scalar1=1.0 / LOG10E20, scalar2=None,
                                op0=ALU.mult)
        nc.sync.dma_start(bass.AP(ddbg, 0, [[1, NL]]), kept3[0:1, 0:NL])
        nc.sync.dma_start(bass.AP(scr["sdram"], 0, [[1, NL]]), kept3[0:1, 0:NL])

        nc.vector.memset(dvec[:], 0.0)
        nc.sync.dma_start(dvec[:, 0:12],
                          bass.AP(scr["sdram"], 0, [[1, 128], [128, 12]]))
        nc.sync.dma_start(dvec[0:24, 12:13],
                          bass.AP(scr["sdram"], 1536, [[1, 24]]))
        nc.vector.memset(drep[:], 0.0)
        nc.gpsimd.partition_broadcast(drep[:, 0:NL], kept3[0:1, 0:NL])
        nc.gpsimd.partition_broadcast(wrep_r[:], kept3[0:1, NL:2*NL])
        nc.gpsimd.partition_broadcast(wrep_i[:], kept3[0:1, 2*NL:3*NL])

    # ---------------- P5: Ht build + v = Ht d ----------------
    vsum = late.tile([128, 2 * NB], F32)
    lam = late.tile([128, 1], F32)
    with tc.tile_pool(name="p5_work", bufs=2) as work:
        nc.vector.memset(vsum[:], 0.0)
        for i in range(NB):
            Gq = work.tile([128, 80], F32, tag="h_gq")
            Iq = work.tile([128, 80], F32, tag="h_iq")
            f_ap = fsc_s[:, i:i+1]
            nc.vector.tensor_scalar(out=Gq[:, 0:40], in0=BF[i][:, 168:208],
                                    scalar1=f_ap, scalar2=None, op0=ALU.mult)
            nc.vector.tensor_scalar(out=Gq[:, 0:40], in0=Gq[:, 0:40],
                                    scalar1=-1.0, scalar2=None, op0=ALU.mult)
            nc.vector.tensor_scalar(out=Gq[:, 40:80], in0=BF[i][:, 40:80],
                                    scalar1=f_ap, scalar2=None, op0=ALU.mult)
            nc.vector.tensor_scalar(out=Iq[:, 0:40], in0=BF[i][:, 128:168],
                                    scalar1=f_ap, scalar2=None, op0=ALU.mult)
            nc.vector.tensor_scalar(out=Iq[:, 0:40], in0=Iq[:, 0:40],
                                    scalar1=-1.0, scalar2=None, op0=ALU.mult)
            nc.vector.tensor_scalar(out=Iq[:, 40:80], in0=BF[i][:, 0:40],
                                    scalar1=f_ap, scalar2=None, op0=ALU.mult)
            Gg_r = work.tile([128, NL], F32, tag="h_ggr")
            Gg_i = work.tile([128, NL], F32, tag="h_ggi")
            qr = work.tile([128, NL], F32, tag="h_qr")
            qi = work.tile([128, NL], F32, tag="h_qi")
            base = 0
            for (t, rs_list) in link_groups:
                o = base
                for (s0, ln) in _contig_segments(rs_list):
                    nc.vector.tensor_copy(Gg_r[:, o:o+ln], Gq[:, s0:s0+ln])
                    nc.vector.tensor_copy(Gg_i[:, o:o+ln], Gq[:, 40+s0:40+s0+ln])
                    o += ln
                base += len(rs_list)
            uniform = (len(link_groups) == 40
                       and all(len(rs) == 39 for _, rs in link_groups))
            if uniform:
                # full-width inc multiply via 0-stride replicated APs
                IncR = Iq[:, 0:40].rearrange("p (t o) -> p t o", o=1
                                             ).broadcast_to([128, 40, 39])
                IncI = Iq[:, 40:80].rearrange("p (t o) -> p t o", o=1
                                              ).broadcast_to([128, 40, 39])
                Gg_r3 = Gg_r[:].rearrange("p (t j) -> p t j", t=40)
                Gg_i3 = Gg_i[:].rearrange("p (t j) -> p t j", t=40)
                qr3 = qr[:].rearrange("p (t j) -> p t j", t=40)
                qi3 = qi[:].rearrange("p (t j) -> p t j", t=40)
                nc.vector.tensor_tensor(out=qr3, in0=Gg_r3, in1=IncR,
                                        op=ALU.mult)
                nc.vector.tensor_tensor(out=qi3, in0=Gg_i3, in1=IncR,
                                        op=ALU.mult)
                nc.vector.tensor_tensor(out=Gg_i3, in0=Gg_i3, in1=IncI,
                                        op=ALU.mult)
                nc.vector.tensor_tensor(out=Gg_r3, in0=Gg_r3, in1=IncI,
                                        op=ALU.mult)
            else:
                base = 0
                for (t, rs_list) in link_groups:
                    sl = slice(base, base + len(rs_list))
                    nc.vector.tensor_scalar(out=qr[:, sl], in0=Gg_r[:, sl],
                                            scalar1=Iq[:, t:t+1], scalar2=None,
                                            op0=ALU.mult)
                    nc.vector.tensor_scalar(out=qi[:, sl], in0=Gg_i[:, sl],
                                            scalar1=Iq[:, t:t+1], scalar2=None,
                                            op0=ALU.mult)
                    nc.vector.tensor_scalar(out=Gg_i[:, sl], in0=Gg_i[:, sl],
                                            scalar1=Iq[:, 40+t:40+t+1],
                                            scalar2=None, op0=ALU.mult)
                    nc.vector.tensor_scalar(out=Gg_r[:, sl], in0=Gg_r[:, sl],
                                            scalar1=Iq[:, 40+t:40+t+1],
                                            scalar2=None, op0=ALU.mult)
                    base += len(rs_list)
            nc.vector.tensor_tensor(out=qr[:], in0=qr[:], in1=Gg_i[:],
                                    op=ALU.subtract)
            nc.vector.tensor_tensor(out=qi[:], in0=qi[:], in1=Gg_r[:],
                                    op=ALU.add)
            hr = work.tile([128, LPAD], F32, tag="h_hr")
            hi = work.tile([128, LPAD], F32, tag="h_hi")
            t2 = work.tile([128, NL], F32, tag="h_t2")
            nc.vector.memset(hr[:], 0.0)
            nc.vector.memset(hi[:], 0.0)
            nc.vector.tensor_tensor(out=hr[:, 0:NL], in0=qr[:], in1=wrep_r[:],
                                    op=ALU.mult)
            nc.vector.tensor_tensor(out=t2[:], in0=qi[:], in1=wrep_i[:],
                                    op=ALU.mult)
            nc.vector.tensor_tensor(out=hr[:, 0:NL], in0=hr[:, 0:NL], in1=t2[:],
                                    op=ALU.subtract)
            nc.vector.tensor_tensor(out=hi[:, 0:NL], in0=qr[:], in1=wrep_i[:],
                                    op=ALU.mult)
            nc.vector.tensor_tensor(out=t2[:], in0=qi[:], in1=wrep_r[:],
                                    op=ALU.mult)
            nc.vector.tensor_tensor(out=hi[:, 0:NL], in0=hi[:, 0:NL], in1=t2[:],
                                    op=ALU.add)
            nc.vector.tensor_scalar(out=hi[:], in0=hi[:], scalar1=-1.0,
                                    scalar2=None, op0=ALU.mult)
            nc.sync.dma_start(scr["htdram"][128*i:128*(i+1), :], hr[:])
            nc.sync.dma_start(scr["htdram"][N+128*i:N+128*(i+1), :], hi[:])
            nc.vector.tensor_tensor(out=t2[:], in0=hr[:, 0:NL],
                                    in1=drep[:, 0:NL], op=ALU.mult)
            nc.vector.tensor_reduce(vsum[:, i:i+1], t2[:], axis=AXX, op=ALU.add)
            nc.vector.tensor_tensor(out=t2[:], in0=hi[:, 0:NL],
                                    in1=drep[:, 0:NL], op=ALU.mult)
            nc.vector.tensor_reduce(vsum[:, NB+i:NB+i+1], t2[:], axis=AXX,
                                    op=ALU.add)
        vsq = work.tile([128, 2 * NB], F32, tag="vsq")
        nc.vector.tensor_tensor(out=vsq[:], in0=vsum[:], in1=vsum[:],
                                op=ALU.mult)
        vred = work.tile([128, 1], F32, tag="vred")
        nc.vector.tensor_reduce(vred[:], vsq[:], axis=AXX, op=ALU.add)
        nc.gpsimd.partition_all_reduce(vred[:], vred[:], 128,
                                       bass_isa.ReduceOp.add)
        nc.scalar.activation(lam[:], vred[:], AF.Sqrt)
        nc.vector.tensor_scalar(out=lam[:], in0=lam[:], scalar1=float(alpha),
                                scalar2=None, op0=ALU.mult)

    # ---------------- P7: Gram ----------------
    st_ = late.tile([128, LB], F32)
    srep = late.tile([128, LPAD], F32)
    with (
        tc.tile_pool(name="g_acc", bufs=1) as gacc,
        tc.tile_pool(name="g_work", bufs=2) as work,
        tc.tile_pool(name="g_psum", bufs=4, space="PSUM") as pg,
    ):
        GA = [gacc.tile([128, LPAD], F32, tag=f"ga{l}", name=f"ga{l}") for l in range(LB)]
        for l in range(LB):
            nc.vector.memset(GA[l][:], 0.0)
        for ch in range(2 * NB):
            htc = work.tile([128, LPAD], F32, tag="g_htc")
            nc.sync.dma_start(htc[:], scr["htdram"][128*ch:128*(ch+1), :])
            htr = work.tile([128, LPAD], F32R, tag="g_htr")
            nc.vector.tensor_copy(htr[:], htc[:])
            for l in range(LB):
                c0 = 128 * l
                for cc in range(c0, LPAD, 416):
                    cw = min(416, LPAD - cc)
                    pgt = pg.tile([128, 416], F32, tag="g_pg")
                    nc.tensor.matmul(pgt[:, 0:cw], htr[:, c0:c0+128],
                                     htr[:, cc:cc+cw], start=True, stop=True)
                    nc.vector.tensor_tensor(out=GA[l][:, cc:cc+cw],
                                            in0=GA[l][:, cc:cc+cw],
                                            in1=pgt[:, 0:cw], op=ALU.add)
        for l in range(LB):
            nc.sync.dma_start(scr["gramdram"][128*l:128*(l+1), :], GA[l][:])
        gd = work.tile([128, LB], F32, tag="gd")
        nc.sync.dma_start(gd[:], bass.AP(scr["gramdram"], 0,
                                         [[LPAD + 1, 128],
                                          [128 * (LPAD + 1), LB]]))
        nc.vector.tensor_scalar(out=gd[:], in0=gd[:], scalar1=lam[:],
                                scalar2=None, op0=ALU.add)
        nc.scalar.activation(st_[:], gd[:], AF.Sqrt)
        nc.vector.reciprocal(st_[:], st_[:])
        ps_ = pg.tile([LB, 128], F32, tag="s_ps")
        nc.tensor.matmul(ps_[:], st_[:], id_s[:], start=True, stop=True)
        s13 = work.tile([LB, 128], F32, tag="s13")
        nc.vector.tensor_copy(s13[:], ps_[:])
        nc.sync.dma_start(bass.AP(scr["srowdram"], 0, [[1, LPAD]]), s13[:])
        srow = work.tile([1, LPAD], F32, tag="srow")
        nc.sync.dma_start(srow[:], bass.AP(scr["srowdram"], 0, [[1, LPAD]]))
        nc.gpsimd.partition_broadcast(srep[:], srow[:])

    # ---------------- P8: scaled SPD solve ----------------
    bf2_pool = ctx.enter_context(tc.tile_pool(name="bf2", bufs=1))
    BF2 = [bf2_pool.tile([128, 128], F32R, tag=f"bf2_{l}", name=f"bf2_{l}") for l in range(LB)]
    with (
        tc.tile_pool(name="s_tri", bufs=1) as tri2,
        tc.tile_pool(name="s_work", bufs=2) as work,
        tc.tile_pool(name="s_pmm", bufs=2, space="PSUM") as pmm,
        tc.tile_pool(name="s_pmisc", bufs=1, space="PSUM") as pmisc,
    ):
        dsc = work.tile([128, LB], F32, tag="dsc")
        nc.vector.tensor_tensor(out=dsc[:], in0=dvec[:], in1=st_[:], op=ALU.mult)
        zz = work.tile([128, 128], F32, tag="zz")
        nc.vector.memset(zz[:], 0.0)
        for l in range(LB):
            nc.vector.tensor_copy(BF2[l][:], zz[:])
            nc.vector.tensor_copy(BF2[l][:, 0:1], dsc[:, l:l+1])
        GT = {}
        for i in range(LB):
            for j in range(i, LB):
                GT[(i, j)] = tri2.tile([128, 128], F32R, tag=f"g{i}_{j}", name=f"g{i}_{j}")
                gload = work.tile([128, 128], F32, tag="g_load")
                nc.sync.dma_start(gload[:],
                                  scr["gramdram"][128*i:128*(i+1),
                                                  128*j:128*(j+1)])
                nc.vector.tensor_scalar(out=gload[:], in0=gload[:],
                                        scalar1=st_[:, i:i+1], scalar2=None,
                                        op0=ALU.mult)
                nc.vector.tensor_tensor(out=gload[:], in0=gload[:],
                                        in1=srep[:, 128*j:128*(j+1)],
                                        op=ALU.mult)
                if i == j:
                    ones1 = work.tile([128, 1], F32, tag="diag1")
                    nc.vector.memset(ones1[:], 1.0)
                    nc.vector.copy_predicated(gload[:], idu_s[:],
                                              ones1[:].broadcast_to([128, 128]))
                nc.vector.tensor_copy(GT[(i, j)][:], gload[:])
        for k in range(LB):
            V = work.tile([128, 128], F32R, tag="lu2_V")
            _newton_real(nc, work, pmm, pmisc, GT[(k, k)], V, id_s, NEWTON_SPD)
            nc.sync.dma_start(scr["v2dram"][128*k:128*(k+1), :], V[:])
            for i in range(k + 1, LB):
                ptr = pmisc.tile([128, 128], F32R, tag="lu2_ptr")
                nc.tensor.transpose(ptr[:], GT[(k, i)][:], idr_s[:])
                utt = work.tile([128, 128], F32R, tag="lu2_utt")
                nc.vector.tensor_copy(utt[:], ptr[:])
                nc.sync.dma_start(
                    scr["ut2dram"][128*i:128*(i+1), 128*k:128*(k+1)], utt[:])
            for i in range(k + 1, LB):
                pl = pmm.tile([128, 128], F32, tag="cmmp1")
                nc.tensor.matmul(pl[:], V[:], GT[(k, i)][:], start=True,
                                 stop=True)
                LT = work.tile([128, 128], F32R, tag="lu2_LT")
                nc.vector.tensor_copy(LT[:], pl[:])
                pb = pmm.tile([128, 128], F32, tag="cmmp2")
                nc.tensor.matmul(pb[:], LT[:], BF2[k][:], start=True, stop=True)
                nc.vector.tensor_tensor(out=BF2[i][:], in0=BF2[i][:],
                                        in1=pb[:], op=ALU.subtract)
                for j in range(i, LB):
                    pt_ = pmm.tile([128, 128], F32, tag="cmmp1")
                    nc.tensor.matmul(pt_[:], LT[:], GT[(k, j)][:], start=True,
                                     stop=True)
                    nc.vector.tensor_tensor(out=GT[(i, j)][:],
                                            in0=GT[(i, j)][:], in1=pt_[:],
                                            op=ALU.subtract)

    ys = late.tile([128, LB], F32)
    yrep = late.tile([128, LPAD], F32)
    with (
        tc.tile_pool(name="b2_work", bufs=3) as work,
        tc.tile_pool(name="b2_pacc", bufs=1, space="PSUM") as pacc,
        tc.tile_pool(name="b2_pmm", bufs=2, space="PSUM") as pmm,
    ):
        for k in range(LB - 1, -1, -1):
            W = work.tile([128, 128], F32R, tag="bs2_W")
            nc.vector.tensor_copy(W[:], BF2[k][:])
            if k < LB - 1:
                P1 = pacc.tile([128, 128], F32, tag="bs2_p1")
                for idx, j in enumerate(range(k + 1, LB)):
                    utt = work.tile([128, 128], F32R, tag="bs2_utt")
                    nc.sync.dma_start(
                        utt[:], scr["ut2dram"][128*j:128*(j+1),
                                               128*k:128*(k+1)])
                    nc.tensor.matmul(P1[:], utt[:], BF2[j][:],
                                     start=(idx == 0), stop=(j == LB - 1))
                nc.vector.tensor_tensor(out=W[:], in0=W[:], in1=P1[:],
                                        op=ALU.subtract)
            Vk = work.tile([128, 128], F32R, tag="bs2_V")
            nc.sync.dma_start(Vk[:], scr["v2dram"][128*k:128*(k+1), :])
            Pf = pmm.tile([128, 128], F32, tag="bs2_pf")
            nc.tensor.matmul(Pf[:], Vk[:], W[:], start=True, stop=True)
            nc.vector.tensor_copy(BF2[k][:], Pf[:])
        for l in range(LB):
            nc.vector.tensor_copy(ys[:, l:l+1], BF2[l][:, 0:1])
        nc.vector.tensor_tensor(out=ys[:], in0=ys[:], in1=st_[:], op=ALU.mult)
        psy = pmm.tile([LB, 128], F32, tag="y_ps")
        nc.tensor.matmul(psy[:], ys[:], id_s[:], start=True, stop=True)
        y13 = work.tile([LB, 128], F32, tag="y13")
        nc.vector.tensor_copy(y13[:], psy[:])
        nc.sync.dma_start(bass.AP(scr["yrowdram"], 0, [[1, LPAD]]), y13[:])
        yrow = work.tile([1, LPAD], F32, tag="yrow")
        nc.sync.dma_start(yrow[:], bass.AP(scr["yrowdram"], 0, [[1, LPAD]]))
        nc.gpsimd.partition_broadcast(yrep[:], yrow[:])

    # ---------------- P9: chi = Ht y ----------------
    with tc.tile_pool(name="p9_work", bufs=2) as work:
        chi = late.tile([128, 2 * NB], F32)
        for ch in range(2 * NB):
            htc = work.tile([128, LPAD], F32, tag="c_htc")
            nc.sync.dma_start(htc[:], scr["htdram"][128*ch:128*(ch+1), :])
            tm = work.tile([128, LPAD], F32, tag="c_tm")
            nc.vector.tensor_tensor(out=tm[:], in0=htc[:], in1=yrep[:],
                                    op=ALU.mult)
            nc.vector.tensor_reduce(chi[:, ch:ch+1], tm[:], axis=AXX,
                                    op=ALU.add)
        nc.sync.dma_start(bass.AP(out_chi, 0, [[1, 128], [128, 2 * NB]]),
                          chi[:])
    ctx.close()


_GBASE = {}

def _contig_segments(rs_list):
    segs = []
    s = rs_list[0]; prev = s
    for r in rs_list[1:]:
        if r == prev + 1:
            prev = r
        else:
            segs.append((s, prev - s + 1)); s = r; prev = r
    segs.append((s, prev - s + 1))
    return segs


_CACHED = {}


def kernel(epsilon_r_iter, chi_iter, total_power, alpha, grid_x, grid_y,
           direct_field, incident_field, G_freespace, G_freespace_scaled,
           sensor_links):
    eps = np.asarray(epsilon_r_iter)
    chi_it = np.asarray(chi_iter)
    tp = np.asarray(total_power, dtype=np.float32)
    alpha_f = float(np.asarray(alpha))
    gx = np.asarray(grid_x, dtype=np.float32)
    gy = np.asarray(grid_y, dtype=np.float32)
    df = np.asarray(direct_field)
    einc = np.asarray(incident_field)
    gfs = np.asarray(G_freespace)
    gsc = np.asarray(G_freespace_scaled)
    links = np.asarray(sensor_links)

    x = gx.T.reshape(N).astype(np.float32)
    y = gy.T.reshape(N).astype(np.float32)
    scat = np.real(eps.T.reshape(N)).astype(np.float32)

    geomS = np.stack([np.ones(N, np.float32), -2.0*x, -2.0*y,
                      (x*x + y*y)]).astype(np.float32)
    geomR = np.stack([(x*x + y*y), x, y,
                      np.ones(N, np.float32)]).astype(np.float32)
    scat_t = scat.reshape(NB, 128).T.copy()

    bpack = np.zeros((N, RW), np.float32)
    bpack[:, 0:40] = -einc.real; bpack[:, 40:80] = -gfs.real
    bpack[:, 128:168] = -einc.imag; bpack[:, 168:208] = -gfs.imag
    gscT = np.concatenate([gsc.real.T, gsc.imag.T], axis=1).astype(np.float32)
    dfpack = np.concatenate([df.real, df.imag], axis=1).astype(np.float32)
    tpT = tp.T.copy().astype(np.float32)

    groups = []
    i = 0
    while i < len(links):
        t = int(links[i, 0])
        rs_list = []
        while i < len(links) and int(links[i, 0]) == t:
            rs_list.append(int(links[i, 1]))
            i += 1
        groups.append((t, rs_list))

    _GBASE.clear()
    o = 0
    for (t, rs_list) in groups:
        _GBASE[t] = o
        o += len(rs_list)
    key = (hash(links.tobytes()), alpha_f)
    if key not in _CACHED:
        _CACHED[key] = build_program(groups, alpha_f)
    nc = _CACHED[key]

    id128 = np.eye(128, dtype=np.float32)
    im = {
        "geomS": geomS, "geomR": geomR, "scat_t": scat_t, "bpack": bpack,
        "gscT": gscT, "dfpack": dfpack, "tpT": tpT,
        "id128": id128, "idu8": id128.astype(np.uint8),
    }
    import os as _os
    _tr = _os.environ.get("KTRACE", "0") == "1"
    res = run_bass_kernel_spmd(nc, [im] * 8, core_ids=list(range(8)), trace=_tr)
    out = res.results[0]
    _CACHED["last"] = (res, out)

    chi = out["out_chi"]
    dchi_r = chi[:N].reshape(M, M).T
    dchi_i = chi[N:].reshape(M, M).T
    chi_new = (chi_it + (dchi_r + 1j * dchi_i)).astype(np.complex64)
    return chi_new + 1.0, chi_new



# revision 28
# speedup vs baseline: 1.3862x; 1.3862x over previous
"""DRIM layer (distorted Rytov inverse-scattering iteration) on Trainium2.

One Bass/Tile program per core (replicated SPMD on 8 cores):
  P1  Z-matrix build via large-branch Hankel evaluation (upper triangle only;
      Z is complex-symmetric), resident in SBUF as fp32r planes
  P2  block LDL^T elimination, Newton-iterated 128x128 block inverses,
      fp32r tensor-engine matmuls
  P3  back-substitution -> X = Z^-1 [-E_inc | -G]
  P4  total field, RSS power model, data vector
  P5  Rytov H^T rows (4608 x 1664 padded) + H^T d
  P7  Gram H H^T (upper blocks) + Jacobi scaling
  P8  scaled SPD block solve (same Newton machinery, real)
  P9  chi = H^T y, output dchi

Host does input packing / output reshape only.
"""
import math
import os
import numpy as np

import concourse.bass as bass
import concourse.bacc as bacc
import concourse.bass_isa as bass_isa
import concourse.mybir as mybir
import concourse.tile as tile
from concourse.bass_utils import run_bass_kernel_spmd

F32 = mybir.dt.float32
F32R = mybir.dt.float32r
U8 = mybir.dt.uint8
AF = mybir.ActivationFunctionType
ALU = mybir.AluOpType
AXX = mybir.AxisListType.X

M = 48
N = M * M
NB = N // 128               # 18
TX = RX = 40
NL = TX * (RX - 1)          # 1560
LPAD = 1664
LB = LPAD // 128            # 13
RW = 256                    # [0:128]=Re plane, [128:256]=Im plane
CW = 256                    # Z-build column chunk
DOI = 3.0
WL = 0.125
K0 = 2.0 * math.pi / WL
IMP = 120.0 * math.pi
GRID_LEN = DOI / M
GRID_RADIUS = math.sqrt(GRID_LEN ** 2 / math.pi)
NOISE = 1e-6

def _j1s(x):
    t2 = (x / 3.0) ** 2
    return x * (0.5 - 0.56249985*t2 + 0.21093573*t2**2 - 0.03954289*t2**3
                + 0.00443319*t2**4 - 0.00031761*t2**5 + 0.00001109*t2**6)

def _y1s(x):
    t2 = (x / 3.0) ** 2
    p = (-0.6366198 + 0.2212091*t2 + 2.1682709*t2**2 - 1.3164827*t2**3
         + 0.3123951*t2**4 - 0.0400976*t2**5 + 0.0027873*t2**6)
    return ((2.0/math.pi) * x * math.log(0.5*x) * _j1s(x) + p) / x

X0C = K0 * GRID_RADIUS
GRID_AREA = 4.0*math.pi*GRID_RADIUS/(2.0*K0) * _j1s(X0C)
C1 = -IMP * math.pi * GRID_RADIUS / 2.0
C2 = _j1s(X0C)
C3R, C3I = _j1s(X0C), _y1s(X0C)
C1C2 = C1 * C2
ZD_RE = C1 * C3R
ZD_IM_C = C1 * C3I
SA = GRID_AREA * K0 * K0
TWO_PI = 2.0 * math.pi
INV_2PI = 1.0 / TWO_PI
LOG10E20 = 20.0 * math.log10(math.e)
CADD = 10.0 * math.log10(WL * WL / (4.0 * math.pi * IMP) / 1e-3)
C20L = 20.0 / math.log(10.0)

F0C = [0.79788456, -0.00000077, -0.00552740, -0.00009512,
       0.00137237, -0.00072805, 0.00014476]
THC = [-0.78539816, -0.04166397, -0.00003954, 0.00262573,
       -0.00054125, -0.00029333, 0.00013558]
F0CS = [c * (3.0 ** k) * C1C2 for k, c in enumerate(F0C)]
THCS = [c * (3.0 ** k) for k, c in enumerate(THC)]
# short-series Z build: th = x - pi/4 + 3*THC[1]*rx ; amp = C1C2*(F0C0 + 3*F0C1*rx)/sqrt(x)
TH1 = 3.0 * THC[1]
A0C = C1C2 * F0C[0]
A1C = 3.0 * C1C2 * F0C[1]
K0K0 = K0 * K0
PI4 = math.pi / 4.0

NEWTON_Z = 14
NEWTON_SPD = 10


def _horner(nc, out_ap, s_ap, coeffs):
    cs = coeffs[::-1]
    nc.vector.tensor_scalar(out=out_ap, in0=s_ap, scalar1=float(cs[0]),
                            scalar2=float(cs[1]), op0=ALU.mult, op1=ALU.add)
    for c in cs[2:]:
        nc.vector.tensor_tensor(out=out_ap, in0=out_ap, in1=s_ap, op=ALU.mult)
        nc.vector.tensor_scalar(out=out_ap, in0=out_ap, scalar1=float(c),
                                scalar2=None, op0=ALU.add)


def _cmm(nc, pool, lhsT, rhs, n=RW):
    P1 = pool.tile([128, n], F32, tag="cmmp1")
    P2 = pool.tile([128, n], F32, tag="cmmp2")
    nc.tensor.matmul(P1[:], lhsT[:, 0:128], rhs, start=True, stop=True)
    nc.tensor.matmul(P2[:], lhsT[:, 128:256], rhs, start=True, stop=True)
    return P1, P2


def _combine_sub(nc, dst, P1, P2):
    nc.vector.tensor_tensor(out=dst[:, 0:256], in0=dst[:, 0:256],
                            in1=P1[:, 0:256], op=ALU.subtract)
    nc.vector.tensor_tensor(out=dst[:, 0:128], in0=dst[:, 0:128],
                            in1=P2[:, 128:256], op=ALU.add)
    nc.vector.tensor_tensor(out=dst[:, 128:256], in0=dst[:, 128:256],
                            in1=P2[:, 0:128], op=ALU.subtract)


def _combine_set(nc, dst, P1, P2):
    nc.vector.tensor_copy(dst[:, 0:256], P1[:, 0:256])
    nc.vector.tensor_tensor(out=dst[:, 0:128], in0=dst[:, 0:128],
                            in1=P2[:, 128:256], op=ALU.subtract)
    nc.vector.tensor_tensor(out=dst[:, 128:256], in0=dst[:, 128:256],
                            in1=P2[:, 0:128], op=ALU.add)


def _newton_scale(nc, work, pmisc, m, tag):
    """1/(colmax * rowmax) of m [128,128] -> [128,1] fp32 AP."""
    ones = work.tile([128, 1], F32, tag=f"nwo_{tag}")
    nc.vector.memset(ones[:], 1.0)
    pc = pmisc.tile([128, 1], F32, tag=f"nwpc_{tag}")
    nc.tensor.matmul(pc[:], m[:], ones[:], start=True, stop=True)
    pr = pmisc.tile([1, 128], F32, tag=f"nwpr_{tag}")
    nc.tensor.matmul(pr[:], ones[:], m[:], start=True, stop=True)
    cs = work.tile([128, 1], F32, tag=f"nwcs_{tag}")
    nc.vector.tensor_copy(cs[:], pc[:])
    rs = work.tile([1, 128], F32, tag=f"nwrs_{tag}")
    nc.vector.tensor_copy(rs[:], pr[:])
    nc.gpsimd.partition_all_reduce(cs[:], cs[:], 128, bass_isa.ReduceOp.max)
    rmax = work.tile([1, 1], F32, tag=f"nwrm_{tag}")
    nc.vector.tensor_reduce(rmax[:], rs[:], axis=AXX, op=ALU.max)
    rmax_b = work.tile([128, 1], F32, tag=f"nwrb_{tag}")
    nc.gpsimd.partition_broadcast(rmax_b[:], rmax[:])
    a = work.tile([128, 1], F32, tag=f"nwa_{tag}")
    nc.vector.tensor_tensor(out=a[:], in0=cs[:], in1=rmax_b[:], op=ALU.mult)
    nc.vector.reciprocal(a[:], a[:])
    return a


def _newton_cplx(nc, work, pmm, pmisc, D, Xout, id2_s, iters):
    """Swap-free Newton: maintains X=[Xr|Xi] and Xs=[-Xi|Xr].
    D@X via lhsT=Dr,rhs=X + lhsT=Di,rhs=Xs (both width-256, PSUM acc).
    """
    m = work.tile([128, 128], F32, tag="nw_m")
    m2 = work.tile([128, 128], F32, tag="nw_m2")
    nc.scalar.activation(m[:], D[:, 0:128], AF.Abs)
    nc.scalar.activation(m2[:], D[:, 128:256], AF.Abs)
    nc.vector.tensor_tensor(out=m[:], in0=m[:], in1=m2[:], op=ALU.max)
    a = _newton_scale(nc, work, pmisc, m, "c")
    nc.vector.tensor_scalar(out=Xout[:, 0:128], in0=D[:, 0:128], scalar1=a[:],
                            scalar2=None, op0=ALU.mult)
    Xs = work.tile([128, RW], F32R, tag="nw_Xs")
    nc.vector.tensor_scalar(out=Xs[:, 0:128], in0=D[:, 128:256], scalar1=a[:],
                            scalar2=None, op0=ALU.mult)
    # Xi = -Di*a = -(Xs lo)
    nc.vector.tensor_scalar(out=Xout[:, 128:256], in0=Xs[:, 0:128],
                            scalar1=-1.0, scalar2=None, op0=ALU.mult)
    nc.scalar.copy(Xs[:, 128:256], Xout[:, 0:128])
    R = work.tile([128, RW], F32R, tag="nw_R")
    Rs = work.tile([128, RW], F32R, tag="nw_Rs")
    for _ in range(iters):
        P = pmm.tile([128, RW], F32, tag="cmmp1")
        nc.tensor.matmul(P[:], D[:, 0:128], Xout[:, 0:RW], start=True,
                         stop=False)
        nc.tensor.matmul(P[:], D[:, 128:256], Xs[:, 0:RW], start=False,
                         stop=True)
        # R = [I|0] - P ;  Rs = [-Ri|Rr] = [Pi | Rr]
        nc.vector.tensor_tensor(out=R[:], in0=id2_s[:], in1=P[:],
                                op=ALU.subtract)
        nc.scalar.copy(Rs[:, 0:128], P[:, 128:256])
        nc.scalar.copy(Rs[:, 128:256], R[:, 0:128])
        Q = pmm.tile([128, RW], F32, tag="cmmp2")
        nc.tensor.matmul(Q[:], Xout[:, 0:128], R[:, 0:RW], start=True,
                         stop=False)
        nc.tensor.matmul(Q[:], Xout[:, 128:256], Rs[:, 0:RW], start=False,
                         stop=True)
        # X += Q ; Xs_lo -= Qi ; Xs_hi = new Xr
        nc.vector.tensor_tensor(out=Xout[:, 0:256], in0=Xout[:, 0:256],
                                in1=Q[:, 0:256], op=ALU.add)
        nc.vector.tensor_tensor(out=Xs[:, 0:128], in0=Xs[:, 0:128],
                                in1=Q[:, 128:256], op=ALU.subtract)
        nc.scalar.copy(Xs[:, 128:256], Xout[:, 0:128])


def _newton_real(nc, work, pmm, pmisc, D, Xout, id_s, iters):
    m = work.tile([128, 128], F32, tag="nw_m")
    nc.scalar.activation(m[:], D[:], AF.Abs)
    a = _newton_scale(nc, work, pmisc, m, "r")
    nc.vector.tensor_scalar(out=Xout[:], in0=D[:], scalar1=a[:], scalar2=None,
                            op0=ALU.mult)
    R = work.tile([128, 128], F32R, tag="nw_R")
    for _ in range(iters):
        P1 = pmm.tile([128, 128], F32, tag="cmmp1")
        nc.tensor.matmul(P1[:], D[:], Xout[:], start=True, stop=True)
        nc.vector.tensor_tensor(out=R[:], in0=id_s[:], in1=P1[:],
                                op=ALU.subtract)
        Q1 = pmm.tile([128, 128], F32, tag="cmmp2")
        nc.tensor.matmul(Q1[:], Xout[:], R[:], start=True, stop=True)
        nc.vector.tensor_tensor(out=Xout[:], in0=Xout[:], in1=Q1[:], op=ALU.add)


def build_program(link_groups, alpha):
    nc = bacc.Bacc("TRN2", target_bir_lowering=False, num_devices=8)
    din = {}
    def inp(name, shape, dtype=F32):
        din[name] = nc.dram_tensor(name, shape, dtype, kind="ExternalInput")
    inp("geomS", [4, N]); inp("geomR", [4, N]); inp("scat_t", [128, NB])
    inp("bpack", [N, RW]); inp("gscT", [N, 80]); inp("dfpack", [40, 80])
    inp("tpT", [40, RX - 1]); inp("id128", [128, 128]); inp("idu8", [128, 128], U8)
    out_chi = nc.dram_tensor("out_chi", [2 * N], F32, kind="ExternalOutput")
    xdbg = nc.dram_tensor("xdbg", [N, RW], F32, kind="ExternalOutput")
    tfdbg = nc.dram_tensor("tfdbg", [40, 80], F32, kind="ExternalOutput")
    ddbg = nc.dram_tensor("ddbg", [40, RX - 1], F32, kind="ExternalOutput")
    scr = {}
    scr["vdram"] = nc.dram_tensor("vdram", [NB * 128, RW], F32R, kind="Internal")
    scr["utdram"] = nc.dram_tensor("utdram", [N, 2 * N], F32R, kind="Internal")
    scr["htdram"] = nc.dram_tensor("htdram", [2 * N, LPAD], F32R, kind="Internal")
    scr["gramdram"] = nc.dram_tensor("gramdram", [LPAD, LPAD], F32, kind="Internal")
    scr["v2dram"] = nc.dram_tensor("v2dram", [LB * 128, 128], F32R, kind="Internal")
    scr["ut2dram"] = nc.dram_tensor("ut2dram", [LPAD, LPAD], F32R, kind="Internal")
    scr["sdram"] = nc.dram_tensor("sdram", [NL], F32, kind="Internal")
    scr["wdram"] = nc.dram_tensor("wdram", [2 * NL], F32, kind="Internal")
    scr["srowdram"] = nc.dram_tensor("srowdram", [LPAD], F32, kind="Internal")
    scr["yrowdram"] = nc.dram_tensor("yrowdram", [LPAD], F32, kind="Internal")

    with tile.TileContext(nc) as tc:
        _body(nc, tc, din, out_chi, xdbg, tfdbg, ddbg, scr, link_groups, alpha)
    nc.compile()
    return nc


def _body(nc, tc, din, out_chi, xdbg, tfdbg, ddbg, scr, link_groups, alpha):
    import contextlib
    ctx = contextlib.ExitStack()
    consts = ctx.enter_context(tc.tile_pool(name="consts", bufs=1))
    id_s = consts.tile([128, 128], F32)
    nc.sync.dma_start(id_s[:], din["id128"][:])
    idr_s = consts.tile([128, 128], F32R)
    nc.vector.tensor_copy(idr_s[:], id_s[:])
    idu_s = consts.tile([128, 128], U8)
    nc.sync.dma_start(idu_s[:], din["idu8"][:])
    id2_s = consts.tile([128, RW], F32)
    nc.vector.memset(id2_s[:], 0.0)
    nc.vector.tensor_copy(id2_s[:, 0:128], id_s[:])
    scat_s = consts.tile([128, NB], F32)
    nc.sync.dma_start(scat_s[:], din["scat_t"][:])

    zdi_s = consts.tile([128, NB], F32)
    fsc_s = consts.tile([128, NB], F32)
    t0 = consts.tile([128, NB], F32)
    nc.vector.tensor_scalar(out=t0[:], in0=scat_s[:], scalar1=-1.0,
                            scalar2=None, op0=ALU.add)
    nc.vector.reciprocal(t0[:], t0[:])
    nc.vector.tensor_scalar(out=fsc_s[:], in0=t0[:], scalar1=(IMP / K0),
                            scalar2=None, op0=ALU.mult)
    nc.vector.tensor_tensor(out=t0[:], in0=t0[:], in1=scat_s[:], op=ALU.mult)
    nc.vector.tensor_scalar(out=zdi_s[:], in0=t0[:], scalar1=-(IMP / K0),
                            scalar2=ZD_IM_C, op0=ALU.mult, op1=ALU.add)
    zdr_c = consts.tile([128, 1], F32)
    nc.vector.memset(zdr_c[:], float(ZD_RE))

    bf_pool = ctx.enter_context(tc.tile_pool(name="bf", bufs=1))
    BF = [bf_pool.tile([128, RW], F32R, tag=f"bf{i}", name=f"bf{i}") for i in range(NB)]

    with tc.tile_pool(name="tri", bufs=1) as tri:
        ZT = {}
        for i in range(NB):
            for j in range(i, NB):
                ZT[(i, j)] = tri.tile([128, RW], F32R, tag=f"z{i}_{j}", name=f"z{i}_{j}")

        # ---------------- P1: Z build ----------------
        with (
            tc.tile_pool(name="zb_geom", bufs=2) as gpool,
            tc.tile_pool(name="zb_work", bufs=1) as work,
            tc.tile_pool(name="zb_psum", bufs=2, space="PSUM") as pz,
        ):
            for k in range(NB):
                r0 = 128 * k
                gS = gpool.tile([4, 128], F32, tag="gS", name="gS")
                nc.sync.dma_start(gS[:], din["geomS"][:, r0:r0+128])
                j = k
                while j < NB:
                    c0 = 128 * j
                    w = 256 if j + 1 < NB else 128
                    gR = work.tile([4, CW], F32, tag="gR", name="gR")
                    nc.sync.dma_start(gR[:, 0:w], din["geomR"][:, c0:c0+w])
                    # one 128-col block per chunk (CW=256 covers Re|Im writes)
                    pd = pz.tile([128, CW], F32, tag="zb_pd")
                    nc.tensor.matmul(pd[:, 0:w], gS[:], gR[:, 0:w],
                                     start=True, stop=True)
                    dsq = work.tile([128, CW], F32, tag="zb_dsq")
                    nc.vector.tensor_scalar(out=dsq[:, 0:w], in0=pd[:, 0:w],
                                            scalar1=0.002, scalar2=None,
                                            op0=ALU.max)
                    x = work.tile([128, CW], F32, tag="zb_x")
                    nc.scalar.activation(x[:, 0:w], dsq[:, 0:w], AF.Sqrt,
                                         scale=float(K0 * K0))
                    sp = work.tile([128, CW], F32, tag="zb_sp")
                    nc.vector.reciprocal(sp[:, 0:w], x[:, 0:w])
                    f0 = work.tile([128, CW], F32, tag="zb_f0")
                    _horner(nc, f0[:, 0:w], sp[:, 0:w], F0CS)
                    th = work.tile([128, CW], F32, tag="zb_th")
                    _horner(nc, th[:, 0:w], sp[:, 0:w], THCS)
                    nc.vector.tensor_tensor(out=th[:, 0:w], in0=th[:, 0:w],
                                            in1=x[:, 0:w], op=ALU.add)
                    nc.scalar.activation(x[:, 0:w], sp[:, 0:w], AF.Sqrt)
                    nc.vector.tensor_tensor(out=f0[:, 0:w], in0=f0[:, 0:w],
                                            in1=x[:, 0:w], op=ALU.mult)
                    u = work.tile([128, CW], F32, tag="zb_u")
                    nc.vector.tensor_scalar(out=u[:, 0:w], in0=th[:, 0:w],
                                            scalar1=INV_2PI, scalar2=None,
                                            op0=ALU.mult)
                    ki = work.tile([128, CW], mybir.dt.int32, tag="zb_ki")
                    nc.vector.tensor_copy(ki[:, 0:w], u[:, 0:w])
                    mf = work.tile([128, CW], F32, tag="zb_mf")
                    nc.vector.tensor_copy(mf[:, 0:w], ki[:, 0:w])
                    r1 = work.tile([128, CW], F32, tag="zb_r1")
                    nc.vector.tensor_scalar(out=r1[:, 0:w], in0=mf[:, 0:w],
                                            scalar1=-TWO_PI, scalar2=None,
                                            op0=ALU.mult)
                    nc.vector.tensor_tensor(out=r1[:, 0:w], in0=r1[:, 0:w],
                                            in1=th[:, 0:w], op=ALU.add)
                    sinr = work.tile([128, CW], F32, tag="zb_sin")
                    nc.scalar.activation(sinr[:, 0:w], r1[:, 0:w], AF.Sin)
                    nc.vector.tensor_scalar(out=u[:, 0:w], in0=u[:, 0:w],
                                            scalar1=0.25, scalar2=None, op0=ALU.add)
                    nc.vector.tensor_copy(ki[:, 0:w], u[:, 0:w])
                    nc.vector.tensor_copy(mf[:, 0:w], ki[:, 0:w])
                    nc.vector.tensor_scalar(out=mf[:, 0:w], in0=mf[:, 0:w],
                                            scalar1=-TWO_PI,
                                            scalar2=(math.pi / 2.0),
                                            op0=ALU.mult, op1=ALU.add)
                    nc.vector.tensor_tensor(out=mf[:, 0:w], in0=mf[:, 0:w],
                                            in1=th[:, 0:w], op=ALU.add)
                    cosr = work.tile([128, CW], F32, tag="zb_cos")
                    nc.scalar.activation(cosr[:, 0:w], mf[:, 0:w], AF.Sin)
                    nc.vector.tensor_tensor(out=cosr[:, 0:w], in0=cosr[:, 0:w],
                                            in1=f0[:, 0:w], op=ALU.mult)
                    nc.vector.tensor_tensor(out=sinr[:, 0:w], in0=sinr[:, 0:w],
                                            in1=f0[:, 0:w], op=ALU.mult)
                    if j == k:
                        nc.vector.copy_predicated(
                            cosr[:, 0:128], idu_s[:],
                            zdr_c[:].broadcast_to([128, 128]))
                        nc.vector.copy_predicated(
                            sinr[:, 0:128], idu_s[:],
                            zdi_s[:, k:k+1].broadcast_to([128, 128]))
                    for b in range(w // 128):
                        nc.vector.tensor_copy(ZT[(k, j + b)][:, 0:128],
                                              cosr[:, 128*b:128*b+128])
                        nc.vector.tensor_copy(ZT[(k, j + b)][:, 128:256],
                                              sinr[:, 128*b:128*b+128])
                    j += w // 128

        # ---------------- P2: block LDL^T ----------------
        with (
            tc.tile_pool(name="lu_big", bufs=1) as work,
            tc.tile_pool(name="lu_sm", bufs=2) as wsm,
            tc.tile_pool(name="lu_pmm", bufs=1, space="PSUM") as pmm,
            tc.tile_pool(name="lu_pmisc", bufs=1, space="PSUM") as pmisc,
        ):
            ldtmp0 = wsm.tile([128, RW], F32, tag="ldtmp")
            for i in range(NB):
                nc.sync.dma_start(ldtmp0[:], din["bpack"][128*i:128*(i+1), :])
                nc.vector.tensor_copy(BF[i][:], ldtmp0[:])
                ldtmp0 = wsm.tile([128, RW], F32, tag="ldtmp")
            IC = 3
            for k in range(NB):
                V = work.tile([128, RW], F32R, tag="lu_V")
                _newton_cplx(nc, work, pmm, pmisc, ZT[(k, k)], V, id2_s,
                             NEWTON_Z)
                nc.sync.dma_start(scr["vdram"][128*k:128*(k+1), :], V[:])
                for i in range(k + 1, NB):
                    ptr = pmisc.tile([128, 128], F32R, tag="lu_ptr")
                    nc.tensor.transpose(ptr[:], ZT[(k, i)][:, 0:128], idr_s[:])
                    utt = wsm.tile([128, RW], F32R, tag="lu_utt")
                    nc.vector.tensor_copy(utt[:, 0:128], ptr[:])
                    pti = pmisc.tile([128, 128], F32R, tag="lu_pti")
                    nc.tensor.transpose(pti[:], ZT[(k, i)][:, 128:256], idr_s[:])
                    nc.vector.tensor_copy(utt[:, 128:256], pti[:])
                    nc.sync.dma_start(
                        scr["utdram"][128*i:128*(i+1), 256*k:256*(k+1)], utt[:])
                if k == NB - 1:
                    continue
                bswap = work.tile([128, RW], F32R, tag="lu_bs")
                nc.vector.tensor_scalar(out=bswap[:, 0:128],
                                        in0=BF[k][:, 128:256], scalar1=-1.0,
                                        scalar2=None, op0=ALU.mult)
                nc.scalar.copy(bswap[:, 128:256], BF[k][:, 0:128])
                for a in range(k + 1, NB, IC):
                    b = min(a + IC, NB)
                    LTs = {}
                    for j in range(a, NB):
                        zsw = wsm.tile([128, RW], F32R, tag="lu_zsw")
                        nc.vector.tensor_scalar(out=zsw[:, 0:128],
                                                in0=ZT[(k, j)][:, 128:256],
                                                scalar1=-1.0, scalar2=None,
                                                op0=ALU.mult)
                        nc.scalar.copy(zsw[:, 128:256], ZT[(k, j)][:, 0:128])
                        if j < b:
                            pl = pmm.tile([128, RW], F32, tag="cmmp1")
                            nc.tensor.matmul(pl[:], V[:, 0:128],
                                             ZT[(k, j)][:, 0:RW],
                                             start=True, stop=False)
                            nc.tensor.matmul(pl[:], V[:, 128:256],
                                             zsw[:, 0:RW],
                                             start=False, stop=True)
                            LT = work.tile([128, RW], F32R,
                                           tag=f"lu_LT{j - a}",
                                           name=f"lu_LT{j - a}")
                            nc.vector.tensor_copy(LT[:], pl[:])
                            LTs[j] = LT
                            pb = pmm.tile([128, RW], F32, tag="cmmp2")
                            nc.tensor.matmul(pb[:], LT[:, 0:128],
                                             BF[k][:, 0:RW],
                                             start=True, stop=False)
                            nc.tensor.matmul(pb[:], LT[:, 128:256],
                                             bswap[:, 0:RW],
                                             start=False, stop=True)
                            nc.vector.tensor_tensor(out=BF[j][:, 0:256],
                                                    in0=BF[j][:, 0:256],
                                                    in1=pb[:, 0:256],
                                                    op=ALU.subtract)
                        for i in range(a, min(b, j + 1)):
                            pu = pmm.tile([128, RW], F32,
                                          tag=f"updp{(i - a) % 2}",
                                          name=f"updp{(i - a) % 2}")
                            nc.tensor.matmul(pu[:], LTs[i][:, 0:128],
                                             ZT[(k, j)][:, 0:RW],
                                             start=True, stop=False)
                            nc.tensor.matmul(pu[:], LTs[i][:, 128:256],
                                             zsw[:, 0:RW],
                                             start=False, stop=True)
                            nc.vector.tensor_tensor(out=ZT[(i, j)][:, 0:256],
                                                    in0=ZT[(i, j)][:, 0:256],
                                                    in1=pu[:, 0:256],
                                                    op=ALU.subtract)

    # ---------------- P3: back-substitution ----------------
    with (
        tc.tile_pool(name="bs_work", bufs=3) as work,
        tc.tile_pool(name="bs_pacc", bufs=1, space="PSUM") as pacc,
        tc.tile_pool(name="bs_pmm", bufs=2, space="PSUM") as pmm,
    ):
        for k in range(NB - 1, -1, -1):
            W = work.tile([128, RW], F32R, tag="bs_W")
            nc.vector.tensor_copy(W[:], BF[k][:])
            if k < NB - 1:
                P1 = pacc.tile([128, RW], F32, tag="bs_p1")
                P2 = pacc.tile([128, RW], F32, tag="bs_p2")
                for idx, j in enumerate(range(k + 1, NB)):
                    utt = work.tile([128, RW], F32R, tag="bs_utt")
                    nc.sync.dma_start(
                        utt[:], scr["utdram"][128*j:128*(j+1), 256*k:256*(k+1)])
                    st = (idx == 0); sp_ = (j == NB - 1)
                    nc.tensor.matmul(P1[:], utt[:, 0:128], BF[j][:, 0:RW],
                                     start=st, stop=sp_)
                    nc.tensor.matmul(P2[:], utt[:, 128:256], BF[j][:, 0:RW],
                                     start=st, stop=sp_)
                _combine_sub(nc, W, P1, P2)
            Vk = work.tile([128, RW], F32R, tag="bs_V")
            nc.sync.dma_start(Vk[:], scr["vdram"][128*k:128*(k+1), :])
            P1, P2 = _cmm(nc, pmm, Vk, W[:, 0:RW])
            _combine_set(nc, BF[k], P1, P2)
            nc.sync.dma_start(xdbg[128*k:128*(k+1), :], BF[k][:].bitcast(F32))

    # ---------------- P4: tf + data vector ----------------
    late = ctx.enter_context(tc.tile_pool(name="late", bufs=1))
    dvec = late.tile([128, LB], F32)
    drep = late.tile([128, LPAD], F32)
    wrep_r = late.tile([128, NL], F32)
    wrep_i = late.tile([128, NL], F32)
    with (
        tc.tile_pool(name="p4_work", bufs=2) as work,
        tc.tile_pool(name="p4_pacc", bufs=1, space="PSUM") as pacc,
        tc.tile_pool(name="p4_pmisc", bufs=1, space="PSUM") as pmisc,
    ):
        Ptf1 = pacc.tile([40, RW], F32, tag="tf_p1")
        Ptf2 = pacc.tile([40, RW], F32, tag="tf_p2")
        for i in range(NB):
            gt = work.tile([128, 80], F32, tag="tf_g")
            nc.sync.dma_start(gt[:], din["gscT"][128*i:128*(i+1), :])
            gtr = work.tile([128, 80], F32R, tag="tf_gr")
            nc.vector.tensor_copy(gtr[:], gt[:])
            st = (i == 0); sp_ = (i == NB - 1)
            nc.tensor.matmul(Ptf1[:], gtr[:, 0:40], BF[i][:, 0:RW],
                             start=st, stop=sp_)
            nc.tensor.matmul(Ptf2[:], gtr[:, 40:80], BF[i][:, 0:RW],
                             start=st, stop=sp_)
        df = work.tile([40, 80], F32, tag="tf_df")
        nc.sync.dma_start(df[:], din["dfpack"][:])
        tfr = work.tile([40, 40], F32, tag="tfr")
        tfi = work.tile([40, 40], F32, tag="tfi")
        nc.vector.tensor_tensor(out=tfr[:], in0=df[:, 0:40],
                                in1=Ptf1[:, 0:40], op=ALU.add)
        nc.vector.tensor_tensor(out=tfr[:], in0=tfr[:],
                                in1=Ptf2[:, 128:168], op=ALU.subtract)
        nc.vector.tensor_tensor(out=tfi[:], in0=df[:, 40:80],
                                in1=Ptf1[:, 128:168], op=ALU.add)
        nc.vector.tensor_tensor(out=tfi[:], in0=tfi[:],
                                in1=Ptf2[:, 0:40], op=ALU.add)
        tfd = work.tile([40, 80], F32, tag="tf_out")
        nc.vector.tensor_copy(tfd[:, 0:40], tfr[:])
        nc.vector.tensor_copy(tfd[:, 40:80], tfi[:])
        nc.sync.dma_start(tfdbg[:], tfd[:])

        pw = work.tile([40, 40], F32, tag="pw")
        nc.vector.tensor_tensor(out=pw[:], in0=tfr[:], in1=tfr[:], op=ALU.mult)
        t1 = work.tile([40, 40], F32, tag="pw_t")
        nc.vector.tensor_tensor(out=t1[:], in0=tfi[:], in1=tfi[:], op=ALU.mult)
        nc.vector.tensor_tensor(out=pw[:], in0=pw[:], in1=t1[:], op=ALU.add)
        amp = work.tile([40, 40], F32, tag="amp")
        nc.scalar.activation(amp[:], pw[:], AF.Sqrt)
        nc.vector.tensor_scalar(out=amp[:], in0=amp[:], scalar1=NOISE,
                                scalar2=None, op0=ALU.add)
        nc.scalar.activation(amp[:], amp[:], AF.Ln)
        tpi = work.tile([40, 40], F32, tag="tpi")
        nc.vector.tensor_scalar(out=tpi[:], in0=amp[:], scalar1=C20L,
                                scalar2=CADD, op0=ALU.mult, op1=ALU.add)
        rec = work.tile([40, 40], F32, tag="rec")
        nc.vector.reciprocal(rec[:], pw[:])
        wr = work.tile([40, 40], F32, tag="wr")
        nc.vector.tensor_tensor(out=wr[:], in0=tfr[:], in1=rec[:], op=ALU.mult)
        nc.vector.tensor_scalar(out=wr[:], in0=wr[:], scalar1=SA, scalar2=None,
                                op0=ALU.mult)
        wi = work.tile([40, 40], F32, tag="wi")
        nc.vector.tensor_tensor(out=wi[:], in0=tfi[:], in1=rec[:], op=ALU.mult)
        nc.vector.tensor_scalar(out=wi[:], in0=wi[:], scalar1=-SA, scalar2=None,
                                op0=ALU.mult)

        def t40(src, name):
            pt = pmisc.tile([40, 40], F32, tag=f"t40p_{name}")
            nc.tensor.matmul(pt[:], src[:], id_s[0:40, 0:40], start=True,
                             stop=True)
            d = work.tile([40, 40], F32, tag=f"t40_{name}")
            nc.vector.tensor_copy(d[:], pt[:])
            return d
        tpiT = t40(tpi, "tpi"); wrT = t40(wr, "wr"); wiT = t40(wi, "wi")

        pack = work.tile([40, 120], F32, tag="pack")
        nc.vector.tensor_copy(pack[:, 0:40], tpiT[:])
        nc.vector.tensor_copy(pack[:, 40:80], wrT[:])
        nc.vector.tensor_copy(pack[:, 80:120], wiT[:])
        kept3 = work.tile([1, 3 * NL], F32, tag="kept3")
        pack3d = pack[:].rearrange("p (a b) -> p a b", a=3)
        kept3d = kept3[:].rearrange("p (a b) -> p a b", a=3)
        for (t, rs_list) in link_groups:
            o = _GBASE[t]
            for (s0, ln) in _contig_segments(rs_list):
                nc.sync.dma_start(kept3d[0:1, :, o:o+ln],
                                  pack3d[t:t+1, :, s0:s0+ln])
                o += ln
        # data = (tpT - tpi_kept)/LOG10E20 on the packed row
        tprow = work.tile([1, NL], F32, tag="tprow")
        nc.sync.dma_start(tprow[:], bass.AP(din["tpT"], 0, [[1, NL]]))
        nc.vector.tensor_tensor(out=kept3[0:1, 0:NL], in0=tprow[:],
                                in1=kept3[0:1, 0:NL], op=ALU.subtract)
        nc.vector.tensor_scalar(out=kept3[0:1, 0:NL], in0=kept3[0:1, 0:NL],
                                scalar1=1.0 / LOG10E20, scalar2=None,
                                op0=ALU.mult)
        nc.sync.dma_start(bass.AP(ddbg, 0, [[1, NL]]), kept3[0:1, 0:NL])
        nc.sync.dma_start(bass.AP(scr["sdram"], 0, [[1, NL]]), kept3[0:1, 0:NL])

        nc.vector.memset(dvec[:], 0.0)
        nc.sync.dma_start(dvec[:, 0:12],
                          bass.AP(scr["sdram"], 0, [[1, 128], [128, 12]]))
        nc.sync.dma_start(dvec[0:24, 12:13],
                          bass.AP(scr["sdram"], 1536, [[1, 24]]))
        nc.vector.memset(drep[:], 0.0)
        nc.gpsimd.partition_broadcast(drep[:, 0:NL], kept3[0:1, 0:NL])
        nc.gpsimd.partition_broadcast(wrep_r[:], kept3[0:1, NL:2*NL])
        nc.gpsimd.partition_broadcast(wrep_i[:], kept3[0:1, 2*NL:3*NL])

    # ---------------- P5: Ht build + v = Ht d ----------------
    vsum = late.tile([128, 2 * NB], F32)
    lam = late.tile([128, 1], F32)
    with tc.tile_pool(name="p5_work", bufs=2) as work:
        nc.vector.memset(vsum[:], 0.0)
        for i in range(NB):
            Gq = work.tile([128, 80], F32, tag="h_gq")
            Iq = work.tile([128, 80], F32, tag="h_iq")
            f_ap = fsc_s[:, i:i+1]
            nc.vector.tensor_scalar(out=Gq[:, 0:40], in0=BF[i][:, 168:208],
                                    scalar1=f_ap, scalar2=None, op0=ALU.mult)
            nc.vector.tensor_scalar(out=Gq[:, 0:40], in0=Gq[:, 0:40],
                                    scalar1=-1.0, scalar2=None, op0=ALU.mult)
            nc.vector.tensor_scalar(out=Gq[:, 40:80], in0=BF[i][:, 40:80],
                                    scalar1=f_ap, scalar2=None, op0=ALU.mult)
            nc.vector.tensor_scalar(out=Iq[:, 0:40], in0=BF[i][:, 128:168],
                                    scalar1=f_ap, scalar2=None, op0=ALU.mult)
            nc.vector.tensor_scalar(out=Iq[:, 0:40], in0=Iq[:, 0:40],
                                    scalar1=-1.0, scalar2=None, op0=ALU.mult)
            nc.vector.tensor_scalar(out=Iq[:, 40:80], in0=BF[i][:, 0:40],
                                    scalar1=f_ap, scalar2=None, op0=ALU.mult)
            Gg_r = work.tile([128, NL], F32, tag="h_ggr")
            Gg_i = work.tile([128, NL], F32, tag="h_ggi")
            qr = work.tile([128, NL], F32, tag="h_qr")
            qi = work.tile([128, NL], F32, tag="h_qi")
            base = 0
            for (t, rs_list) in link_groups:
                o = base
                for (s0, ln) in _contig_segments(rs_list):
                    nc.vector.tensor_copy(Gg_r[:, o:o+ln], Gq[:, s0:s0+ln])
                    nc.vector.tensor_copy(Gg_i[:, o:o+ln], Gq[:, 40+s0:40+s0+ln])
                    o += ln
                base += len(rs_list)
            uniform = (len(link_groups) == 40
                       and all(len(rs) == 39 for _, rs in link_groups))
            if uniform:
                # full-width inc multiply via 0-stride replicated APs
                IncR = Iq[:, 0:40].rearrange("p (t o) -> p t o", o=1
                                             ).broadcast_to([128, 40, 39])
                IncI = Iq[:, 40:80].rearrange("p (t o) -> p t o", o=1
                                              ).broadcast_to([128, 40, 39])
                Gg_r3 = Gg_r[:].rearrange("p (t j) -> p t j", t=40)
                Gg_i3 = Gg_i[:].rearrange("p (t j) -> p t j", t=40)
                qr3 = qr[:].rearrange("p (t j) -> p t j", t=40)
                qi3 = qi[:].rearrange("p (t j) -> p t j", t=40)
                nc.vector.tensor_tensor(out=qr3, in0=Gg_r3, in1=IncR,
                                        op=ALU.mult)
                nc.vector.tensor_tensor(out=qi3, in0=Gg_i3, in1=IncR,
                                        op=ALU.mult)
                nc.vector.tensor_tensor(out=Gg_i3, in0=Gg_i3, in1=IncI,
                                        op=ALU.mult)
                nc.vector.tensor_tensor(out=Gg_r3, in0=Gg_r3, in1=IncI,
                                        op=ALU.mult)
            else:
                base = 0
                for (t, rs_list) in link_groups:
                    sl = slice(base, base + len(rs_list))
                    nc.vector.tensor_scalar(out=qr[:, sl], in0=Gg_r[:, sl],
                                            scalar1=Iq[:, t:t+1], scalar2=None,
                                            op0=ALU.mult)
                    nc.vector.tensor_scalar(out=qi[:, sl], in0=Gg_i[:, sl],
                                            scalar1=Iq[:, t:t+1], scalar2=None,
                                            op0=ALU.mult)
                    nc.vector.tensor_scalar(out=Gg_i[:, sl], in0=Gg_i[:, sl],
                                            scalar1=Iq[:, 40+t:40+t+1],
                                            scalar2=None, op0=ALU.mult)
                    nc.vector.tensor_scalar(out=Gg_r[:, sl], in0=Gg_r[:, sl],
                                            scalar1=Iq[:, 40+t:40+t+1],
                                            scalar2=None, op0=ALU.mult)
                    base += len(rs_list)
            nc.vector.tensor_tensor(out=qr[:], in0=qr[:], in1=Gg_i[:],
                                    op=ALU.subtract)
            nc.vector.tensor_tensor(out=qi[:], in0=qi[:], in1=Gg_r[:],
                                    op=ALU.add)
            hr = work.tile([128, LPAD], F32R, tag="h_hr")
            hi = work.tile([128, LPAD], F32R, tag="h_hi")
            t2 = work.tile([128, NL], F32, tag="h_t2")
            nc.vector.memset(hr[:, NL:LPAD].bitcast(F32), 0.0)
            nc.vector.memset(hi[:, NL:LPAD].bitcast(F32), 0.0)
            nc.vector.tensor_tensor(out=hr[:, 0:NL], in0=qr[:], in1=wrep_r[:],
                                    op=ALU.mult)
            nc.vector.tensor_tensor(out=t2[:], in0=qi[:], in1=wrep_i[:],
                                    op=ALU.mult)
            nc.vector.tensor_tensor(out=hr[:, 0:NL], in0=hr[:, 0:NL], in1=t2[:],
                                    op=ALU.subtract)
            nc.vector.tensor_tensor(out=hi[:, 0:NL], in0=qr[:], in1=wrep_i[:],
                                    op=ALU.mult)
            nc.vector.tensor_tensor(out=t2[:], in0=qi[:], in1=wrep_r[:],
                                    op=ALU.mult)
            nc.vector.tensor_tensor(out=hi[:, 0:NL], in0=hi[:, 0:NL], in1=t2[:],
                                    op=ALU.add)
            nc.vector.tensor_scalar(out=hi[:], in0=hi[:], scalar1=-1.0,
                                    scalar2=None, op0=ALU.mult)
            nc.sync.dma_start(scr["htdram"][128*i:128*(i+1), :], hr[:])
            nc.sync.dma_start(scr["htdram"][N+128*i:N+128*(i+1), :], hi[:])
            nc.vector.tensor_tensor(out=t2[:], in0=hr[:, 0:NL],
                                    in1=drep[:, 0:NL], op=ALU.mult)
            nc.vector.tensor_reduce(vsum[:, i:i+1], t2[:], axis=AXX, op=ALU.add)
            nc.vector.tensor_tensor(out=t2[:], in0=hi[:, 0:NL],
                                    in1=drep[:, 0:NL], op=ALU.mult)
            nc.vector.tensor_reduce(vsum[:, NB+i:NB+i+1], t2[:], axis=AXX,
                                    op=ALU.add)
        vsq = work.tile([128, 2 * NB], F32, tag="vsq")
        nc.vector.tensor_tensor(out=vsq[:], in0=vsum[:], in1=vsum[:],
                                op=ALU.mult)
        vred = work.tile([128, 1], F32, tag="vred")
        nc.vector.tensor_reduce(vred[:], vsq[:], axis=AXX, op=ALU.add)
        nc.gpsimd.partition_all_reduce(vred[:], vred[:], 128,
                                       bass_isa.ReduceOp.add)
        nc.scalar.activation(lam[:], vred[:], AF.Sqrt)
        nc.vector.tensor_scalar(out=lam[:], in0=lam[:], scalar1=float(alpha),
                                scalar2=None, op0=ALU.mult)

    # ---------------- P7: Gram ----------------
    st_ = late.tile([128, LB], F32)
    srep = late.tile([128, LPAD], F32)
    with (
        tc.tile_pool(name="g_acc", bufs=1) as gacc,
        tc.tile_pool(name="g_work", bufs=1) as work,
        tc.tile_pool(name="g_psum", bufs=4, space="PSUM") as pg,
    ):
        GA = [gacc.tile([128, LPAD], F32, tag=f"ga{l}", name=f"ga{l}") for l in range(LB)]
        GRP = 4
        for g0 in range(0, 2 * NB, GRP):
            htrs = []
            for gi in range(GRP):
                ch = g0 + gi
                htr = work.tile([128, LPAD], F32R, tag=f"g_htr{gi}",
                                name=f"g_htr{gi}")
                nc.sync.dma_start(htr[:], scr["htdram"][128*ch:128*(ch+1), :])
                htrs.append(htr)
            for l in range(LB):
                c0 = 128 * l
                for cc in range(c0, LPAD, 416):
                    cw = min(416, LPAD - cc)
                    pgt = pg.tile([128, 416], F32, tag="g_pg")
                    for gi in range(GRP):
                        nc.tensor.matmul(pgt[:, 0:cw],
                                         htrs[gi][:, c0:c0+128],
                                         htrs[gi][:, cc:cc+cw],
                                         start=(gi == 0), stop=(gi == GRP - 1))
                    if g0 == 0:
                        nc.vector.tensor_copy(GA[l][:, cc:cc+cw], pgt[:, 0:cw])
                    else:
                        nc.vector.tensor_tensor(out=GA[l][:, cc:cc+cw],
                                                in0=GA[l][:, cc:cc+cw],
                                                in1=pgt[:, 0:cw], op=ALU.add)
        for l in range(LB):
            nc.sync.dma_start(scr["gramdram"][128*l:128*(l+1), :], GA[l][:])
        gd = work.tile([128, LB], F32, tag="gd")
        nc.sync.dma_start(gd[:], bass.AP(scr["gramdram"], 0,
                                         [[LPAD + 1, 128],
                                          [128 * (LPAD + 1), LB]]))
        nc.vector.tensor_scalar(out=gd[:], in0=gd[:], scalar1=lam[:],
                                scalar2=None, op0=ALU.add)
        nc.scalar.activation(st_[:], gd[:], AF.Sqrt)
        nc.vector.reciprocal(st_[:], st_[:])
        ps_ = pg.tile([LB, 128], F32, tag="s_ps")
        nc.tensor.matmul(ps_[:], st_[:], id_s[:], start=True, stop=True)
        s13 = work.tile([LB, 128], F32, tag="s13")
        nc.vector.tensor_copy(s13[:], ps_[:])
        nc.sync.dma_start(bass.AP(scr["srowdram"], 0, [[1, LPAD]]), s13[:])
        srow = work.tile([1, LPAD], F32, tag="srow")
        nc.sync.dma_start(srow[:], bass.AP(scr["srowdram"], 0, [[1, LPAD]]))
        nc.gpsimd.partition_broadcast(srep[:], srow[:])

    # ---------------- P8: scaled SPD solve ----------------
    bf2_pool = ctx.enter_context(tc.tile_pool(name="bf2", bufs=1))
    BF2 = [bf2_pool.tile([128, 128], F32R, tag=f"bf2_{l}", name=f"bf2_{l}") for l in range(LB)]
    with (
        tc.tile_pool(name="s_tri", bufs=1) as tri2,
        tc.tile_pool(name="s_work", bufs=2) as work,
        tc.tile_pool(name="s_pmm", bufs=2, space="PSUM") as pmm,
        tc.tile_pool(name="s_pmisc", bufs=1, space="PSUM") as pmisc,
    ):
        dsc = work.tile([128, LB], F32, tag="dsc")
        nc.vector.tensor_tensor(out=dsc[:], in0=dvec[:], in1=st_[:], op=ALU.mult)
        zz = work.tile([128, 128], F32, tag="zz")
        nc.vector.memset(zz[:], 0.0)
        for l in range(LB):
            nc.vector.tensor_copy(BF2[l][:], zz[:])
            nc.vector.tensor_copy(BF2[l][:, 0:1], dsc[:, l:l+1])
        GT = {}
        for i in range(LB):
            for j in range(i, LB):
                GT[(i, j)] = tri2.tile([128, 128], F32R, tag=f"g{i}_{j}", name=f"g{i}_{j}")
                gload = work.tile([128, 128], F32, tag="g_load")
                nc.sync.dma_start(gload[:],
                                  scr["gramdram"][128*i:128*(i+1),
                                                  128*j:128*(j+1)])
                nc.vector.tensor_scalar(out=gload[:], in0=gload[:],
                                        scalar1=st_[:, i:i+1], scalar2=None,
                                        op0=ALU.mult)
                nc.vector.tensor_tensor(out=gload[:], in0=gload[:],
                                        in1=srep[:, 128*j:128*(j+1)],
                                        op=ALU.mult)
                if i == j:
                    ones1 = work.tile([128, 1], F32, tag="diag1")
                    nc.vector.memset(ones1[:], 1.0)
                    nc.vector.copy_predicated(gload[:], idu_s[:],
                                              ones1[:].broadcast_to([128, 128]))
                nc.vector.tensor_copy(GT[(i, j)][:], gload[:])
        for k in range(LB):
            V = work.tile([128, 128], F32R, tag="lu2_V")
            _newton_real(nc, work, pmm, pmisc, GT[(k, k)], V, id_s, NEWTON_SPD)
            nc.sync.dma_start(scr["v2dram"][128*k:128*(k+1), :], V[:])
            for i in range(k + 1, LB):
                ptr = pmisc.tile([128, 128], F32R, tag="lu2_ptr")
                nc.tensor.transpose(ptr[:], GT[(k, i)][:], idr_s[:])
                utt = work.tile([128, 128], F32R, tag="lu2_utt")
                nc.vector.tensor_copy(utt[:], ptr[:])
                nc.sync.dma_start(
                    scr["ut2dram"][128*i:128*(i+1), 128*k:128*(k+1)], utt[:])
            for i in range(k + 1, LB):
                pl = pmm.tile([128, 128], F32, tag="cmmp1")
                nc.tensor.matmul(pl[:], V[:], GT[(k, i)][:], start=True,
                                 stop=True)
                LT = work.tile([128, 128], F32R, tag="lu2_LT")
                nc.vector.tensor_copy(LT[:], pl[:])
                pb = pmm.tile([128, 128], F32, tag="cmmp2")
                nc.tensor.matmul(pb[:], LT[:], BF2[k][:], start=True, stop=True)
                nc.vector.tensor_tensor(out=BF2[i][:], in0=BF2[i][:],
                                        in1=pb[:], op=ALU.subtract)
                for j in range(i, LB):
                    pt_ = pmm.tile([128, 128], F32, tag="cmmp1")
                    nc.tensor.matmul(pt_[:], LT[:], GT[(k, j)][:], start=True,
                                     stop=True)
                    nc.vector.tensor_tensor(out=GT[(i, j)][:],
                                            in0=GT[(i, j)][:], in1=pt_[:],
                                            op=ALU.subtract)

    ys = late.tile([128, LB], F32)
    yrep = late.tile([128, LPAD], F32)
    with (
        tc.tile_pool(name="b2_work", bufs=3) as work,
        tc.tile_pool(name="b2_pacc", bufs=1, space="PSUM") as pacc,
        tc.tile_pool(name="b2_pmm", bufs=2, space="PSUM") as pmm,
    ):
        for k in range(LB - 1, -1, -1):
            W = work.tile([128, 128], F32R, tag="bs2_W")
            nc.vector.tensor_copy(W[:], BF2[k][:])
            if k < LB - 1:
                P1 = pacc.tile([128, 128], F32, tag="bs2_p1")
                for idx, j in enumerate(range(k + 1, LB)):
                    utt = work.tile([128, 128], F32R, tag="bs2_utt")
                    nc.sync.dma_start(
                        utt[:], scr["ut2dram"][128*j:128*(j+1),
                                               128*k:128*(k+1)])
                    nc.tensor.matmul(P1[:], utt[:], BF2[j][:],
                                     start=(idx == 0), stop=(j == LB - 1))
                nc.vector.tensor_tensor(out=W[:], in0=W[:], in1=P1[:],
                                        op=ALU.subtract)
            Vk = work.tile([128, 128], F32R, tag="bs2_V")
            nc.sync.dma_start(Vk[:], scr["v2dram"][128*k:128*(k+1), :])
            Pf = pmm.tile([128, 128], F32, tag="bs2_pf")
            nc.tensor.matmul(Pf[:], Vk[:], W[:], start=True, stop=True)
            nc.vector.tensor_copy(BF2[k][:], Pf[:])
        for l in range(LB):
            nc.vector.tensor_copy(ys[:, l:l+1], BF2[l][:, 0:1])
        nc.vector.tensor_tensor(out=ys[:], in0=ys[:], in1=st_[:], op=ALU.mult)
        psy = pmm.tile([LB, 128], F32, tag="y_ps")
        nc.tensor.matmul(psy[:], ys[:], id_s[:], start=True, stop=True)
        y13 = work.tile([LB, 128], F32, tag="y13")
        nc.vector.tensor_copy(y13[:], psy[:])
        nc.sync.dma_start(bass.AP(scr["yrowdram"], 0, [[1, LPAD]]), y13[:])
        yrow = work.tile([1, LPAD], F32, tag="yrow")
        nc.sync.dma_start(yrow[:], bass.AP(scr["yrowdram"], 0, [[1, LPAD]]))
        nc.gpsimd.partition_broadcast(yrep[:], yrow[:])

    # ---------------- P9: chi = Ht y ----------------
    with tc.tile_pool(name="p9_work", bufs=2) as work:
        chi = late.tile([128, 2 * NB], F32)
        for ch in range(2 * NB):
            htc = work.tile([128, LPAD], F32R, tag="c_htc")
            nc.sync.dma_start(htc[:], scr["htdram"][128*ch:128*(ch+1), :])
            tm = work.tile([128, LPAD], F32, tag="c_tm")
            nc.vector.tensor_tensor(out=tm[:], in0=htc[:], in1=yrep[:],
                                    op=ALU.mult)
            nc.vector.tensor_reduce(chi[:, ch:ch+1], tm[:], axis=AXX,
                                    op=ALU.add)
        nc.sync.dma_start(bass.AP(out_chi, 0, [[1, 128], [128, 2 * NB]]),
                          chi[:])
    ctx.close()


_GBASE = {}

def _contig_segments(rs_list):
    segs = []
    s = rs_list[0]; prev = s
    for r in rs_list[1:]:
        if r == prev + 1:
            prev = r
        else:
            segs.append((s, prev - s + 1)); s = r; prev = r
    segs.append((s, prev - s + 1))
    return segs


_CACHED = {}


def kernel(epsilon_r_iter, chi_iter, total_power, alpha, grid_x, grid_y,
           direct_field, incident_field, G_freespace, G_freespace_scaled,
           sensor_links):
    eps = np.asarray(epsilon_r_iter)
    chi_it = np.asarray(chi_iter)
    tp = np.asarray(total_power, dtype=np.float32)
    alpha_f = float(np.asarray(alpha))
    gx = np.asarray(grid_x, dtype=np.float32)
    gy = np.asarray(grid_y, dtype=np.float32)
    df = np.asarray(direct_field)
    einc = np.asarray(incident_field)
    gfs = np.asarray(G_freespace)
    gsc = np.asarray(G_freespace_scaled)
    links = np.asarray(sensor_links)

    x = gx.T.reshape(N).astype(np.float32)
    y = gy.T.reshape(N).astype(np.float32)
    scat = np.real(eps.T.reshape(N)).astype(np.float32)

    geomS = np.stack([np.ones(N, np.float32), -2.0*x, -2.0*y,
                      (x*x + y*y)]).astype(np.float32)
    geomR = np.stack([(x*x + y*y), x, y,
                      np.ones(N, np.float32)]).astype(np.float32)
    scat_t = scat.reshape(NB, 128).T.copy()

    bpack = np.zeros((N, RW), np.float32)
    bpack[:, 0:40] = -einc.real; bpack[:, 40:80] = -gfs.real
    bpack[:, 128:168] = -einc.imag; bpack[:, 168:208] = -gfs.imag
    gscT = np.concatenate([gsc.real.T, gsc.imag.T], axis=1).astype(np.float32)
    dfpack = np.concatenate([df.real, df.imag], axis=1).astype(np.float32)
    tpT = tp.T.copy().astype(np.float32)

    groups = []
    i = 0
    while i < len(links):
        t = int(links[i, 0])
        rs_list = []
        while i < len(links) and int(links[i, 0]) == t:
            rs_list.append(int(links[i, 1]))
            i += 1
        groups.append((t, rs_list))

    _GBASE.clear()
    o = 0
    for (t, rs_list) in groups:
        _GBASE[t] = o
        o += len(rs_list)
    key = (hash(links.tobytes()), alpha_f)
    if key not in _CACHED:
        _CACHED[key] = build_program(groups, alpha_f)
    nc = _CACHED[key]

    id128 = np.eye(128, dtype=np.float32)
    im = {
        "geomS": geomS, "geomR": geomR, "scat_t": scat_t, "bpack": bpack,
        "gscT": gscT, "dfpack": dfpack, "tpT": tpT,
        "id128": id128, "idu8": id128.astype(np.uint8),
    }
    import os as _os
    _tr = _os.environ.get("KTRACE", "0") == "1"
    res = run_bass_kernel_spmd(nc, [im] * 8, core_ids=list(range(8)), trace=_tr)
    out = res.results[0]
    _CACHED["last"] = (res, out)

    chi = out["out_chi"]
    dchi_r = chi[:N].reshape(M, M).T
    dchi_i = chi[N:].reshape(M, M).T
    chi_new = (chi_it + (dchi_r + 1j * dchi_i)).astype(np.complex64)
    return chi_new + 1.0, chi_new



# revision 32
# speedup vs baseline: 1.4967x; 1.0797x over previous
"""DRIM layer (distorted Rytov inverse-scattering iteration) on Trainium2.

One Bass/Tile program per core (replicated SPMD on 8 cores):
  P1  Z-matrix build via large-branch Hankel evaluation (upper triangle only;
      Z is complex-symmetric), resident in SBUF as fp32r planes
  P2  block LDL^T elimination, Newton-iterated 128x128 block inverses,
      fp32r tensor-engine matmuls
  P3  back-substitution -> X = Z^-1 [-E_inc | -G]
  P4  total field, RSS power model, data vector
  P5  Rytov H^T rows (4608 x 1664 padded) + H^T d
  P7  Gram H H^T (upper blocks) + Jacobi scaling
  P8  scaled SPD block solve (same Newton machinery, real)
  P9  chi = H^T y, output dchi

Host does input packing / output reshape only.
"""
import math
import os
import numpy as np

import concourse.bass as bass
import concourse.bacc as bacc
import concourse.bass_isa as bass_isa
import concourse.mybir as mybir
import concourse.tile as tile
from concourse.bass_utils import run_bass_kernel_spmd

F32 = mybir.dt.float32
F32R = mybir.dt.float32r
U8 = mybir.dt.uint8
AF = mybir.ActivationFunctionType
ALU = mybir.AluOpType
AXX = mybir.AxisListType.X

M = 48
N = M * M
NB = N // 128               # 18
TX = RX = 40
NL = TX * (RX - 1)          # 1560
LPAD = 1664
LB = LPAD // 128            # 13
RW = 256                    # [0:128]=Re plane, [128:256]=Im plane
CW = 256                    # Z-build column chunk
DOI = 3.0
WL = 0.125
K0 = 2.0 * math.pi / WL
IMP = 120.0 * math.pi
GRID_LEN = DOI / M
GRID_RADIUS = math.sqrt(GRID_LEN ** 2 / math.pi)
NOISE = 1e-6

def _j1s(x):
    t2 = (x / 3.0) ** 2
    return x * (0.5 - 0.56249985*t2 + 0.21093573*t2**2 - 0.03954289*t2**3
                + 0.00443319*t2**4 - 0.00031761*t2**5 + 0.00001109*t2**6)

def _y1s(x):
    t2 = (x / 3.0) ** 2
    p = (-0.6366198 + 0.2212091*t2 + 2.1682709*t2**2 - 1.3164827*t2**3
         + 0.3123951*t2**4 - 0.0400976*t2**5 + 0.0027873*t2**6)
    return ((2.0/math.pi) * x * math.log(0.5*x) * _j1s(x) + p) / x

X0C = K0 * GRID_RADIUS
GRID_AREA = 4.0*math.pi*GRID_RADIUS/(2.0*K0) * _j1s(X0C)
C1 = -IMP * math.pi * GRID_RADIUS / 2.0
C2 = _j1s(X0C)
C3R, C3I = _j1s(X0C), _y1s(X0C)
C1C2 = C1 * C2
ZD_RE = C1 * C3R
ZD_IM_C = C1 * C3I
SA = GRID_AREA * K0 * K0
TWO_PI = 2.0 * math.pi
INV_2PI = 1.0 / TWO_PI
LOG10E20 = 20.0 * math.log10(math.e)
CADD = 10.0 * math.log10(WL * WL / (4.0 * math.pi * IMP) / 1e-3)
C20L = 20.0 / math.log(10.0)

F0C = [0.79788456, -0.00000077, -0.00552740, -0.00009512,
       0.00137237, -0.00072805, 0.00014476]
THC = [-0.78539816, -0.04166397, -0.00003954, 0.00262573,
       -0.00054125, -0.00029333, 0.00013558]
F0CS = [c * (3.0 ** k) * C1C2 for k, c in enumerate(F0C)]
THCS = [c * (3.0 ** k) for k, c in enumerate(THC)]
# short-series Z build: th = x + 3*THC[1]/x ; amp = C1C2*(F0C0 + 3*F0C1/x)/sqrt(x)
TH1 = 3.0 * THC[1]
A0C = C1C2 * F0C[0]
A1C = 3.0 * C1C2 * F0C[1]
K0K0 = K0 * K0
PI4 = math.pi / 4.0
# short-series Z build: th = x - pi/4 + 3*THC[1]*rx ; amp = C1C2*(F0C0 + 3*F0C1*rx)/sqrt(x)
TH1 = 3.0 * THC[1]
A0C = C1C2 * F0C[0]
A1C = 3.0 * C1C2 * F0C[1]
K0K0 = K0 * K0
PI4 = math.pi / 4.0

NEWTON_Z = 14
NEWTON_SPD = 10


def _horner(nc, out_ap, s_ap, coeffs):
    cs = coeffs[::-1]
    nc.vector.tensor_scalar(out=out_ap, in0=s_ap, scalar1=float(cs[0]),
                            scalar2=float(cs[1]), op0=ALU.mult, op1=ALU.add)
    for c in cs[2:]:
        nc.vector.tensor_tensor(out=out_ap, in0=out_ap, in1=s_ap, op=ALU.mult)
        nc.vector.tensor_scalar(out=out_ap, in0=out_ap, scalar1=float(c),
                                scalar2=None, op0=ALU.add)


def _cmm(nc, pool, lhsT, rhs, n=RW):
    P1 = pool.tile([128, n], F32, tag="cmmp1")
    P2 = pool.tile([128, n], F32, tag="cmmp2")
    nc.tensor.matmul(P1[:], lhsT[:, 0:128], rhs, start=True, stop=True)
    nc.tensor.matmul(P2[:], lhsT[:, 128:256], rhs, start=True, stop=True)
    return P1, P2


def _combine_sub(nc, dst, P1, P2):
    nc.vector.tensor_tensor(out=dst[:, 0:256], in0=dst[:, 0:256],
                            in1=P1[:, 0:256], op=ALU.subtract)
    nc.vector.tensor_tensor(out=dst[:, 0:128], in0=dst[:, 0:128],
                            in1=P2[:, 128:256], op=ALU.add)
    nc.vector.tensor_tensor(out=dst[:, 128:256], in0=dst[:, 128:256],
                            in1=P2[:, 0:128], op=ALU.subtract)


def _combine_set(nc, dst, P1, P2):
    nc.vector.tensor_copy(dst[:, 0:256], P1[:, 0:256])
    nc.vector.tensor_tensor(out=dst[:, 0:128], in0=dst[:, 0:128],
                            in1=P2[:, 128:256], op=ALU.subtract)
    nc.vector.tensor_tensor(out=dst[:, 128:256], in0=dst[:, 128:256],
                            in1=P2[:, 0:128], op=ALU.add)


def _newton_scale(nc, work, pmisc, m, tag):
    """1/(colmax * rowmax) of m [128,128] -> [128,1] fp32 AP."""
    ones = work.tile([128, 1], F32, tag=f"nwo_{tag}")
    nc.vector.memset(ones[:], 1.0)
    pc = pmisc.tile([128, 1], F32, tag=f"nwpc_{tag}")
    nc.tensor.matmul(pc[:], m[:], ones[:], start=True, stop=True)
    pr = pmisc.tile([1, 128], F32, tag=f"nwpr_{tag}")
    nc.tensor.matmul(pr[:], ones[:], m[:], start=True, stop=True)
    cs = work.tile([128, 1], F32, tag=f"nwcs_{tag}")
    nc.vector.tensor_copy(cs[:], pc[:])
    rs = work.tile([1, 128], F32, tag=f"nwrs_{tag}")
    nc.vector.tensor_copy(rs[:], pr[:])
    nc.gpsimd.partition_all_reduce(cs[:], cs[:], 128, bass_isa.ReduceOp.max)
    rmax = work.tile([1, 1], F32, tag=f"nwrm_{tag}")
    nc.vector.tensor_reduce(rmax[:], rs[:], axis=AXX, op=ALU.max)
    rmax_b = work.tile([128, 1], F32, tag=f"nwrb_{tag}")
    nc.gpsimd.partition_broadcast(rmax_b[:], rmax[:])
    a = work.tile([128, 1], F32, tag=f"nwa_{tag}")
    nc.vector.tensor_tensor(out=a[:], in0=cs[:], in1=rmax_b[:], op=ALU.mult)
    nc.vector.reciprocal(a[:], a[:])
    return a


def _newton_cplx(nc, work, pmm, pmisc, D, Xout, id2_s, iters):
    """Swap-free Newton: maintains X=[Xr|Xi] and Xs=[-Xi|Xr].
    D@X via lhsT=Dr,rhs=X + lhsT=Di,rhs=Xs (both width-256, PSUM acc).
    """
    m = work.tile([128, 128], F32, tag="nw_m")
    m2 = work.tile([128, 128], F32, tag="nw_m2")
    nc.scalar.activation(m[:], D[:, 0:128], AF.Abs)
    nc.scalar.activation(m2[:], D[:, 128:256], AF.Abs)
    nc.vector.tensor_tensor(out=m[:], in0=m[:], in1=m2[:], op=ALU.max)
    a = _newton_scale(nc, work, pmisc, m, "c")
    nc.vector.tensor_scalar(out=Xout[:, 0:128], in0=D[:, 0:128], scalar1=a[:],
                            scalar2=None, op0=ALU.mult)
    Xs = work.tile([128, RW], F32R, tag="nw_Xs")
    nc.vector.tensor_scalar(out=Xs[:, 0:128], in0=D[:, 128:256], scalar1=a[:],
                            scalar2=None, op0=ALU.mult)
    # Xi = -Di*a = -(Xs lo)
    nc.vector.tensor_scalar(out=Xout[:, 128:256], in0=Xs[:, 0:128],
                            scalar1=-1.0, scalar2=None, op0=ALU.mult)
    nc.scalar.copy(Xs[:, 128:256], Xout[:, 0:128])
    R = work.tile([128, RW], F32R, tag="nw_R")
    Rs = work.tile([128, RW], F32R, tag="nw_Rs")
    for _ in range(iters):
        P = pmm.tile([128, RW], F32, tag="cmmp1")
        nc.tensor.matmul(P[:], D[:, 0:128], Xout[:, 0:RW], start=True,
                         stop=False)
        nc.tensor.matmul(P[:], D[:, 128:256], Xs[:, 0:RW], start=False,
                         stop=True)
        # R = [I|0] - P ;  Rs = [-Ri|Rr] = [Pi | Rr]
        nc.vector.tensor_tensor(out=R[:], in0=id2_s[:], in1=P[:],
                                op=ALU.subtract)
        nc.scalar.copy(Rs[:, 0:128], P[:, 128:256])
        nc.scalar.copy(Rs[:, 128:256], R[:, 0:128])
        Q = pmm.tile([128, RW], F32, tag="cmmp2")
        nc.tensor.matmul(Q[:], Xout[:, 0:128], R[:, 0:RW], start=True,
                         stop=False)
        nc.tensor.matmul(Q[:], Xout[:, 128:256], Rs[:, 0:RW], start=False,
                         stop=True)
        # X += Q ; Xs_lo -= Qi ; Xs_hi = new Xr
        nc.vector.tensor_tensor(out=Xout[:, 0:256], in0=Xout[:, 0:256],
                                in1=Q[:, 0:256], op=ALU.add)
        nc.vector.tensor_tensor(out=Xs[:, 0:128], in0=Xs[:, 0:128],
                                in1=Q[:, 128:256], op=ALU.subtract)
        nc.scalar.copy(Xs[:, 128:256], Xout[:, 0:128])


def _newton_real(nc, work, pmm, pmisc, D, Xout, id_s, iters):
    m = work.tile([128, 128], F32, tag="nw_m")
    nc.scalar.activation(m[:], D[:], AF.Abs)
    a = _newton_scale(nc, work, pmisc, m, "r")
    nc.vector.tensor_scalar(out=Xout[:], in0=D[:], scalar1=a[:], scalar2=None,
                            op0=ALU.mult)
    R = work.tile([128, 128], F32R, tag="nw_R")
    for _ in range(iters):
        P1 = pmm.tile([128, 128], F32, tag="cmmp1")
        nc.tensor.matmul(P1[:], D[:], Xout[:], start=True, stop=True)
        nc.vector.tensor_tensor(out=R[:], in0=id_s[:], in1=P1[:],
                                op=ALU.subtract)
        Q1 = pmm.tile([128, 128], F32, tag="cmmp2")
        nc.tensor.matmul(Q1[:], Xout[:], R[:], start=True, stop=True)
        nc.vector.tensor_tensor(out=Xout[:], in0=Xout[:], in1=Q1[:], op=ALU.add)


def build_program(link_groups, alpha):
    nc = bacc.Bacc("TRN2", target_bir_lowering=False, num_devices=8)
    din = {}
    def inp(name, shape, dtype=F32):
        din[name] = nc.dram_tensor(name, shape, dtype, kind="ExternalInput")
    inp("geomS", [4, N]); inp("geomR", [4, N]); inp("scat_t", [128, NB])
    inp("bpack", [N, RW]); inp("gscT", [N, 80]); inp("dfpack", [40, 80])
    inp("tpT", [40, RX - 1]); inp("id128", [128, 128]); inp("idu8", [128, 128], U8)
    out_chi = nc.dram_tensor("out_chi", [2 * N], F32, kind="ExternalOutput")
    xdbg = nc.dram_tensor("xdbg", [N, RW], F32, kind="ExternalOutput")
    tfdbg = nc.dram_tensor("tfdbg", [40, 80], F32, kind="ExternalOutput")
    ddbg = nc.dram_tensor("ddbg", [40, RX - 1], F32, kind="ExternalOutput")
    scr = {}
    scr["vdram"] = nc.dram_tensor("vdram", [NB * 128, RW], F32R, kind="Internal")
    scr["utdram"] = nc.dram_tensor("utdram", [N, 2 * N], F32R, kind="Internal")
    scr["htdram"] = nc.dram_tensor("htdram", [2 * N, LPAD], F32R, kind="Internal")
    scr["gramdram"] = nc.dram_tensor("gramdram", [LPAD, LPAD], F32, kind="Internal")
    scr["v2dram"] = nc.dram_tensor("v2dram", [LB * 128, 128], F32R, kind="Internal")
    scr["ut2dram"] = nc.dram_tensor("ut2dram", [LPAD, LPAD], F32R, kind="Internal")
    scr["sdram"] = nc.dram_tensor("sdram", [NL], F32, kind="Internal")
    scr["wdram"] = nc.dram_tensor("wdram", [2 * NL], F32, kind="Internal")
    scr["srowdram"] = nc.dram_tensor("srowdram", [LPAD], F32, kind="Internal")
    scr["yrowdram"] = nc.dram_tensor("yrowdram", [LPAD], F32, kind="Internal")

    with tile.TileContext(nc) as tc:
        _body(nc, tc, din, out_chi, xdbg, tfdbg, ddbg, scr, link_groups, alpha)
    nc.compile()
    return nc


def _body(nc, tc, din, out_chi, xdbg, tfdbg, ddbg, scr, link_groups, alpha):
    import contextlib
    ctx = contextlib.ExitStack()
    consts = ctx.enter_context(tc.tile_pool(name="consts", bufs=1))
    id_s = consts.tile([128, 128], F32)
    nc.sync.dma_start(id_s[:], din["id128"][:])
    idr_s = consts.tile([128, 128], F32R)
    nc.vector.tensor_copy(idr_s[:], id_s[:])
    idu_s = consts.tile([128, 128], U8)
    nc.sync.dma_start(idu_s[:], din["idu8"][:])
    id2_s = consts.tile([128, RW], F32)
    nc.vector.memset(id2_s[:], 0.0)
    nc.vector.tensor_copy(id2_s[:, 0:128], id_s[:])
    scat_s = consts.tile([128, NB], F32)
    nc.sync.dma_start(scat_s[:], din["scat_t"][:])

    zdi_s = consts.tile([128, NB], F32)
    fsc_s = consts.tile([128, NB], F32)
    t0 = consts.tile([128, NB], F32)
    nc.vector.tensor_scalar(out=t0[:], in0=scat_s[:], scalar1=-1.0,
                            scalar2=None, op0=ALU.add)
    nc.vector.reciprocal(t0[:], t0[:])
    nc.vector.tensor_scalar(out=fsc_s[:], in0=t0[:], scalar1=(IMP / K0),
                            scalar2=None, op0=ALU.mult)
    nc.vector.tensor_tensor(out=t0[:], in0=t0[:], in1=scat_s[:], op=ALU.mult)
    nc.vector.tensor_scalar(out=zdi_s[:], in0=t0[:], scalar1=-(IMP / K0),
                            scalar2=ZD_IM_C, op0=ALU.mult, op1=ALU.add)
    zdr_c = consts.tile([128, 1], F32)
    nc.vector.memset(zdr_c[:], float(ZD_RE))

    bf_pool = ctx.enter_context(tc.tile_pool(name="bf", bufs=1))
    BF = [bf_pool.tile([128, RW], F32R, tag=f"bf{i}", name=f"bf{i}") for i in range(NB)]

    with tc.tile_pool(name="tri", bufs=1) as tri:
        ZT = {}
        for i in range(NB):
            for j in range(i, NB):
                ZT[(i, j)] = tri.tile([128, RW], F32R, tag=f"z{i}_{j}", name=f"z{i}_{j}")

        # ---------------- P1: Z build ----------------
        with (
            tc.tile_pool(name="zb_geom", bufs=2) as gpool,
            tc.tile_pool(name="zb_work", bufs=1) as work,
            tc.tile_pool(name="zb_psum", bufs=2, space="PSUM") as pz,
        ):
            for k in range(NB):
                r0 = 128 * k
                gS = gpool.tile([4, 128], F32, tag="gS", name="gS")
                nc.sync.dma_start(gS[:], din["geomS"][:, r0:r0+128])
                chunks = []
                j = k
                while j < NB:
                    w = 256 if j + 1 < NB else 128
                    chunks.append((j, w))
                    j += w // 128

                def stage_aps(j, w):
                    # r1/r2 staged in the ZT tiles themselves (scratch reuse)
                    if w == 256:
                        return ZT[(k, j)][:, 0:256], ZT[(k, j + 1)][:, 0:256]
                    return ZT[(k, j)][:, 0:128], ZT[(k, j)][:, 128:256]

                for b0 in range(0, len(chunks), 5):
                    batch = chunks[b0:b0+5]
                    amps = []
                    # pass A: sqrt act-table (+Copy for int round-trips)
                    for ci, (j, w) in enumerate(batch):
                        c0 = 128 * j
                        gR = gpool.tile([4, CW], F32, tag="gR", name="gR")
                        nc.sync.dma_start(gR[:, 0:w], din["geomR"][:, c0:c0+w])
                        pd = pz.tile([128, CW], F32, tag="zb_pd")
                        nc.tensor.matmul(pd[:, 0:w], gS[:], gR[:, 0:w],
                                         start=True, stop=True)
                        ts1 = work.tile([128, CW], F32, tag="zb_ts1")
                        nc.vector.tensor_scalar(out=ts1[:, 0:w], in0=pd[:, 0:w],
                                                scalar1=0.002,
                                                scalar2=float(K0K0),
                                                op0=ALU.max, op1=ALU.mult)
                        xf = work.tile([128, CW], F32, tag="zb_xf")
                        nc.scalar.activation(xf[:, 0:w], ts1[:, 0:w], AF.Sqrt)
                        rx = work.tile([128, CW], F32, tag="zb_rx")
                        nc.vector.reciprocal(rx[:, 0:w], xf[:, 0:w])
                        th = work.tile([128, CW], F32, tag="zb_ts1")
                        nc.vector.scalar_tensor_tensor(
                            out=th[:, 0:w], in0=rx[:, 0:w], scalar=float(TH1),
                            in1=xf[:, 0:w], op0=ALU.mult, op1=ALU.add)
                        srx = work.tile([128, CW], F32, tag="zb_xf")
                        nc.scalar.activation(srx[:, 0:w], rx[:, 0:w], AF.Sqrt)
                        r1ap, r2ap = stage_aps(j, w)
                        ki = work.tile([128, CW], mybir.dt.int32, tag="zb_ki")
                        mf = work.tile([128, CW], F32, tag="zb_mf")
                        # r1 = (th - pi/4) - 2pi*round((th - pi/4)/2pi)
                        nc.scalar.activation(ki[:, 0:w], th[:, 0:w], AF.Copy,
                                             scale=float(INV_2PI),
                                             bias=-0.125)
                        nc.scalar.activation(mf[:, 0:w], ki[:, 0:w], AF.Copy,
                                             bias=0.125)
                        nc.vector.scalar_tensor_tensor(
                            out=r1ap, in0=mf[:, 0:w],
                            scalar=float(-TWO_PI), in1=th[:, 0:w],
                            op0=ALU.mult, op1=ALU.add)
                        # r2 = (th + pi/4) - 2pi*round((th + pi/4)/2pi)
                        nc.scalar.activation(ki[:, 0:w], th[:, 0:w], AF.Copy,
                                             scale=float(INV_2PI),
                                             bias=0.125)
                        nc.scalar.activation(mf[:, 0:w], ki[:, 0:w], AF.Copy,
                                             bias=-0.125)
                        nc.vector.scalar_tensor_tensor(
                            out=r2ap, in0=mf[:, 0:w],
                            scalar=float(-TWO_PI), in1=th[:, 0:w],
                            op0=ALU.mult, op1=ALU.add)
                        f0t = work.tile([128, CW], F32, tag="zb_ts1")
                        nc.vector.tensor_scalar(out=f0t[:, 0:w],
                                                in0=rx[:, 0:w],
                                                scalar1=float(A1C),
                                                scalar2=float(A0C),
                                                op0=ALU.mult, op1=ALU.add)
                        amp = work.tile([128, CW], F32, tag=f"zb_amp{ci}",
                                        name=f"zb_amp{ci}")
                        nc.vector.tensor_tensor(out=amp[:, 0:w],
                                                in0=f0t[:, 0:w],
                                                in1=srx[:, 0:w], op=ALU.mult)
                        amps.append(amp)
                    # pass B: trig act-table
                    for ci, (j, w) in enumerate(batch):
                        amp = amps[ci]
                        r1ap, r2ap = stage_aps(j, w)
                        sinr = work.tile([128, CW], F32, tag="zb_rx")
                        nc.scalar.activation(sinr[:, 0:w], r1ap, AF.Sin)
                        cosr = work.tile([128, CW], F32, tag="zb_xf")
                        nc.scalar.activation(cosr[:, 0:w], r2ap, AF.Sin)
                        if j == k:
                            # diag block: predicate on F32 staging, then copy
                            stg = work.tile([128, CW], F32, tag="zb_mf")
                            nc.vector.tensor_tensor(
                                out=stg[:, 0:128], in0=amp[:, 0:128],
                                in1=cosr[:, 0:128], op=ALU.mult)
                            nc.vector.tensor_tensor(
                                out=stg[:, 128:256], in0=amp[:, 0:128],
                                in1=sinr[:, 0:128], op=ALU.mult)
                            nc.vector.copy_predicated(
                                stg[:, 0:128], idu_s[:],
                                zdr_c[:].broadcast_to([128, 128]))
                            nc.vector.copy_predicated(
                                stg[:, 128:256], idu_s[:],
                                zdi_s[:, k:k+1].broadcast_to([128, 128]))
                            nc.vector.tensor_copy(ZT[(k, k)][:, 0:256],
                                                  stg[:, 0:256])
                            bstart = 1
                        else:
                            bstart = 0
                        for b in range(bstart, w // 128):
                            sl = slice(128 * b, 128 * b + 128)
                            nc.vector.tensor_tensor(
                                out=ZT[(k, j + b)][:, 0:128],
                                in0=amp[:, sl], in1=cosr[:, sl], op=ALU.mult)
                            nc.vector.tensor_tensor(
                                out=ZT[(k, j + b)][:, 128:256],
                                in0=amp[:, sl], in1=sinr[:, sl], op=ALU.mult)

        # ---------------- P2: block LDL^T ----------------
        with (
            tc.tile_pool(name="lu_big", bufs=1) as work,
            tc.tile_pool(name="lu_sm", bufs=2) as wsm,
            tc.tile_pool(name="lu_pmm", bufs=1, space="PSUM") as pmm,
            tc.tile_pool(name="lu_pmisc", bufs=1, space="PSUM") as pmisc,
        ):
            ldtmp0 = wsm.tile([128, RW], F32, tag="ldtmp")
            for i in range(NB):
                nc.sync.dma_start(ldtmp0[:], din["bpack"][128*i:128*(i+1), :])
                nc.vector.tensor_copy(BF[i][:], ldtmp0[:])
                ldtmp0 = wsm.tile([128, RW], F32, tag="ldtmp")
            IC = 3
            for k in range(NB):
                V = work.tile([128, RW], F32R, tag="lu_V")
                _newton_cplx(nc, work, pmm, pmisc, ZT[(k, k)], V, id2_s,
                             NEWTON_Z)
                nc.sync.dma_start(scr["vdram"][128*k:128*(k+1), :], V[:])
                for i in range(k + 1, NB):
                    ptr = pmisc.tile([128, 128], F32R, tag="lu_ptr")
                    nc.tensor.transpose(ptr[:], ZT[(k, i)][:, 0:128], idr_s[:])
                    utt = wsm.tile([128, RW], F32R, tag="lu_utt")
                    nc.vector.tensor_copy(utt[:, 0:128], ptr[:])
                    pti = pmisc.tile([128, 128], F32R, tag="lu_pti")
                    nc.tensor.transpose(pti[:], ZT[(k, i)][:, 128:256], idr_s[:])
                    nc.vector.tensor_copy(utt[:, 128:256], pti[:])
                    nc.sync.dma_start(
                        scr["utdram"][128*i:128*(i+1), 256*k:256*(k+1)], utt[:])
                if k == NB - 1:
                    continue
                bswap = work.tile([128, RW], F32R, tag="lu_bs")
                nc.vector.tensor_scalar(out=bswap[:, 0:128],
                                        in0=BF[k][:, 128:256], scalar1=-1.0,
                                        scalar2=None, op0=ALU.mult)
                nc.scalar.copy(bswap[:, 128:256], BF[k][:, 0:128])
                for a in range(k + 1, NB, IC):
                    b = min(a + IC, NB)
                    LTs = {}
                    for j in range(a, NB):
                        zsw = wsm.tile([128, RW], F32R, tag="lu_zsw")
                        nc.vector.tensor_scalar(out=zsw[:, 0:128],
                                                in0=ZT[(k, j)][:, 128:256],
                                                scalar1=-1.0, scalar2=None,
                                                op0=ALU.mult)
                        nc.scalar.copy(zsw[:, 128:256], ZT[(k, j)][:, 0:128])
                        if j < b:
                            pl = pmm.tile([128, RW], F32, tag="cmmp1")
                            nc.tensor.matmul(pl[:], V[:, 0:128],
                                             ZT[(k, j)][:, 0:RW],
                                             start=True, stop=False)
                            nc.tensor.matmul(pl[:], V[:, 128:256],
                                             zsw[:, 0:RW],
                                             start=False, stop=True)
                            LT = work.tile([128, RW], F32R,
                                           tag=f"lu_LT{j - a}",
                                           name=f"lu_LT{j - a}")
                            nc.vector.tensor_copy(LT[:], pl[:])
                            LTs[j] = LT
                            pb = pmm.tile([128, RW], F32, tag="cmmp2")
                            nc.tensor.matmul(pb[:], LT[:, 0:128],
                                             BF[k][:, 0:RW],
                                             start=True, stop=False)
                            nc.tensor.matmul(pb[:], LT[:, 128:256],
                                             bswap[:, 0:RW],
                                             start=False, stop=True)
                            nc.vector.tensor_tensor(out=BF[j][:, 0:256],
                                                    in0=BF[j][:, 0:256],
                                                    in1=pb[:, 0:256],
                                                    op=ALU.subtract)
                        for i in range(a, min(b, j + 1)):
                            pu = pmm.tile([128, RW], F32,
                                          tag=f"updp{(i - a) % 2}",
                                          name=f"updp{(i - a) % 2}")
                            nc.tensor.matmul(pu[:], LTs[i][:, 0:128],
                                             ZT[(k, j)][:, 0:RW],
                                             start=True, stop=False)
                            nc.tensor.matmul(pu[:], LTs[i][:, 128:256],
                                             zsw[:, 0:RW],
                                             start=False, stop=True)
                            nc.vector.tensor_tensor(out=ZT[(i, j)][:, 0:256],
                                                    in0=ZT[(i, j)][:, 0:256],
                                                    in1=pu[:, 0:256],
                                                    op=ALU.subtract)

    # ---------------- P3: back-substitution ----------------
    with (
        tc.tile_pool(name="bs_work", bufs=3) as work,
        tc.tile_pool(name="bs_pacc", bufs=1, space="PSUM") as pacc,
        tc.tile_pool(name="bs_pmm", bufs=2, space="PSUM") as pmm,
    ):
        for k in range(NB - 1, -1, -1):
            W = work.tile([128, RW], F32R, tag="bs_W")
            nc.vector.tensor_copy(W[:], BF[k][:])
            if k < NB - 1:
                P1 = pacc.tile([128, RW], F32, tag="bs_p1")
                P2 = pacc.tile([128, RW], F32, tag="bs_p2")
                for idx, j in enumerate(range(k + 1, NB)):
                    utt = work.tile([128, RW], F32R, tag="bs_utt")
                    nc.sync.dma_start(
                        utt[:], scr["utdram"][128*j:128*(j+1), 256*k:256*(k+1)])
                    st = (idx == 0); sp_ = (j == NB - 1)
                    nc.tensor.matmul(P1[:], utt[:, 0:128], BF[j][:, 0:RW],
                                     start=st, stop=sp_)
                    nc.tensor.matmul(P2[:], utt[:, 128:256], BF[j][:, 0:RW],
                                     start=st, stop=sp_)
                _combine_sub(nc, W, P1, P2)
            Vk = work.tile([128, RW], F32R, tag="bs_V")
            nc.sync.dma_start(Vk[:], scr["vdram"][128*k:128*(k+1), :])
            P1, P2 = _cmm(nc, pmm, Vk, W[:, 0:RW])
            _combine_set(nc, BF[k], P1, P2)
            nc.sync.dma_start(xdbg[128*k:128*(k+1), :], BF[k][:].bitcast(F32))

    # ---------------- P4: tf + data vector ----------------
    late = ctx.enter_context(tc.tile_pool(name="late", bufs=1))
    dvec = late.tile([128, LB], F32)
    drep = late.tile([128, LPAD], F32)
    wrep_r = late.tile([128, NL], F32)
    wrep_i = late.tile([128, NL], F32)
    with (
        tc.tile_pool(name="p4_work", bufs=2) as work,
        tc.tile_pool(name="p4_pacc", bufs=1, space="PSUM") as pacc,
        tc.tile_pool(name="p4_pmisc", bufs=1, space="PSUM") as pmisc,
    ):
        Ptf1 = pacc.tile([40, RW], F32, tag="tf_p1")
        Ptf2 = pacc.tile([40, RW], F32, tag="tf_p2")
        for i in range(NB):
            gt = work.tile([128, 80], F32, tag="tf_g")
            nc.sync.dma_start(gt[:], din["gscT"][128*i:128*(i+1), :])
            gtr = work.tile([128, 80], F32R, tag="tf_gr")
            nc.vector.tensor_copy(gtr[:], gt[:])
            st = (i == 0); sp_ = (i == NB - 1)
            nc.tensor.matmul(Ptf1[:], gtr[:, 0:40], BF[i][:, 0:RW],
                             start=st, stop=sp_)
            nc.tensor.matmul(Ptf2[:], gtr[:, 40:80], BF[i][:, 0:RW],
                             start=st, stop=sp_)
        df = work.tile([40, 80], F32, tag="tf_df")
        nc.sync.dma_start(df[:], din["dfpack"][:])
        tfr = work.tile([40, 40], F32, tag="tfr")
        tfi = work.tile([40, 40], F32, tag="tfi")
        nc.vector.tensor_tensor(out=tfr[:], in0=df[:, 0:40],
                                in1=Ptf1[:, 0:40], op=ALU.add)
        nc.vector.tensor_tensor(out=tfr[:], in0=tfr[:],
                                in1=Ptf2[:, 128:168], op=ALU.subtract)
        nc.vector.tensor_tensor(out=tfi[:], in0=df[:, 40:80],
                                in1=Ptf1[:, 128:168], op=ALU.add)
        nc.vector.tensor_tensor(out=tfi[:], in0=tfi[:],
                                in1=Ptf2[:, 0:40], op=ALU.add)
        tfd = work.tile([40, 80], F32, tag="tf_out")
        nc.vector.tensor_copy(tfd[:, 0:40], tfr[:])
        nc.vector.tensor_copy(tfd[:, 40:80], tfi[:])
        nc.sync.dma_start(tfdbg[:], tfd[:])

        pw = work.tile([40, 40], F32, tag="pw")
        nc.vector.tensor_tensor(out=pw[:], in0=tfr[:], in1=tfr[:], op=ALU.mult)
        t1 = work.tile([40, 40], F32, tag="pw_t")
        nc.vector.tensor_tensor(out=t1[:], in0=tfi[:], in1=tfi[:], op=ALU.mult)
        nc.vector.tensor_tensor(out=pw[:], in0=pw[:], in1=t1[:], op=ALU.add)
        amp = work.tile([40, 40], F32, tag="amp")
        nc.scalar.activation(amp[:], pw[:], AF.Sqrt)
        nc.vector.tensor_scalar(out=amp[:], in0=amp[:], scalar1=NOISE,
                                scalar2=None, op0=ALU.add)
        nc.scalar.activation(amp[:], amp[:], AF.Ln)
        tpi = work.tile([40, 40], F32, tag="tpi")
        nc.vector.tensor_scalar(out=tpi[:], in0=amp[:], scalar1=C20L,
                                scalar2=CADD, op0=ALU.mult, op1=ALU.add)
        rec = work.tile([40, 40], F32, tag="rec")
        nc.vector.reciprocal(rec[:], pw[:])
        wr = work.tile([40, 40], F32, tag="wr")
        nc.vector.tensor_tensor(out=wr[:], in0=tfr[:], in1=rec[:], op=ALU.mult)
        nc.vector.tensor_scalar(out=wr[:], in0=wr[:], scalar1=SA, scalar2=None,
                                op0=ALU.mult)
        wi = work.tile([40, 40], F32, tag="wi")
        nc.vector.tensor_tensor(out=wi[:], in0=tfi[:], in1=rec[:], op=ALU.mult)
        nc.vector.tensor_scalar(out=wi[:], in0=wi[:], scalar1=-SA, scalar2=None,
                                op0=ALU.mult)

        def t40(src, name):
            pt = pmisc.tile([40, 40], F32, tag=f"t40p_{name}")
            nc.tensor.matmul(pt[:], src[:], id_s[0:40, 0:40], start=True,
                             stop=True)
            d = work.tile([40, 40], F32, tag=f"t40_{name}")
            nc.vector.tensor_copy(d[:], pt[:])
            return d
        tpiT = t40(tpi, "tpi"); wrT = t40(wr, "wr"); wiT = t40(wi, "wi")

        pack = work.tile([40, 120], F32, tag="pack")
        nc.vector.tensor_copy(pack[:, 0:40], tpiT[:])
        nc.vector.tensor_copy(pack[:, 40:80], wrT[:])
        nc.vector.tensor_copy(pack[:, 80:120], wiT[:])
        kept3 = work.tile([1, 3 * NL], F32, tag="kept3")
        pack3d = pack[:].rearrange("p (a b) -> p a b", a=3)
        kept3d = kept3[:].rearrange("p (a b) -> p a b", a=3)
        for (t, rs_list) in link_groups:
            o = _GBASE[t]
            for (s0, ln) in _contig_segments(rs_list):
                nc.sync.dma_start(kept3d[0:1, :, o:o+ln],
                                  pack3d[t:t+1, :, s0:s0+ln])
                o += ln
        # data = (tpT - tpi_kept)/LOG10E20 on the packed row
        tprow = work.tile([1, NL], F32, tag="tprow")
        nc.sync.dma_start(tprow[:], bass.AP(din["tpT"], 0, [[1, NL]]))
        nc.vector.tensor_tensor(out=kept3[0:1, 0:NL], in0=tprow[:],
                                in1=kept3[0:1, 0:NL], op=ALU.subtract)
        nc.vector.tensor_scalar(out=kept3[0:1, 0:NL], in0=kept3[0:1, 0:NL],
                                scalar1=1.0 / LOG10E20, scalar2=None,
                                op0=ALU.mult)
        nc.sync.dma_start(bass.AP(ddbg, 0, [[1, NL]]), kept3[0:1, 0:NL])
        nc.sync.dma_start(bass.AP(scr["sdram"], 0, [[1, NL]]), kept3[0:1, 0:NL])

        nc.vector.memset(dvec[:], 0.0)
        nc.sync.dma_start(dvec[:, 0:12],
                          bass.AP(scr["sdram"], 0, [[1, 128], [128, 12]]))
        nc.sync.dma_start(dvec[0:24, 12:13],
                          bass.AP(scr["sdram"], 1536, [[1, 24]]))
        nc.vector.memset(drep[:], 0.0)
        nc.gpsimd.partition_broadcast(drep[:, 0:NL], kept3[0:1, 0:NL])
        nc.gpsimd.partition_broadcast(wrep_r[:], kept3[0:1, NL:2*NL])
        nc.gpsimd.partition_broadcast(wrep_i[:], kept3[0:1, 2*NL:3*NL])

    # ---------------- P5: Ht build + v = Ht d ----------------
    vsum = late.tile([128, 2 * NB], F32)
    lam = late.tile([128, 1], F32)
    with tc.tile_pool(name="p5_work", bufs=2) as work:
        nc.vector.memset(vsum[:], 0.0)
        for i in range(NB):
            Gq = work.tile([128, 80], F32, tag="h_gq")
            Iq = work.tile([128, 80], F32, tag="h_iq")
            f_ap = fsc_s[:, i:i+1]
            nc.vector.tensor_scalar(out=Gq[:, 0:40], in0=BF[i][:, 168:208],
                                    scalar1=f_ap, scalar2=None, op0=ALU.mult)
            nc.vector.tensor_scalar(out=Gq[:, 0:40], in0=Gq[:, 0:40],
                                    scalar1=-1.0, scalar2=None, op0=ALU.mult)
            nc.vector.tensor_scalar(out=Gq[:, 40:80], in0=BF[i][:, 40:80],
                                    scalar1=f_ap, scalar2=None, op0=ALU.mult)
            nc.vector.tensor_scalar(out=Iq[:, 0:40], in0=BF[i][:, 128:168],
                                    scalar1=f_ap, scalar2=None, op0=ALU.mult)
            nc.vector.tensor_scalar(out=Iq[:, 0:40], in0=Iq[:, 0:40],
                                    scalar1=-1.0, scalar2=None, op0=ALU.mult)
            nc.vector.tensor_scalar(out=Iq[:, 40:80], in0=BF[i][:, 0:40],
                                    scalar1=f_ap, scalar2=None, op0=ALU.mult)
            Gg_r = work.tile([128, NL], F32, tag="h_ggr")
            Gg_i = work.tile([128, NL], F32, tag="h_ggi")
            qr = work.tile([128, NL], F32, tag="h_qr")
            qi = work.tile([128, NL], F32, tag="h_qi")
            base = 0
            for (t, rs_list) in link_groups:
                o = base
                for (s0, ln) in _contig_segments(rs_list):
                    nc.vector.tensor_copy(Gg_r[:, o:o+ln], Gq[:, s0:s0+ln])
                    nc.vector.tensor_copy(Gg_i[:, o:o+ln], Gq[:, 40+s0:40+s0+ln])
                    o += ln
                base += len(rs_list)
            uniform = (len(link_groups) == 40
                       and all(len(rs) == 39 for _, rs in link_groups))
            if uniform:
                # full-width inc multiply via 0-stride replicated APs
                IncR = Iq[:, 0:40].rearrange("p (t o) -> p t o", o=1
                                             ).broadcast_to([128, 40, 39])
                IncI = Iq[:, 40:80].rearrange("p (t o) -> p t o", o=1
                                              ).broadcast_to([128, 40, 39])
                Gg_r3 = Gg_r[:].rearrange("p (t j) -> p t j", t=40)
                Gg_i3 = Gg_i[:].rearrange("p (t j) -> p t j", t=40)
                qr3 = qr[:].rearrange("p (t j) -> p t j", t=40)
                qi3 = qi[:].rearrange("p (t j) -> p t j", t=40)
                nc.vector.tensor_tensor(out=qr3, in0=Gg_r3, in1=IncR,
                                        op=ALU.mult)
                nc.vector.tensor_tensor(out=qi3, in0=Gg_i3, in1=IncR,
                                        op=ALU.mult)
                nc.vector.tensor_tensor(out=Gg_i3, in0=Gg_i3, in1=IncI,
                                        op=ALU.mult)
                nc.vector.tensor_tensor(out=Gg_r3, in0=Gg_r3, in1=IncI,
                                        op=ALU.mult)
            else:
                base = 0
                for (t, rs_list) in link_groups:
                    sl = slice(base, base + len(rs_list))
                    nc.vector.tensor_scalar(out=qr[:, sl], in0=Gg_r[:, sl],
                                            scalar1=Iq[:, t:t+1], scalar2=None,
                                            op0=ALU.mult)
                    nc.vector.tensor_scalar(out=qi[:, sl], in0=Gg_i[:, sl],
                                            scalar1=Iq[:, t:t+1], scalar2=None,
                                            op0=ALU.mult)
                    nc.vector.tensor_scalar(out=Gg_i[:, sl], in0=Gg_i[:, sl],
                                            scalar1=Iq[:, 40+t:40+t+1],
                                            scalar2=None, op0=ALU.mult)
                    nc.vector.tensor_scalar(out=Gg_r[:, sl], in0=Gg_r[:, sl],
                                            scalar1=Iq[:, 40+t:40+t+1],
                                            scalar2=None, op0=ALU.mult)
                    base += len(rs_list)
            nc.vector.tensor_tensor(out=qr[:], in0=qr[:], in1=Gg_i[:],
                                    op=ALU.subtract)
            nc.vector.tensor_tensor(out=qi[:], in0=qi[:], in1=Gg_r[:],
                                    op=ALU.add)
            hr = work.tile([128, LPAD], F32R, tag="h_hr")
            hi = work.tile([128, LPAD], F32R, tag="h_hi")
            t2 = work.tile([128, NL], F32, tag="h_t2")
            nc.vector.memset(hr[:, NL:LPAD].bitcast(F32), 0.0)
            nc.vector.memset(hi[:, NL:LPAD].bitcast(F32), 0.0)
            nc.vector.tensor_tensor(out=hr[:, 0:NL], in0=qr[:], in1=wrep_r[:],
                                    op=ALU.mult)
            nc.vector.tensor_tensor(out=t2[:], in0=qi[:], in1=wrep_i[:],
                                    op=ALU.mult)
            nc.vector.tensor_tensor(out=hr[:, 0:NL], in0=hr[:, 0:NL], in1=t2[:],
                                    op=ALU.subtract)
            nc.vector.tensor_tensor(out=hi[:, 0:NL], in0=qr[:], in1=wrep_i[:],
                                    op=ALU.mult)
            nc.vector.tensor_tensor(out=t2[:], in0=qi[:], in1=wrep_r[:],
                                    op=ALU.mult)
            nc.vector.tensor_tensor(out=hi[:, 0:NL], in0=hi[:, 0:NL], in1=t2[:],
                                    op=ALU.add)
            nc.vector.tensor_scalar(out=hi[:], in0=hi[:], scalar1=-1.0,
                                    scalar2=None, op0=ALU.mult)
            nc.sync.dma_start(scr["htdram"][128*i:128*(i+1), :], hr[:])
            nc.sync.dma_start(scr["htdram"][N+128*i:N+128*(i+1), :], hi[:])
            nc.vector.tensor_tensor(out=t2[:], in0=hr[:, 0:NL],
                                    in1=drep[:, 0:NL], op=ALU.mult)
            nc.vector.tensor_reduce(vsum[:, i:i+1], t2[:], axis=AXX, op=ALU.add)
            nc.vector.tensor_tensor(out=t2[:], in0=hi[:, 0:NL],
                                    in1=drep[:, 0:NL], op=ALU.mult)
            nc.vector.tensor_reduce(vsum[:, NB+i:NB+i+1], t2[:], axis=AXX,
                                    op=ALU.add)
        vsq = work.tile([128, 2 * NB], F32, tag="vsq")
        nc.vector.tensor_tensor(out=vsq[:], in0=vsum[:], in1=vsum[:],
                                op=ALU.mult)
        vred = work.tile([128, 1], F32, tag="vred")
        nc.vector.tensor_reduce(vred[:], vsq[:], axis=AXX, op=ALU.add)
        nc.gpsimd.partition_all_reduce(vred[:], vred[:], 128,
                                       bass_isa.ReduceOp.add)
        nc.scalar.activation(lam[:], vred[:], AF.Sqrt)
        nc.vector.tensor_scalar(out=lam[:], in0=lam[:], scalar1=float(alpha),
                                scalar2=None, op0=ALU.mult)

    # ---------------- P7: Gram ----------------
    st_ = late.tile([128, LB], F32)
    srep = late.tile([128, LPAD], F32)
    with (
        tc.tile_pool(name="g_acc", bufs=1) as gacc,
        tc.tile_pool(name="g_work", bufs=1) as work,
        tc.tile_pool(name="g_psum", bufs=4, space="PSUM") as pg,
    ):
        GA = [gacc.tile([128, LPAD], F32, tag=f"ga{l}", name=f"ga{l}") for l in range(LB)]
        GRP = 4
        for g0 in range(0, 2 * NB, GRP):
            htrs = []
            for gi in range(GRP):
                ch = g0 + gi
                htr = work.tile([128, LPAD], F32R, tag=f"g_htr{gi}",
                                name=f"g_htr{gi}")
                nc.sync.dma_start(htr[:], scr["htdram"][128*ch:128*(ch+1), :])
                htrs.append(htr)
            for l in range(LB):
                c0 = 128 * l
                for cc in range(c0, LPAD, 416):
                    cw = min(416, LPAD - cc)
                    pgt = pg.tile([128, 416], F32, tag="g_pg")
                    for gi in range(GRP):
                        nc.tensor.matmul(pgt[:, 0:cw],
                                         htrs[gi][:, c0:c0+128],
                                         htrs[gi][:, cc:cc+cw],
                                         start=(gi == 0), stop=(gi == GRP - 1))
                    if g0 == 0:
                        nc.vector.tensor_copy(GA[l][:, cc:cc+cw], pgt[:, 0:cw])
                    else:
                        nc.vector.tensor_tensor(out=GA[l][:, cc:cc+cw],
                                                in0=GA[l][:, cc:cc+cw],
                                                in1=pgt[:, 0:cw], op=ALU.add)
        for l in range(LB):
            nc.sync.dma_start(scr["gramdram"][128*l:128*(l+1), :], GA[l][:])
        gd = work.tile([128, LB], F32, tag="gd")
        nc.sync.dma_start(gd[:], bass.AP(scr["gramdram"], 0,
                                         [[LPAD + 1, 128],
                                          [128 * (LPAD + 1), LB]]))
        nc.vector.tensor_scalar(out=gd[:], in0=gd[:], scalar1=lam[:],
                                scalar2=None, op0=ALU.add)
        nc.scalar.activation(st_[:], gd[:], AF.Sqrt)
        nc.vector.reciprocal(st_[:], st_[:])
        ps_ = pg.tile([LB, 128], F32, tag="s_ps")
        nc.tensor.matmul(ps_[:], st_[:], id_s[:], start=True, stop=True)
        s13 = work.tile([LB, 128], F32, tag="s13")
        nc.vector.tensor_copy(s13[:], ps_[:])
        nc.sync.dma_start(bass.AP(scr["srowdram"], 0, [[1, LPAD]]), s13[:])
        srow = work.tile([1, LPAD], F32, tag="srow")
        nc.sync.dma_start(srow[:], bass.AP(scr["srowdram"], 0, [[1, LPAD]]))
        nc.gpsimd.partition_broadcast(srep[:], srow[:])

    # ---------------- P8: scaled SPD solve ----------------
    bf2_pool = ctx.enter_context(tc.tile_pool(name="bf2", bufs=1))
    BF2 = [bf2_pool.tile([128, 128], F32R, tag=f"bf2_{l}", name=f"bf2_{l}") for l in range(LB)]
    with (
        tc.tile_pool(name="s_tri", bufs=1) as tri2,
        tc.tile_pool(name="s_work", bufs=2) as work,
        tc.tile_pool(name="s_pmm", bufs=2, space="PSUM") as pmm,
        tc.tile_pool(name="s_pmisc", bufs=1, space="PSUM") as pmisc,
    ):
        dsc = work.tile([128, LB], F32, tag="dsc")
        nc.vector.tensor_tensor(out=dsc[:], in0=dvec[:], in1=st_[:], op=ALU.mult)
        zz = work.tile([128, 128], F32, tag="zz")
        nc.vector.memset(zz[:], 0.0)
        for l in range(LB):
            nc.vector.tensor_copy(BF2[l][:], zz[:])
            nc.vector.tensor_copy(BF2[l][:, 0:1], dsc[:, l:l+1])
        GT = {}
        for i in range(LB):
            for j in range(i, LB):
                GT[(i, j)] = tri2.tile([128, 128], F32R, tag=f"g{i}_{j}", name=f"g{i}_{j}")
                gload = work.tile([128, 128], F32, tag="g_load")
                nc.sync.dma_start(gload[:],
                                  scr["gramdram"][128*i:128*(i+1),
                                                  128*j:128*(j+1)])
                nc.vector.tensor_scalar(out=gload[:], in0=gload[:],
                                        scalar1=st_[:, i:i+1], scalar2=None,
                                        op0=ALU.mult)
                nc.vector.tensor_tensor(out=gload[:], in0=gload[:],
                                        in1=srep[:, 128*j:128*(j+1)],
                                        op=ALU.mult)
                if i == j:
                    ones1 = work.tile([128, 1], F32, tag="diag1")
                    nc.vector.memset(ones1[:], 1.0)
                    nc.vector.copy_predicated(gload[:], idu_s[:],
                                              ones1[:].broadcast_to([128, 128]))
                nc.vector.tensor_copy(GT[(i, j)][:], gload[:])
        for k in range(LB):
            V = work.tile([128, 128], F32R, tag="lu2_V")
            _newton_real(nc, work, pmm, pmisc, GT[(k, k)], V, id_s, NEWTON_SPD)
            nc.sync.dma_start(scr["v2dram"][128*k:128*(k+1), :], V[:])
            for i in range(k + 1, LB):
                ptr = pmisc.tile([128, 128], F32R, tag="lu2_ptr")
                nc.tensor.transpose(ptr[:], GT[(k, i)][:], idr_s[:])
                utt = work.tile([128, 128], F32R, tag="lu2_utt")
                nc.vector.tensor_copy(utt[:], ptr[:])
                nc.sync.dma_start(
                    scr["ut2dram"][128*i:128*(i+1), 128*k:128*(k+1)], utt[:])
            for i in range(k + 1, LB):
                pl = pmm.tile([128, 128], F32, tag="cmmp1")
                nc.tensor.matmul(pl[:], V[:], GT[(k, i)][:], start=True,
                                 stop=True)
                LT = work.tile([128, 128], F32R, tag="lu2_LT")
                nc.vector.tensor_copy(LT[:], pl[:])
                pb = pmm.tile([128, 128], F32, tag="cmmp2")
                nc.tensor.matmul(pb[:], LT[:], BF2[k][:], start=True, stop=True)
                nc.vector.tensor_tensor(out=BF2[i][:], in0=BF2[i][:],
                                        in1=pb[:], op=ALU.subtract)
                for j in range(i, LB):
                    pt_ = pmm.tile([128, 128], F32, tag="cmmp1")
                    nc.tensor.matmul(pt_[:], LT[:], GT[(k, j)][:], start=True,
                                     stop=True)
                    nc.vector.tensor_tensor(out=GT[(i, j)][:],
                                            in0=GT[(i, j)][:], in1=pt_[:],
                                            op=ALU.subtract)

    ys = late.tile([128, LB], F32)
    yrep = late.tile([128, LPAD], F32)
    with (
        tc.tile_pool(name="b2_work", bufs=3) as work,
        tc.tile_pool(name="b2_pacc", bufs=1, space="PSUM") as pacc,
        tc.tile_pool(name="b2_pmm", bufs=2, space="PSUM") as pmm,
    ):
        for k in range(LB - 1, -1, -1):
            W = work.tile([128, 128], F32R, tag="bs2_W")
            nc.vector.tensor_copy(W[:], BF2[k][:])
            if k < LB - 1:
                P1 = pacc.tile([128, 128], F32, tag="bs2_p1")
                for idx, j in enumerate(range(k + 1, LB)):
                    utt = work.tile([128, 128], F32R, tag="bs2_utt")
                    nc.sync.dma_start(
                        utt[:], scr["ut2dram"][128*j:128*(j+1),
                                               128*k:128*(k+1)])
                    nc.tensor.matmul(P1[:], utt[:], BF2[j][:],
                                     start=(idx == 0), stop=(j == LB - 1))
                nc.vector.tensor_tensor(out=W[:], in0=W[:], in1=P1[:],
                                        op=ALU.subtract)
            Vk = work.tile([128, 128], F32R, tag="bs2_V")
            nc.sync.dma_start(Vk[:], scr["v2dram"][128*k:128*(k+1), :])
            Pf = pmm.tile([128, 128], F32, tag="bs2_pf")
            nc.tensor.matmul(Pf[:], Vk[:], W[:], start=True, stop=True)
            nc.vector.tensor_copy(BF2[k][:], Pf[:])
        for l in range(LB):
            nc.vector.tensor_copy(ys[:, l:l+1], BF2[l][:, 0:1])
        nc.vector.tensor_tensor(out=ys[:], in0=ys[:], in1=st_[:], op=ALU.mult)
        psy = pmm.tile([LB, 128], F32, tag="y_ps")
        nc.tensor.matmul(psy[:], ys[:], id_s[:], start=True, stop=True)
        y13 = work.tile([LB, 128], F32, tag="y13")
        nc.vector.tensor_copy(y13[:], psy[:])
        nc.sync.dma_start(bass.AP(scr["yrowdram"], 0, [[1, LPAD]]), y13[:])
        yrow = work.tile([1, LPAD], F32, tag="yrow")
        nc.sync.dma_start(yrow[:], bass.AP(scr["yrowdram"], 0, [[1, LPAD]]))
        nc.gpsimd.partition_broadcast(yrep[:], yrow[:])

    # ---------------- P9: chi = Ht y ----------------
    with tc.tile_pool(name="p9_work", bufs=2) as work:
        chi = late.tile([128, 2 * NB], F32)
        for ch in range(2 * NB):
            htc = work.tile([128, LPAD], F32R, tag="c_htc")
            nc.sync.dma_start(htc[:], scr["htdram"][128*ch:128*(ch+1), :])
            tm = work.tile([128, LPAD], F32, tag="c_tm")
            nc.vector.tensor_tensor(out=tm[:], in0=htc[:], in1=yrep[:],
                                    op=ALU.mult)
            nc.vector.tensor_reduce(chi[:, ch:ch+1], tm[:], axis=AXX,
                                    op=ALU.add)
        nc.sync.dma_start(bass.AP(out_chi, 0, [[1, 128], [128, 2 * NB]]),
                          chi[:])
    ctx.close()


_GBASE = {}

def _contig_segments(rs_list):
    segs = []
    s = rs_list[0]; prev = s
    for r in rs_list[1:]:
        if r == prev + 1:
            prev = r
        else:
            segs.append((s, prev - s + 1)); s = r; prev = r
    segs.append((s, prev - s + 1))
    return segs


_CACHED = {}


def kernel(epsilon_r_iter, chi_iter, total_power, alpha, grid_x, grid_y,
           direct_field, incident_field, G_freespace, G_freespace_scaled,
           sensor_links):
    eps = np.asarray(epsilon_r_iter)
    chi_it = np.asarray(chi_iter)
    tp = np.asarray(total_power, dtype=np.float32)
    alpha_f = float(np.asarray(alpha))
    gx = np.asarray(grid_x, dtype=np.float32)
    gy = np.asarray(grid_y, dtype=np.float32)
    df = np.asarray(direct_field)
    einc = np.asarray(incident_field)
    gfs = np.asarray(G_freespace)
    gsc = np.asarray(G_freespace_scaled)
    links = np.asarray(sensor_links)

    x = gx.T.reshape(N).astype(np.float32)
    y = gy.T.reshape(N).astype(np.float32)
    scat = np.real(eps.T.reshape(N)).astype(np.float32)

    geomS = np.stack([np.ones(N, np.float32), -2.0*x, -2.0*y,
                      (x*x + y*y)]).astype(np.float32)
    geomR = np.stack([(x*x + y*y), x, y,
                      np.ones(N, np.float32)]).astype(np.float32)
    scat_t = scat.reshape(NB, 128).T.copy()

    bpack = np.zeros((N, RW), np.float32)
    bpack[:, 0:40] = -einc.real; bpack[:, 40:80] = -gfs.real
    bpack[:, 128:168] = -einc.imag; bpack[:, 168:208] = -gfs.imag
    gscT = np.concatenate([gsc.real.T, gsc.imag.T], axis=1).astype(np.float32)
    dfpack = np.concatenate([df.real, df.imag], axis=1).astype(np.float32)
    tpT = tp.T.copy().astype(np.float32)

    groups = []
    i = 0
    while i < len(links):
        t = int(links[i, 0])
        rs_list = []
        while i < len(links) and int(links[i, 0]) == t:
            rs_list.append(int(links[i, 1]))
            i += 1
        groups.append((t, rs_list))

    _GBASE.clear()
    o = 0
    for (t, rs_list) in groups:
        _GBASE[t] = o
        o += len(rs_list)
    key = (hash(links.tobytes()), alpha_f)
    if key not in _CACHED:
        _CACHED[key] = build_program(groups, alpha_f)
    nc = _CACHED[key]

    id128 = np.eye(128, dtype=np.float32)
    im = {
        "geomS": geomS, "geomR": geomR, "scat_t": scat_t, "bpack": bpack,
        "gscT": gscT, "dfpack": dfpack, "tpT": tpT,
        "id128": id128, "idu8": id128.astype(np.uint8),
    }
    import os as _os
    _tr = _os.environ.get("KTRACE", "0") == "1"
    res = run_bass_kernel_spmd(nc, [im] * 8, core_ids=list(range(8)), trace=_tr)
    out = res.results[0]
    _CACHED["last"] = (res, out)

    chi = out["out_chi"]
    dchi_r = chi[:N].reshape(M, M).T
    dchi_i = chi[N:].reshape(M, M).T
    chi_new = (chi_it + (dchi_r + 1j * dchi_i)).astype(np.complex64)
    return chi_new + 1.0, chi_new



# revision 33
# speedup vs baseline: 1.5315x; 1.0233x over previous
"""DRIM layer (distorted Rytov inverse-scattering iteration) on Trainium2.

One Bass/Tile program per core (replicated SPMD on 8 cores):
  P1  Z-matrix build via large-branch Hankel evaluation (upper triangle only;
      Z is complex-symmetric), resident in SBUF as fp32r planes
  P2  block LDL^T elimination, Newton-iterated 128x128 block inverses,
      fp32r tensor-engine matmuls
  P3  back-substitution -> X = Z^-1 [-E_inc | -G]
  P4  total field, RSS power model, data vector
  P5  Rytov H^T rows (4608 x 1664 padded) + H^T d
  P7  Gram H H^T (upper blocks) + Jacobi scaling
  P8  scaled SPD block solve (same Newton machinery, real)
  P9  chi = H^T y, output dchi

Host does input packing / output reshape only.
"""
import math
import os
import numpy as np

import concourse.bass as bass
import concourse.bacc as bacc
import concourse.bass_isa as bass_isa
import concourse.mybir as mybir
import concourse.tile as tile
from concourse.bass_utils import run_bass_kernel_spmd

F32 = mybir.dt.float32
F32R = mybir.dt.float32r
U8 = mybir.dt.uint8
AF = mybir.ActivationFunctionType
ALU = mybir.AluOpType
AXX = mybir.AxisListType.X

M = 48
N = M * M
NB = N // 128               # 18
TX = RX = 40
NL = TX * (RX - 1)          # 1560
LPAD = 1664
LB = LPAD // 128            # 13
RW = 256                    # [0:128]=Re plane, [128:256]=Im plane
CW = 256                    # Z-build column chunk
DOI = 3.0
WL = 0.125
K0 = 2.0 * math.pi / WL
IMP = 120.0 * math.pi
GRID_LEN = DOI / M
GRID_RADIUS = math.sqrt(GRID_LEN ** 2 / math.pi)
NOISE = 1e-6

def _j1s(x):
    t2 = (x / 3.0) ** 2
    return x * (0.5 - 0.56249985*t2 + 0.21093573*t2**2 - 0.03954289*t2**3
                + 0.00443319*t2**4 - 0.00031761*t2**5 + 0.00001109*t2**6)

def _y1s(x):
    t2 = (x / 3.0) ** 2
    p = (-0.6366198 + 0.2212091*t2 + 2.1682709*t2**2 - 1.3164827*t2**3
         + 0.3123951*t2**4 - 0.0400976*t2**5 + 0.0027873*t2**6)
    return ((2.0/math.pi) * x * math.log(0.5*x) * _j1s(x) + p) / x

X0C = K0 * GRID_RADIUS
GRID_AREA = 4.0*math.pi*GRID_RADIUS/(2.0*K0) * _j1s(X0C)
C1 = -IMP * math.pi * GRID_RADIUS / 2.0
C2 = _j1s(X0C)
C3R, C3I = _j1s(X0C), _y1s(X0C)
C1C2 = C1 * C2
ZD_RE = C1 * C3R
ZD_IM_C = C1 * C3I
SA = GRID_AREA * K0 * K0
TWO_PI = 2.0 * math.pi
INV_2PI = 1.0 / TWO_PI
LOG10E20 = 20.0 * math.log10(math.e)
CADD = 10.0 * math.log10(WL * WL / (4.0 * math.pi * IMP) / 1e-3)
C20L = 20.0 / math.log(10.0)

F0C = [0.79788456, -0.00000077, -0.00552740, -0.00009512,
       0.00137237, -0.00072805, 0.00014476]
THC = [-0.78539816, -0.04166397, -0.00003954, 0.00262573,
       -0.00054125, -0.00029333, 0.00013558]
F0CS = [c * (3.0 ** k) * C1C2 for k, c in enumerate(F0C)]
THCS = [c * (3.0 ** k) for k, c in enumerate(THC)]
# short-series Z build: th = x + 3*THC[1]/x ; amp = C1C2*(F0C0 + 3*F0C1/x)/sqrt(x)
TH1 = 3.0 * THC[1]
A0C = C1C2 * F0C[0]
A1C = 3.0 * C1C2 * F0C[1]
K0K0 = K0 * K0
PI4 = math.pi / 4.0
# short-series Z build: th = x - pi/4 + 3*THC[1]*rx ; amp = C1C2*(F0C0 + 3*F0C1*rx)/sqrt(x)
TH1 = 3.0 * THC[1]
A0C = C1C2 * F0C[0]
A1C = 3.0 * C1C2 * F0C[1]
K0K0 = K0 * K0
PI4 = math.pi / 4.0

NEWTON_Z = 13
NEWTON_SPD = 9


def _horner(nc, out_ap, s_ap, coeffs):
    cs = coeffs[::-1]
    nc.vector.tensor_scalar(out=out_ap, in0=s_ap, scalar1=float(cs[0]),
                            scalar2=float(cs[1]), op0=ALU.mult, op1=ALU.add)
    for c in cs[2:]:
        nc.vector.tensor_tensor(out=out_ap, in0=out_ap, in1=s_ap, op=ALU.mult)
        nc.vector.tensor_scalar(out=out_ap, in0=out_ap, scalar1=float(c),
                                scalar2=None, op0=ALU.add)


def _cmm(nc, pool, lhsT, rhs, n=RW):
    P1 = pool.tile([128, n], F32, tag="cmmp1")
    P2 = pool.tile([128, n], F32, tag="cmmp2")
    nc.tensor.matmul(P1[:], lhsT[:, 0:128], rhs, start=True, stop=True)
    nc.tensor.matmul(P2[:], lhsT[:, 128:256], rhs, start=True, stop=True)
    return P1, P2


def _combine_sub(nc, dst, P1, P2):
    nc.vector.tensor_tensor(out=dst[:, 0:256], in0=dst[:, 0:256],
                            in1=P1[:, 0:256], op=ALU.subtract)
    nc.vector.tensor_tensor(out=dst[:, 0:128], in0=dst[:, 0:128],
                            in1=P2[:, 128:256], op=ALU.add)
    nc.vector.tensor_tensor(out=dst[:, 128:256], in0=dst[:, 128:256],
                            in1=P2[:, 0:128], op=ALU.subtract)


def _combine_set(nc, dst, P1, P2):
    nc.vector.tensor_copy(dst[:, 0:256], P1[:, 0:256])
    nc.vector.tensor_tensor(out=dst[:, 0:128], in0=dst[:, 0:128],
                            in1=P2[:, 128:256], op=ALU.subtract)
    nc.vector.tensor_tensor(out=dst[:, 128:256], in0=dst[:, 128:256],
                            in1=P2[:, 0:128], op=ALU.add)


def _newton_scale(nc, work, pmisc, m, tag):
    """1/(colmax * rowmax) of m [128,128] -> [128,1] fp32 AP."""
    ones = work.tile([128, 1], F32, tag=f"nwo_{tag}")
    nc.vector.memset(ones[:], 1.0)
    pc = pmisc.tile([128, 1], F32, tag=f"nwpc_{tag}")
    nc.tensor.matmul(pc[:], m[:], ones[:], start=True, stop=True)
    pr = pmisc.tile([1, 128], F32, tag=f"nwpr_{tag}")
    nc.tensor.matmul(pr[:], ones[:], m[:], start=True, stop=True)
    cs = work.tile([128, 1], F32, tag=f"nwcs_{tag}")
    nc.vector.tensor_copy(cs[:], pc[:])
    rs = work.tile([1, 128], F32, tag=f"nwrs_{tag}")
    nc.vector.tensor_copy(rs[:], pr[:])
    nc.gpsimd.partition_all_reduce(cs[:], cs[:], 128, bass_isa.ReduceOp.max)
    rmax = work.tile([1, 1], F32, tag=f"nwrm_{tag}")
    nc.vector.tensor_reduce(rmax[:], rs[:], axis=AXX, op=ALU.max)
    rmax_b = work.tile([128, 1], F32, tag=f"nwrb_{tag}")
    nc.gpsimd.partition_broadcast(rmax_b[:], rmax[:])
    a = work.tile([128, 1], F32, tag=f"nwa_{tag}")
    nc.vector.tensor_tensor(out=a[:], in0=cs[:], in1=rmax_b[:], op=ALU.mult)
    nc.vector.reciprocal(a[:], a[:])
    return a


def _newton_cplx(nc, work, pmm, pmisc, D, Xout, id2_s, iters):
    """Swap-free Newton: maintains X=[Xr|Xi] and Xs=[-Xi|Xr].
    D@X via lhsT=Dr,rhs=X + lhsT=Di,rhs=Xs (both width-256, PSUM acc).
    """
    m = work.tile([128, 128], F32, tag="nw_m")
    m2 = work.tile([128, 128], F32, tag="nw_m2")
    nc.scalar.activation(m[:], D[:, 0:128], AF.Abs)
    nc.scalar.activation(m2[:], D[:, 128:256], AF.Abs)
    nc.vector.tensor_tensor(out=m[:], in0=m[:], in1=m2[:], op=ALU.max)
    a = _newton_scale(nc, work, pmisc, m, "c")
    nc.vector.tensor_scalar(out=Xout[:, 0:128], in0=D[:, 0:128], scalar1=a[:],
                            scalar2=None, op0=ALU.mult)
    Xs = work.tile([128, RW], F32R, tag="nw_Xs")
    nc.vector.tensor_scalar(out=Xs[:, 0:128], in0=D[:, 128:256], scalar1=a[:],
                            scalar2=None, op0=ALU.mult)
    # Xi = -Di*a = -(Xs lo)
    nc.vector.tensor_scalar(out=Xout[:, 128:256], in0=Xs[:, 0:128],
                            scalar1=-1.0, scalar2=None, op0=ALU.mult)
    nc.scalar.copy(Xs[:, 128:256], Xout[:, 0:128])
    R = work.tile([128, RW], F32R, tag="nw_R")
    Rs = work.tile([128, RW], F32R, tag="nw_Rs")
    for _ in range(iters):
        P = pmm.tile([128, RW], F32, tag="cmmp1")
        nc.tensor.matmul(P[:], D[:, 0:128], Xout[:, 0:RW], start=True,
                         stop=False)
        nc.tensor.matmul(P[:], D[:, 128:256], Xs[:, 0:RW], start=False,
                         stop=True)
        # R = [I|0] - P ;  Rs = [-Ri|Rr] = [Pi | Rr]
        nc.vector.tensor_tensor(out=R[:], in0=id2_s[:], in1=P[:],
                                op=ALU.subtract)
        nc.scalar.copy(Rs[:, 0:128], P[:, 128:256])
        nc.scalar.copy(Rs[:, 128:256], R[:, 0:128])
        Q = pmm.tile([128, RW], F32, tag="cmmp2")
        nc.tensor.matmul(Q[:], Xout[:, 0:128], R[:, 0:RW], start=True,
                         stop=False)
        nc.tensor.matmul(Q[:], Xout[:, 128:256], Rs[:, 0:RW], start=False,
                         stop=True)
        # X += Q ; Xs_lo -= Qi ; Xs_hi = new Xr
        nc.vector.tensor_tensor(out=Xout[:, 0:256], in0=Xout[:, 0:256],
                                in1=Q[:, 0:256], op=ALU.add)
        nc.vector.tensor_tensor(out=Xs[:, 0:128], in0=Xs[:, 0:128],
                                in1=Q[:, 128:256], op=ALU.subtract)
        nc.scalar.copy(Xs[:, 128:256], Xout[:, 0:128])


def _newton_real(nc, work, pmm, pmisc, D, Xout, id_s, iters):
    m = work.tile([128, 128], F32, tag="nw_m")
    nc.scalar.activation(m[:], D[:], AF.Abs)
    a = _newton_scale(nc, work, pmisc, m, "r")
    nc.vector.tensor_scalar(out=Xout[:], in0=D[:], scalar1=a[:], scalar2=None,
                            op0=ALU.mult)
    R = work.tile([128, 128], F32R, tag="nw_R")
    for _ in range(iters):
        P1 = pmm.tile([128, 128], F32, tag="cmmp1")
        nc.tensor.matmul(P1[:], D[:], Xout[:], start=True, stop=True)
        nc.vector.tensor_tensor(out=R[:], in0=id_s[:], in1=P1[:],
                                op=ALU.subtract)
        Q1 = pmm.tile([128, 128], F32, tag="cmmp2")
        nc.tensor.matmul(Q1[:], Xout[:], R[:], start=True, stop=True)
        nc.vector.tensor_tensor(out=Xout[:], in0=Xout[:], in1=Q1[:], op=ALU.add)


def build_program(link_groups, alpha):
    nc = bacc.Bacc("TRN2", target_bir_lowering=False, num_devices=8)
    din = {}
    def inp(name, shape, dtype=F32):
        din[name] = nc.dram_tensor(name, shape, dtype, kind="ExternalInput")
    inp("geomS", [4, N]); inp("geomR", [4, N]); inp("scat_t", [128, NB])
    inp("bpack", [N, RW]); inp("gscT", [N, 80]); inp("dfpack", [40, 80])
    inp("tpT", [40, RX - 1]); inp("id128", [128, 128]); inp("idu8", [128, 128], U8)
    out_chi = nc.dram_tensor("out_chi", [2 * N], F32, kind="ExternalOutput")
    xdbg = nc.dram_tensor("xdbg", [N, RW], F32, kind="ExternalOutput")
    tfdbg = nc.dram_tensor("tfdbg", [40, 80], F32, kind="ExternalOutput")
    ddbg = nc.dram_tensor("ddbg", [40, RX - 1], F32, kind="ExternalOutput")
    scr = {}
    scr["vdram"] = nc.dram_tensor("vdram", [NB * 128, RW], F32R, kind="Internal")
    scr["utdram"] = nc.dram_tensor("utdram", [N, 2 * N], F32R, kind="Internal")
    scr["htdram"] = nc.dram_tensor("htdram", [2 * N, LPAD], F32R, kind="Internal")
    scr["gramdram"] = nc.dram_tensor("gramdram", [LPAD, LPAD], F32, kind="Internal")
    scr["v2dram"] = nc.dram_tensor("v2dram", [LB * 128, 128], F32R, kind="Internal")
    scr["ut2dram"] = nc.dram_tensor("ut2dram", [LPAD, LPAD], F32R, kind="Internal")
    scr["sdram"] = nc.dram_tensor("sdram", [NL], F32, kind="Internal")
    scr["wdram"] = nc.dram_tensor("wdram", [2 * NL], F32, kind="Internal")
    scr["srowdram"] = nc.dram_tensor("srowdram", [LPAD], F32, kind="Internal")
    scr["yrowdram"] = nc.dram_tensor("yrowdram", [LPAD], F32, kind="Internal")

    with tile.TileContext(nc) as tc:
        _body(nc, tc, din, out_chi, xdbg, tfdbg, ddbg, scr, link_groups, alpha)
    nc.compile()
    return nc


def _body(nc, tc, din, out_chi, xdbg, tfdbg, ddbg, scr, link_groups, alpha):
    import contextlib
    ctx = contextlib.ExitStack()
    consts = ctx.enter_context(tc.tile_pool(name="consts", bufs=1))
    id_s = consts.tile([128, 128], F32)
    nc.sync.dma_start(id_s[:], din["id128"][:])
    idr_s = consts.tile([128, 128], F32R)
    nc.vector.tensor_copy(idr_s[:], id_s[:])
    idu_s = consts.tile([128, 128], U8)
    nc.sync.dma_start(idu_s[:], din["idu8"][:])
    id2_s = consts.tile([128, RW], F32)
    nc.vector.memset(id2_s[:], 0.0)
    nc.vector.tensor_copy(id2_s[:, 0:128], id_s[:])
    scat_s = consts.tile([128, NB], F32)
    nc.sync.dma_start(scat_s[:], din["scat_t"][:])

    zdi_s = consts.tile([128, NB], F32)
    fsc_s = consts.tile([128, NB], F32)
    t0 = consts.tile([128, NB], F32)
    nc.vector.tensor_scalar(out=t0[:], in0=scat_s[:], scalar1=-1.0,
                            scalar2=None, op0=ALU.add)
    nc.vector.reciprocal(t0[:], t0[:])
    nc.vector.tensor_scalar(out=fsc_s[:], in0=t0[:], scalar1=(IMP / K0),
                            scalar2=None, op0=ALU.mult)
    nc.vector.tensor_tensor(out=t0[:], in0=t0[:], in1=scat_s[:], op=ALU.mult)
    nc.vector.tensor_scalar(out=zdi_s[:], in0=t0[:], scalar1=-(IMP / K0),
                            scalar2=ZD_IM_C, op0=ALU.mult, op1=ALU.add)
    zdr_c = consts.tile([128, 1], F32)
    nc.vector.memset(zdr_c[:], float(ZD_RE))

    bf_pool = ctx.enter_context(tc.tile_pool(name="bf", bufs=1))
    BF = [bf_pool.tile([128, RW], F32R, tag=f"bf{i}", name=f"bf{i}") for i in range(NB)]

    with tc.tile_pool(name="tri", bufs=1) as tri:
        ZT = {}
        for i in range(NB):
            for j in range(i, NB):
                ZT[(i, j)] = tri.tile([128, RW], F32R, tag=f"z{i}_{j}", name=f"z{i}_{j}")

        # ---------------- P1: Z build ----------------
        with (
            tc.tile_pool(name="zb_geom", bufs=2) as gpool,
            tc.tile_pool(name="zb_work", bufs=1) as work,
            tc.tile_pool(name="zb_psum", bufs=2, space="PSUM") as pz,
        ):
            for k in range(NB):
                r0 = 128 * k
                gS = gpool.tile([4, 128], F32, tag="gS", name="gS")
                nc.sync.dma_start(gS[:], din["geomS"][:, r0:r0+128])
                chunks = []
                j = k
                while j < NB:
                    w = 256 if j + 1 < NB else 128
                    chunks.append((j, w))
                    j += w // 128

                def stage_aps(j, w):
                    # r1/r2 staged in the ZT tiles themselves (scratch reuse)
                    if w == 256:
                        return ZT[(k, j)][:, 0:256], ZT[(k, j + 1)][:, 0:256]
                    return ZT[(k, j)][:, 0:128], ZT[(k, j)][:, 128:256]

                for b0 in range(0, len(chunks), 5):
                    batch = chunks[b0:b0+5]
                    amps = []
                    # pass A: sqrt act-table (+Copy for int round-trips)
                    for ci, (j, w) in enumerate(batch):
                        c0 = 128 * j
                        gR = gpool.tile([4, CW], F32, tag="gR", name="gR")
                        nc.sync.dma_start(gR[:, 0:w], din["geomR"][:, c0:c0+w])
                        pd = pz.tile([128, CW], F32, tag="zb_pd")
                        nc.tensor.matmul(pd[:, 0:w], gS[:], gR[:, 0:w],
                                         start=True, stop=True)
                        ts1 = work.tile([128, CW], F32, tag="zb_ts1")
                        nc.vector.tensor_scalar(out=ts1[:, 0:w], in0=pd[:, 0:w],
                                                scalar1=0.002,
                                                scalar2=float(K0K0),
                                                op0=ALU.max, op1=ALU.mult)
                        xf = work.tile([128, CW], F32, tag="zb_xf")
                        nc.scalar.activation(xf[:, 0:w], ts1[:, 0:w], AF.Sqrt)
                        rx = work.tile([128, CW], F32, tag="zb_rx")
                        nc.vector.reciprocal(rx[:, 0:w], xf[:, 0:w])
                        th = work.tile([128, CW], F32, tag="zb_ts1")
                        nc.vector.scalar_tensor_tensor(
                            out=th[:, 0:w], in0=rx[:, 0:w], scalar=float(TH1),
                            in1=xf[:, 0:w], op0=ALU.mult, op1=ALU.add)
                        srx = work.tile([128, CW], F32, tag="zb_xf")
                        nc.scalar.activation(srx[:, 0:w], rx[:, 0:w], AF.Sqrt)
                        r1ap, r2ap = stage_aps(j, w)
                        ki = work.tile([128, CW], mybir.dt.int32, tag="zb_ki")
                        mf = work.tile([128, CW], F32, tag="zb_mf")
                        # r1 = (th - pi/4) - 2pi*round((th - pi/4)/2pi)
                        nc.scalar.activation(ki[:, 0:w], th[:, 0:w], AF.Copy,
                                             scale=float(INV_2PI),
                                             bias=-0.125)
                        nc.scalar.activation(mf[:, 0:w], ki[:, 0:w], AF.Copy,
                                             bias=0.125)
                        nc.vector.scalar_tensor_tensor(
                            out=r1ap, in0=mf[:, 0:w],
                            scalar=float(-TWO_PI), in1=th[:, 0:w],
                            op0=ALU.mult, op1=ALU.add)
                        # r2 = (th + pi/4) - 2pi*round((th + pi/4)/2pi)
                        nc.scalar.activation(ki[:, 0:w], th[:, 0:w], AF.Copy,
                                             scale=float(INV_2PI),
                                             bias=0.125)
                        nc.scalar.activation(mf[:, 0:w], ki[:, 0:w], AF.Copy,
                                             bias=-0.125)
                        nc.vector.scalar_tensor_tensor(
                            out=r2ap, in0=mf[:, 0:w],
                            scalar=float(-TWO_PI), in1=th[:, 0:w],
                            op0=ALU.mult, op1=ALU.add)
                        f0t = work.tile([128, CW], F32, tag="zb_ts1")
                        nc.vector.tensor_scalar(out=f0t[:, 0:w],
                                                in0=rx[:, 0:w],
                                                scalar1=float(A1C),
                                                scalar2=float(A0C),
                                                op0=ALU.mult, op1=ALU.add)
                        amp = work.tile([128, CW], F32, tag=f"zb_amp{ci}",
                                        name=f"zb_amp{ci}")
                        nc.vector.tensor_tensor(out=amp[:, 0:w],
                                                in0=f0t[:, 0:w],
                                                in1=srx[:, 0:w], op=ALU.mult)
                        amps.append(amp)
                    # pass B: trig act-table
                    for ci, (j, w) in enumerate(batch):
                        amp = amps[ci]
                        r1ap, r2ap = stage_aps(j, w)
                        sinr = work.tile([128, CW], F32, tag="zb_rx")
                        nc.scalar.activation(sinr[:, 0:w], r1ap, AF.Sin)
                        cosr = work.tile([128, CW], F32, tag="zb_xf")
                        nc.scalar.activation(cosr[:, 0:w], r2ap, AF.Sin)
                        if j == k:
                            # diag block: predicate on F32 staging, then copy
                            stg = work.tile([128, CW], F32, tag="zb_mf")
                            nc.vector.tensor_tensor(
                                out=stg[:, 0:128], in0=amp[:, 0:128],
                                in1=cosr[:, 0:128], op=ALU.mult)
                            nc.vector.tensor_tensor(
                                out=stg[:, 128:256], in0=amp[:, 0:128],
                                in1=sinr[:, 0:128], op=ALU.mult)
                            nc.vector.copy_predicated(
                                stg[:, 0:128], idu_s[:],
                                zdr_c[:].broadcast_to([128, 128]))
                            nc.vector.copy_predicated(
                                stg[:, 128:256], idu_s[:],
                                zdi_s[:, k:k+1].broadcast_to([128, 128]))
                            nc.vector.tensor_copy(ZT[(k, k)][:, 0:256],
                                                  stg[:, 0:256])
                            bstart = 1
                        else:
                            bstart = 0
                        for b in range(bstart, w // 128):
                            sl = slice(128 * b, 128 * b + 128)
                            nc.vector.tensor_tensor(
                                out=ZT[(k, j + b)][:, 0:128],
                                in0=amp[:, sl], in1=cosr[:, sl], op=ALU.mult)
                            nc.vector.tensor_tensor(
                                out=ZT[(k, j + b)][:, 128:256],
                                in0=amp[:, sl], in1=sinr[:, sl], op=ALU.mult)

        # ---------------- P2: block LDL^T ----------------
        with (
            tc.tile_pool(name="lu_big", bufs=1) as work,
            tc.tile_pool(name="lu_sm", bufs=2) as wsm,
            tc.tile_pool(name="lu_pmm", bufs=1, space="PSUM") as pmm,
            tc.tile_pool(name="lu_pmisc", bufs=1, space="PSUM") as pmisc,
        ):
            ldtmp0 = wsm.tile([128, RW], F32, tag="ldtmp")
            for i in range(NB):
                nc.sync.dma_start(ldtmp0[:], din["bpack"][128*i:128*(i+1), :])
                nc.vector.tensor_copy(BF[i][:], ldtmp0[:])
                ldtmp0 = wsm.tile([128, RW], F32, tag="ldtmp")
            IC = 3
            for k in range(NB):
                V = work.tile([128, RW], F32R, tag="lu_V")
                _newton_cplx(nc, work, pmm, pmisc, ZT[(k, k)], V, id2_s,
                             NEWTON_Z)
                nc.sync.dma_start(scr["vdram"][128*k:128*(k+1), :], V[:])
                for i in range(k + 1, NB):
                    ptr = pmisc.tile([128, 128], F32R, tag="lu_ptr")
                    nc.tensor.transpose(ptr[:], ZT[(k, i)][:, 0:128], idr_s[:])
                    utt = wsm.tile([128, RW], F32R, tag="lu_utt")
                    nc.vector.tensor_copy(utt[:, 0:128], ptr[:])
                    pti = pmisc.tile([128, 128], F32R, tag="lu_pti")
                    nc.tensor.transpose(pti[:], ZT[(k, i)][:, 128:256], idr_s[:])
                    nc.vector.tensor_copy(utt[:, 128:256], pti[:])
                    nc.sync.dma_start(
                        scr["utdram"][128*i:128*(i+1), 256*k:256*(k+1)], utt[:])
                if k == NB - 1:
                    continue
                bswap = work.tile([128, RW], F32R, tag="lu_bs")
                nc.scalar.mul(bswap[:, 0:128], BF[k][:, 128:256], -1.0)
                nc.scalar.copy(bswap[:, 128:256], BF[k][:, 0:128])
                for a in range(k + 1, NB, IC):
                    b = min(a + IC, NB)
                    LTs = {}
                    for j in range(a, NB):
                        zsw = wsm.tile([128, RW], F32R, tag="lu_zsw")
                        nc.scalar.mul(zsw[:, 0:128],
                                      ZT[(k, j)][:, 128:256], -1.0)
                        nc.scalar.copy(zsw[:, 128:256], ZT[(k, j)][:, 0:128])
                        if j < b:
                            pl = pmm.tile([128, RW], F32, tag="cmmp1")
                            nc.tensor.matmul(pl[:], V[:, 0:128],
                                             ZT[(k, j)][:, 0:RW],
                                             start=True, stop=False)
                            nc.tensor.matmul(pl[:], V[:, 128:256],
                                             zsw[:, 0:RW],
                                             start=False, stop=True)
                            LT = work.tile([128, RW], F32R,
                                           tag=f"lu_LT{j - a}",
                                           name=f"lu_LT{j - a}")
                            nc.scalar.copy(LT[:], pl[:])
                            LTs[j] = LT
                            pb = pmm.tile([128, RW], F32, tag="cmmp2")
                            nc.tensor.matmul(pb[:], LT[:, 0:128],
                                             BF[k][:, 0:RW],
                                             start=True, stop=False)
                            nc.tensor.matmul(pb[:], LT[:, 128:256],
                                             bswap[:, 0:RW],
                                             start=False, stop=True)
                            nc.vector.tensor_tensor(out=BF[j][:, 0:256],
                                                    in0=BF[j][:, 0:256],
                                                    in1=pb[:, 0:256],
                                                    op=ALU.subtract)
                        for i in range(a, min(b, j + 1)):
                            pu = pmm.tile([128, RW], F32,
                                          tag=f"updp{(i - a) % 2}",
                                          name=f"updp{(i - a) % 2}")
                            nc.tensor.matmul(pu[:], LTs[i][:, 0:128],
                                             ZT[(k, j)][:, 0:RW],
                                             start=True, stop=False)
                            nc.tensor.matmul(pu[:], LTs[i][:, 128:256],
                                             zsw[:, 0:RW],
                                             start=False, stop=True)
                            nc.vector.tensor_tensor(out=ZT[(i, j)][:, 0:256],
                                                    in0=ZT[(i, j)][:, 0:256],
                                                    in1=pu[:, 0:256],
                                                    op=ALU.subtract)

    # ---------------- P3: back-substitution ----------------
    with (
        tc.tile_pool(name="bs_work", bufs=3) as work,
        tc.tile_pool(name="bs_pacc", bufs=1, space="PSUM") as pacc,
        tc.tile_pool(name="bs_pmm", bufs=2, space="PSUM") as pmm,
    ):
        for k in range(NB - 1, -1, -1):
            W = work.tile([128, RW], F32R, tag="bs_W")
            nc.vector.tensor_copy(W[:], BF[k][:])
            if k < NB - 1:
                P1 = pacc.tile([128, RW], F32, tag="bs_p1")
                P2 = pacc.tile([128, RW], F32, tag="bs_p2")
                for idx, j in enumerate(range(k + 1, NB)):
                    utt = work.tile([128, RW], F32R, tag="bs_utt")
                    nc.sync.dma_start(
                        utt[:], scr["utdram"][128*j:128*(j+1), 256*k:256*(k+1)])
                    st = (idx == 0); sp_ = (j == NB - 1)
                    nc.tensor.matmul(P1[:], utt[:, 0:128], BF[j][:, 0:RW],
                                     start=st, stop=sp_)
                    nc.tensor.matmul(P2[:], utt[:, 128:256], BF[j][:, 0:RW],
                                     start=st, stop=sp_)
                _combine_sub(nc, W, P1, P2)
            Vk = work.tile([128, RW], F32R, tag="bs_V")
            nc.sync.dma_start(Vk[:], scr["vdram"][128*k:128*(k+1), :])
            P1, P2 = _cmm(nc, pmm, Vk, W[:, 0:RW])
            _combine_set(nc, BF[k], P1, P2)
            nc.sync.dma_start(xdbg[128*k:128*(k+1), :], BF[k][:].bitcast(F32))

    # ---------------- P4: tf + data vector ----------------
    late = ctx.enter_context(tc.tile_pool(name="late", bufs=1))
    dvec = late.tile([128, LB], F32)
    drep = late.tile([128, LPAD], F32)
    wrep_r = late.tile([128, NL], F32)
    wrep_i = late.tile([128, NL], F32)
    with (
        tc.tile_pool(name="p4_work", bufs=2) as work,
        tc.tile_pool(name="p4_pacc", bufs=1, space="PSUM") as pacc,
        tc.tile_pool(name="p4_pmisc", bufs=1, space="PSUM") as pmisc,
    ):
        Ptf1 = pacc.tile([40, RW], F32, tag="tf_p1")
        Ptf2 = pacc.tile([40, RW], F32, tag="tf_p2")
        for i in range(NB):
            gt = work.tile([128, 80], F32, tag="tf_g")
            nc.sync.dma_start(gt[:], din["gscT"][128*i:128*(i+1), :])
            gtr = work.tile([128, 80], F32R, tag="tf_gr")
            nc.vector.tensor_copy(gtr[:], gt[:])
            st = (i == 0); sp_ = (i == NB - 1)
            nc.tensor.matmul(Ptf1[:], gtr[:, 0:40], BF[i][:, 0:RW],
                             start=st, stop=sp_)
            nc.tensor.matmul(Ptf2[:], gtr[:, 40:80], BF[i][:, 0:RW],
                             start=st, stop=sp_)
        df = work.tile([40, 80], F32, tag="tf_df")
        nc.sync.dma_start(df[:], din["dfpack"][:])
        tfr = work.tile([40, 40], F32, tag="tfr")
        tfi = work.tile([40, 40], F32, tag="tfi")
        nc.vector.tensor_tensor(out=tfr[:], in0=df[:, 0:40],
                                in1=Ptf1[:, 0:40], op=ALU.add)
        nc.vector.tensor_tensor(out=tfr[:], in0=tfr[:],
                                in1=Ptf2[:, 128:168], op=ALU.subtract)
        nc.vector.tensor_tensor(out=tfi[:], in0=df[:, 40:80],
                                in1=Ptf1[:, 128:168], op=ALU.add)
        nc.vector.tensor_tensor(out=tfi[:], in0=tfi[:],
                                in1=Ptf2[:, 0:40], op=ALU.add)
        tfd = work.tile([40, 80], F32, tag="tf_out")
        nc.vector.tensor_copy(tfd[:, 0:40], tfr[:])
        nc.vector.tensor_copy(tfd[:, 40:80], tfi[:])
        nc.sync.dma_start(tfdbg[:], tfd[:])

        pw = work.tile([40, 40], F32, tag="pw")
        nc.vector.tensor_tensor(out=pw[:], in0=tfr[:], in1=tfr[:], op=ALU.mult)
        t1 = work.tile([40, 40], F32, tag="pw_t")
        nc.vector.tensor_tensor(out=t1[:], in0=tfi[:], in1=tfi[:], op=ALU.mult)
        nc.vector.tensor_tensor(out=pw[:], in0=pw[:], in1=t1[:], op=ALU.add)
        amp = work.tile([40, 40], F32, tag="amp")
        nc.scalar.activation(amp[:], pw[:], AF.Sqrt)
        nc.vector.tensor_scalar(out=amp[:], in0=amp[:], scalar1=NOISE,
                                scalar2=None, op0=ALU.add)
        nc.scalar.activation(amp[:], amp[:], AF.Ln)
        tpi = work.tile([40, 40], F32, tag="tpi")
        nc.vector.tensor_scalar(out=tpi[:], in0=amp[:], scalar1=C20L,
                                scalar2=CADD, op0=ALU.mult, op1=ALU.add)
        rec = work.tile([40, 40], F32, tag="rec")
        nc.vector.reciprocal(rec[:], pw[:])
        wr = work.tile([40, 40], F32, tag="wr")
        nc.vector.tensor_tensor(out=wr[:], in0=tfr[:], in1=rec[:], op=ALU.mult)
        nc.vector.tensor_scalar(out=wr[:], in0=wr[:], scalar1=SA, scalar2=None,
                                op0=ALU.mult)
        wi = work.tile([40, 40], F32, tag="wi")
        nc.vector.tensor_tensor(out=wi[:], in0=tfi[:], in1=rec[:], op=ALU.mult)
        nc.vector.tensor_scalar(out=wi[:], in0=wi[:], scalar1=-SA, scalar2=None,
                                op0=ALU.mult)

        def t40(src, name):
            pt = pmisc.tile([40, 40], F32, tag=f"t40p_{name}")
            nc.tensor.matmul(pt[:], src[:], id_s[0:40, 0:40], start=True,
                             stop=True)
            d = work.tile([40, 40], F32, tag=f"t40_{name}")
            nc.vector.tensor_copy(d[:], pt[:])
            return d
        tpiT = t40(tpi, "tpi"); wrT = t40(wr, "wr"); wiT = t40(wi, "wi")

        pack = work.tile([40, 120], F32, tag="pack")
        nc.vector.tensor_copy(pack[:, 0:40], tpiT[:])
        nc.vector.tensor_copy(pack[:, 40:80], wrT[:])
        nc.vector.tensor_copy(pack[:, 80:120], wiT[:])
        kept3 = work.tile([1, 3 * NL], F32, tag="kept3")
        pack3d = pack[:].rearrange("p (a b) -> p a b", a=3)
        kept3d = kept3[:].rearrange("p (a b) -> p a b", a=3)
        for (t, rs_list) in link_groups:
            o = _GBASE[t]
            for (s0, ln) in _contig_segments(rs_list):
                nc.sync.dma_start(kept3d[0:1, :, o:o+ln],
                                  pack3d[t:t+1, :, s0:s0+ln])
                o += ln
        # data = (tpT - tpi_kept)/LOG10E20 on the packed row
        tprow = work.tile([1, NL], F32, tag="tprow")
        nc.sync.dma_start(tprow[:], bass.AP(din["tpT"], 0, [[1, NL]]))
        nc.vector.tensor_tensor(out=kept3[0:1, 0:NL], in0=tprow[:],
                                in1=kept3[0:1, 0:NL], op=ALU.subtract)
        nc.vector.tensor_scalar(out=kept3[0:1, 0:NL], in0=kept3[0:1, 0:NL],
                                scalar1=1.0 / LOG10E20, scalar2=None,
                                op0=ALU.mult)
        nc.sync.dma_start(bass.AP(ddbg, 0, [[1, NL]]), kept3[0:1, 0:NL])
        nc.sync.dma_start(bass.AP(scr["sdram"], 0, [[1, NL]]), kept3[0:1, 0:NL])

        nc.vector.memset(dvec[:], 0.0)
        nc.sync.dma_start(dvec[:, 0:12],
                          bass.AP(scr["sdram"], 0, [[1, 128], [128, 12]]))
        nc.sync.dma_start(dvec[0:24, 12:13],
                          bass.AP(scr["sdram"], 1536, [[1, 24]]))
        nc.vector.memset(drep[:], 0.0)
        nc.gpsimd.partition_broadcast(drep[:, 0:NL], kept3[0:1, 0:NL])
        nc.gpsimd.partition_broadcast(wrep_r[:], kept3[0:1, NL:2*NL])
        nc.gpsimd.partition_broadcast(wrep_i[:], kept3[0:1, 2*NL:3*NL])

    # ---------------- P5: Ht build + v = Ht d ----------------
    vsum = late.tile([128, 2 * NB], F32)
    lam = late.tile([128, 1], F32)
    with tc.tile_pool(name="p5_work", bufs=2) as work:
        nc.vector.memset(vsum[:], 0.0)
        for i in range(NB):
            Gq = work.tile([128, 80], F32, tag="h_gq")
            Iq = work.tile([128, 80], F32, tag="h_iq")
            f_ap = fsc_s[:, i:i+1]
            nc.vector.tensor_scalar(out=Gq[:, 0:40], in0=BF[i][:, 168:208],
                                    scalar1=f_ap, scalar2=None, op0=ALU.mult)
            nc.vector.tensor_scalar(out=Gq[:, 0:40], in0=Gq[:, 0:40],
                                    scalar1=-1.0, scalar2=None, op0=ALU.mult)
            nc.vector.tensor_scalar(out=Gq[:, 40:80], in0=BF[i][:, 40:80],
                                    scalar1=f_ap, scalar2=None, op0=ALU.mult)
            nc.vector.tensor_scalar(out=Iq[:, 0:40], in0=BF[i][:, 128:168],
                                    scalar1=f_ap, scalar2=None, op0=ALU.mult)
            nc.vector.tensor_scalar(out=Iq[:, 0:40], in0=Iq[:, 0:40],
                                    scalar1=-1.0, scalar2=None, op0=ALU.mult)
            nc.vector.tensor_scalar(out=Iq[:, 40:80], in0=BF[i][:, 0:40],
                                    scalar1=f_ap, scalar2=None, op0=ALU.mult)
            Gg_r = work.tile([128, NL], F32, tag="h_ggr")
            Gg_i = work.tile([128, NL], F32, tag="h_ggi")
            qr = work.tile([128, NL], F32, tag="h_qr")
            qi = work.tile([128, NL], F32, tag="h_qi")
            base = 0
            for (t, rs_list) in link_groups:
                o = base
                for (s0, ln) in _contig_segments(rs_list):
                    nc.vector.tensor_copy(Gg_r[:, o:o+ln], Gq[:, s0:s0+ln])
                    nc.vector.tensor_copy(Gg_i[:, o:o+ln], Gq[:, 40+s0:40+s0+ln])
                    o += ln
                base += len(rs_list)
            uniform = (len(link_groups) == 40
                       and all(len(rs) == 39 for _, rs in link_groups))
            if uniform:
                # full-width inc multiply via 0-stride replicated APs
                IncR = Iq[:, 0:40].rearrange("p (t o) -> p t o", o=1
                                             ).broadcast_to([128, 40, 39])
                IncI = Iq[:, 40:80].rearrange("p (t o) -> p t o", o=1
                                              ).broadcast_to([128, 40, 39])
                Gg_r3 = Gg_r[:].rearrange("p (t j) -> p t j", t=40)
                Gg_i3 = Gg_i[:].rearrange("p (t j) -> p t j", t=40)
                qr3 = qr[:].rearrange("p (t j) -> p t j", t=40)
                qi3 = qi[:].rearrange("p (t j) -> p t j", t=40)
                nc.vector.tensor_tensor(out=qr3, in0=Gg_r3, in1=IncR,
                                        op=ALU.mult)
                nc.vector.tensor_tensor(out=qi3, in0=Gg_i3, in1=IncR,
                                        op=ALU.mult)
                nc.vector.tensor_tensor(out=Gg_i3, in0=Gg_i3, in1=IncI,
                                        op=ALU.mult)
                nc.vector.tensor_tensor(out=Gg_r3, in0=Gg_r3, in1=IncI,
                                        op=ALU.mult)
            else:
                base = 0
                for (t, rs_list) in link_groups:
                    sl = slice(base, base + len(rs_list))
                    nc.vector.tensor_scalar(out=qr[:, sl], in0=Gg_r[:, sl],
                                            scalar1=Iq[:, t:t+1], scalar2=None,
                                            op0=ALU.mult)
                    nc.vector.tensor_scalar(out=qi[:, sl], in0=Gg_i[:, sl],
                                            scalar1=Iq[:, t:t+1], scalar2=None,
                                            op0=ALU.mult)
                    nc.vector.tensor_scalar(out=Gg_i[:, sl], in0=Gg_i[:, sl],
                                            scalar1=Iq[:, 40+t:40+t+1],
                                            scalar2=None, op0=ALU.mult)
                    nc.vector.tensor_scalar(out=Gg_r[:, sl], in0=Gg_r[:, sl],
                                            scalar1=Iq[:, 40+t:40+t+1],
                                            scalar2=None, op0=ALU.mult)
                    base += len(rs_list)
            nc.vector.tensor_tensor(out=qr[:], in0=qr[:], in1=Gg_i[:],
                                    op=ALU.subtract)
            nc.vector.tensor_tensor(out=qi[:], in0=qi[:], in1=Gg_r[:],
                                    op=ALU.add)
            hr = work.tile([128, LPAD], F32R, tag="h_hr")
            hi = work.tile([128, LPAD], F32R, tag="h_hi")
            t2 = work.tile([128, NL], F32, tag="h_t2")
            nc.vector.memset(hr[:, NL:LPAD].bitcast(F32), 0.0)
            nc.vector.memset(hi[:, NL:LPAD].bitcast(F32), 0.0)
            nc.vector.tensor_tensor(out=hr[:, 0:NL], in0=qr[:], in1=wrep_r[:],
                                    op=ALU.mult)
            nc.vector.tensor_tensor(out=t2[:], in0=qi[:], in1=wrep_i[:],
                                    op=ALU.mult)
            nc.vector.tensor_tensor(out=hr[:, 0:NL], in0=hr[:, 0:NL], in1=t2[:],
                                    op=ALU.subtract)
            nc.vector.tensor_tensor(out=hi[:, 0:NL], in0=qr[:], in1=wrep_i[:],
                                    op=ALU.mult)
            nc.vector.tensor_tensor(out=t2[:], in0=qi[:], in1=wrep_r[:],
                                    op=ALU.mult)
            nc.vector.tensor_tensor(out=hi[:, 0:NL], in0=hi[:, 0:NL], in1=t2[:],
                                    op=ALU.add)
            nc.vector.tensor_scalar(out=hi[:], in0=hi[:], scalar1=-1.0,
                                    scalar2=None, op0=ALU.mult)
            nc.sync.dma_start(scr["htdram"][128*i:128*(i+1), :], hr[:])
            nc.sync.dma_start(scr["htdram"][N+128*i:N+128*(i+1), :], hi[:])
            nc.vector.tensor_tensor(out=t2[:], in0=hr[:, 0:NL],
                                    in1=drep[:, 0:NL], op=ALU.mult)
            nc.vector.tensor_reduce(vsum[:, i:i+1], t2[:], axis=AXX, op=ALU.add)
            nc.vector.tensor_tensor(out=t2[:], in0=hi[:, 0:NL],
                                    in1=drep[:, 0:NL], op=ALU.mult)
            nc.vector.tensor_reduce(vsum[:, NB+i:NB+i+1], t2[:], axis=AXX,
                                    op=ALU.add)
        vsq = work.tile([128, 2 * NB], F32, tag="vsq")
        nc.vector.tensor_tensor(out=vsq[:], in0=vsum[:], in1=vsum[:],
                                op=ALU.mult)
        vred = work.tile([128, 1], F32, tag="vred")
        nc.vector.tensor_reduce(vred[:], vsq[:], axis=AXX, op=ALU.add)
        nc.gpsimd.partition_all_reduce(vred[:], vred[:], 128,
                                       bass_isa.ReduceOp.add)
        nc.scalar.activation(lam[:], vred[:], AF.Sqrt)
        nc.vector.tensor_scalar(out=lam[:], in0=lam[:], scalar1=float(alpha),
                                scalar2=None, op0=ALU.mult)

    # ---------------- P7: Gram ----------------
    st_ = late.tile([128, LB], F32)
    srep = late.tile([128, LPAD], F32)
    with (
        tc.tile_pool(name="g_acc", bufs=1) as gacc,
        tc.tile_pool(name="g_work", bufs=1) as work,
        tc.tile_pool(name="g_psum", bufs=4, space="PSUM") as pg,
    ):
        GA = [gacc.tile([128, LPAD], F32, tag=f"ga{l}", name=f"ga{l}") for l in range(LB)]
        GRP = 6
        for g0 in range(0, 2 * NB, GRP):
            htrs = []
            for gi in range(GRP):
                ch = g0 + gi
                htr = work.tile([128, LPAD], F32R, tag=f"g_htr{gi}",
                                name=f"g_htr{gi}")
                nc.sync.dma_start(htr[:], scr["htdram"][128*ch:128*(ch+1), :])
                htrs.append(htr)
            for l in range(LB):
                c0 = 128 * l
                for cc in range(c0, LPAD, 416):
                    cw = min(416, LPAD - cc)
                    pgt = pg.tile([128, 416], F32, tag="g_pg")
                    for gi in range(GRP):
                        nc.tensor.matmul(pgt[:, 0:cw],
                                         htrs[gi][:, c0:c0+128],
                                         htrs[gi][:, cc:cc+cw],
                                         start=(gi == 0), stop=(gi == GRP - 1))
                    if g0 == 0:
                        nc.vector.tensor_copy(GA[l][:, cc:cc+cw], pgt[:, 0:cw])
                    else:
                        nc.vector.tensor_tensor(out=GA[l][:, cc:cc+cw],
                                                in0=GA[l][:, cc:cc+cw],
                                                in1=pgt[:, 0:cw], op=ALU.add)
        for l in range(LB):
            nc.sync.dma_start(scr["gramdram"][128*l:128*(l+1), :], GA[l][:])
        gd = work.tile([128, LB], F32, tag="gd")
        nc.sync.dma_start(gd[:], bass.AP(scr["gramdram"], 0,
                                         [[LPAD + 1, 128],
                                          [128 * (LPAD + 1), LB]]))
        nc.vector.tensor_scalar(out=gd[:], in0=gd[:], scalar1=lam[:],
                                scalar2=None, op0=ALU.add)
        nc.scalar.activation(st_[:], gd[:], AF.Sqrt)
        nc.vector.reciprocal(st_[:], st_[:])
        ps_ = pg.tile([LB, 128], F32, tag="s_ps")
        nc.tensor.matmul(ps_[:], st_[:], id_s[:], start=True, stop=True)
        s13 = work.tile([LB, 128], F32, tag="s13")
        nc.vector.tensor_copy(s13[:], ps_[:])
        nc.sync.dma_start(bass.AP(scr["srowdram"], 0, [[1, LPAD]]), s13[:])
        srow = work.tile([1, LPAD], F32, tag="srow")
        nc.sync.dma_start(srow[:], bass.AP(scr["srowdram"], 0, [[1, LPAD]]))
        nc.gpsimd.partition_broadcast(srep[:], srow[:])

    # ---------------- P8: scaled SPD solve ----------------
    bf2_pool = ctx.enter_context(tc.tile_pool(name="bf2", bufs=1))
    BF2 = [bf2_pool.tile([128, 128], F32R, tag=f"bf2_{l}", name=f"bf2_{l}") for l in range(LB)]
    with (
        tc.tile_pool(name="s_tri", bufs=1) as tri2,
        tc.tile_pool(name="s_work", bufs=2) as work,
        tc.tile_pool(name="s_pmm", bufs=2, space="PSUM") as pmm,
        tc.tile_pool(name="s_pmisc", bufs=1, space="PSUM") as pmisc,
    ):
        dsc = work.tile([128, LB], F32, tag="dsc")
        nc.vector.tensor_tensor(out=dsc[:], in0=dvec[:], in1=st_[:], op=ALU.mult)
        zz = work.tile([128, 128], F32, tag="zz")
        nc.vector.memset(zz[:], 0.0)
        for l in range(LB):
            nc.vector.tensor_copy(BF2[l][:], zz[:])
            nc.vector.tensor_copy(BF2[l][:, 0:1], dsc[:, l:l+1])
        GT = {}
        for i in range(LB):
            for j in range(i, LB):
                GT[(i, j)] = tri2.tile([128, 128], F32R, tag=f"g{i}_{j}", name=f"g{i}_{j}")
                gload = work.tile([128, 128], F32, tag="g_load")
                nc.sync.dma_start(gload[:],
                                  scr["gramdram"][128*i:128*(i+1),
                                                  128*j:128*(j+1)])
                nc.vector.tensor_scalar(out=gload[:], in0=gload[:],
                                        scalar1=st_[:, i:i+1], scalar2=None,
                                        op0=ALU.mult)
                nc.vector.tensor_tensor(out=gload[:], in0=gload[:],
                                        in1=srep[:, 128*j:128*(j+1)],
                                        op=ALU.mult)
                if i == j:
                    ones1 = work.tile([128, 1], F32, tag="diag1")
                    nc.vector.memset(ones1[:], 1.0)
                    nc.vector.copy_predicated(gload[:], idu_s[:],
                                              ones1[:].broadcast_to([128, 128]))
                nc.vector.tensor_copy(GT[(i, j)][:], gload[:])
        for k in range(LB):
            V = work.tile([128, 128], F32R, tag="lu2_V")
            _newton_real(nc, work, pmm, pmisc, GT[(k, k)], V, id_s, NEWTON_SPD)
            nc.sync.dma_start(scr["v2dram"][128*k:128*(k+1), :], V[:])
            for i in range(k + 1, LB):
                ptr = pmisc.tile([128, 128], F32R, tag="lu2_ptr")
                nc.tensor.transpose(ptr[:], GT[(k, i)][:], idr_s[:])
                utt = work.tile([128, 128], F32R, tag="lu2_utt")
                nc.vector.tensor_copy(utt[:], ptr[:])
                nc.sync.dma_start(
                    scr["ut2dram"][128*i:128*(i+1), 128*k:128*(k+1)], utt[:])
            for i in range(k + 1, LB):
                pl = pmm.tile([128, 128], F32, tag="cmmp1")
                nc.tensor.matmul(pl[:], V[:], GT[(k, i)][:], start=True,
                                 stop=True)
                LT = work.tile([128, 128], F32R, tag="lu2_LT")
                nc.vector.tensor_copy(LT[:], pl[:])
                pb = pmm.tile([128, 128], F32, tag="cmmp2")
                nc.tensor.matmul(pb[:], LT[:], BF2[k][:], start=True, stop=True)
                nc.vector.tensor_tensor(out=BF2[i][:], in0=BF2[i][:],
                                        in1=pb[:], op=ALU.subtract)
                for j in range(i, LB):
                    pt_ = pmm.tile([128, 128], F32, tag="cmmp1")
                    nc.tensor.matmul(pt_[:], LT[:], GT[(k, j)][:], start=True,
                                     stop=True)
                    nc.vector.tensor_tensor(out=GT[(i, j)][:],
                                            in0=GT[(i, j)][:], in1=pt_[:],
                                            op=ALU.subtract)

    ys = late.tile([128, LB], F32)
    yrep = late.tile([128, LPAD], F32)
    with (
        tc.tile_pool(name="b2_work", bufs=3) as work,
        tc.tile_pool(name="b2_pacc", bufs=1, space="PSUM") as pacc,
        tc.tile_pool(name="b2_pmm", bufs=2, space="PSUM") as pmm,
    ):
        for k in range(LB - 1, -1, -1):
            W = work.tile([128, 128], F32R, tag="bs2_W")
            nc.vector.tensor_copy(W[:], BF2[k][:])
            if k < LB - 1:
                P1 = pacc.tile([128, 128], F32, tag="bs2_p1")
                for idx, j in enumerate(range(k + 1, LB)):
                    utt = work.tile([128, 128], F32R, tag="bs2_utt")
                    nc.sync.dma_start(
                        utt[:], scr["ut2dram"][128*j:128*(j+1),
                                               128*k:128*(k+1)])
                    nc.tensor.matmul(P1[:], utt[:], BF2[j][:],
                                     start=(idx == 0), stop=(j == LB - 1))
                nc.vector.tensor_tensor(out=W[:], in0=W[:], in1=P1[:],
                                        op=ALU.subtract)
            Vk = work.tile([128, 128], F32R, tag="bs2_V")
            nc.sync.dma_start(Vk[:], scr["v2dram"][128*k:128*(k+1), :])
            Pf = pmm.tile([128, 128], F32, tag="bs2_pf")
            nc.tensor.matmul(Pf[:], Vk[:], W[:], start=True, stop=True)
            nc.vector.tensor_copy(BF2[k][:], Pf[:])
        for l in range(LB):
            nc.vector.tensor_copy(ys[:, l:l+1], BF2[l][:, 0:1])
        nc.vector.tensor_tensor(out=ys[:], in0=ys[:], in1=st_[:], op=ALU.mult)
        psy = pmm.tile([LB, 128], F32, tag="y_ps")
        nc.tensor.matmul(psy[:], ys[:], id_s[:], start=True, stop=True)
        y13 = work.tile([LB, 128], F32, tag="y13")
        nc.vector.tensor_copy(y13[:], psy[:])
        nc.sync.dma_start(bass.AP(scr["yrowdram"], 0, [[1, LPAD]]), y13[:])
        yrow = work.tile([1, LPAD], F32, tag="yrow")
        nc.sync.dma_start(yrow[:], bass.AP(scr["yrowdram"], 0, [[1, LPAD]]))
        nc.gpsimd.partition_broadcast(yrep[:], yrow[:])

    # ---------------- P9: chi = Ht y ----------------
    with tc.tile_pool(name="p9_work", bufs=2) as work:
        chi = late.tile([128, 2 * NB], F32)
        for ch in range(2 * NB):
            htc = work.tile([128, LPAD], F32R, tag="c_htc")
            nc.sync.dma_start(htc[:], scr["htdram"][128*ch:128*(ch+1), :])
            tm = work.tile([128, LPAD], F32, tag="c_tm")
            nc.vector.tensor_tensor(out=tm[:], in0=htc[:], in1=yrep[:],
                                    op=ALU.mult)
            nc.vector.tensor_reduce(chi[:, ch:ch+1], tm[:], axis=AXX,
                                    op=ALU.add)
        nc.sync.dma_start(bass.AP(out_chi, 0, [[1, 128], [128, 2 * NB]]),
                          chi[:])
    ctx.close()


_GBASE = {}

def _contig_segments(rs_list):
    segs = []
    s = rs_list[0]; prev = s
    for r in rs_list[1:]:
        if r == prev + 1:
            prev = r
        else:
            segs.append((s, prev - s + 1)); s = r; prev = r
    segs.append((s, prev - s + 1))
    return segs


_CACHED = {}


def kernel(epsilon_r_iter, chi_iter, total_power, alpha, grid_x, grid_y,
           direct_field, incident_field, G_freespace, G_freespace_scaled,
           sensor_links):
    eps = np.asarray(epsilon_r_iter)
    chi_it = np.asarray(chi_iter)
    tp = np.asarray(total_power, dtype=np.float32)
    alpha_f = float(np.asarray(alpha))
    gx = np.asarray(grid_x, dtype=np.float32)
    gy = np.asarray(grid_y, dtype=np.float32)
    df = np.asarray(direct_field)
    einc = np.asarray(incident_field)
    gfs = np.asarray(G_freespace)
    gsc = np.asarray(G_freespace_scaled)
    links = np.asarray(sensor_links)

    x = gx.T.reshape(N).astype(np.float32)
    y = gy.T.reshape(N).astype(np.float32)
    scat = np.real(eps.T.reshape(N)).astype(np.float32)

    geomS = np.stack([np.ones(N, np.float32), -2.0*x, -2.0*y,
                      (x*x + y*y)]).astype(np.float32)
    geomR = np.stack([(x*x + y*y), x, y,
                      np.ones(N, np.float32)]).astype(np.float32)
    scat_t = scat.reshape(NB, 128).T.copy()

    bpack = np.zeros((N, RW), np.float32)
    bpack[:, 0:40] = -einc.real; bpack[:, 40:80] = -gfs.real
    bpack[:, 128:168] = -einc.imag; bpack[:, 168:208] = -gfs.imag
    gscT = np.concatenate([gsc.real.T, gsc.imag.T], axis=1).astype(np.float32)
    dfpack = np.concatenate([df.real, df.imag], axis=1).astype(np.float32)
    tpT = tp.T.copy().astype(np.float32)

    groups = []
    i = 0
    while i < len(links):
        t = int(links[i, 0])
        rs_list = []
        while i < len(links) and int(links[i, 0]) == t:
            rs_list.append(int(links[i, 1]))
            i += 1
        groups.append((t, rs_list))

    _GBASE.clear()
    o = 0
    for (t, rs_list) in groups:
        _GBASE[t] = o
        o += len(rs_list)
    key = (hash(links.tobytes()), alpha_f)
    if key not in _CACHED:
        _CACHED[key] = build_program(groups, alpha_f)
    nc = _CACHED[key]

    id128 = np.eye(128, dtype=np.float32)
    im = {
        "geomS": geomS, "geomR": geomR, "scat_t": scat_t, "bpack": bpack,
        "gscT": gscT, "dfpack": dfpack, "tpT": tpT,
        "id128": id128, "idu8": id128.astype(np.uint8),
    }
    import os as _os
    _tr = _os.environ.get("KTRACE", "0") == "1"
    res = run_bass_kernel_spmd(nc, [im] * 8, core_ids=list(range(8)), trace=_tr)
    out = res.results[0]
    _CACHED["last"] = (res, out)

    chi = out["out_chi"]
    dchi_r = chi[:N].reshape(M, M).T
    dchi_i = chi[N:].reshape(M, M).T
    chi_new = (chi_it + (dchi_r + 1j * dchi_i)).astype(np.complex64)
    return chi_new + 1.0, chi_new



# revision 34
# speedup vs baseline: 1.5663x; 1.0227x over previous
"""DRIM layer (distorted Rytov inverse-scattering iteration) on Trainium2.

One Bass/Tile program per core (replicated SPMD on 8 cores):
  P1  Z-matrix build via large-branch Hankel evaluation (upper triangle only;
      Z is complex-symmetric), resident in SBUF as fp32r planes
  P2  block LDL^T elimination, Newton-iterated 128x128 block inverses,
      fp32r tensor-engine matmuls
  P3  back-substitution -> X = Z^-1 [-E_inc | -G]
  P4  total field, RSS power model, data vector
  P5  Rytov H^T rows (4608 x 1664 padded) + H^T d
  P7  Gram H H^T (upper blocks) + Jacobi scaling
  P8  scaled SPD block solve (same Newton machinery, real)
  P9  chi = H^T y, output dchi

Host does input packing / output reshape only.
"""
import math
import os
import numpy as np

import concourse.bass as bass
import concourse.bacc as bacc
import concourse.bass_isa as bass_isa
import concourse.mybir as mybir
import concourse.tile as tile
from concourse.bass_utils import run_bass_kernel_spmd

F32 = mybir.dt.float32
F32R = mybir.dt.float32r
U8 = mybir.dt.uint8
AF = mybir.ActivationFunctionType
ALU = mybir.AluOpType
AXX = mybir.AxisListType.X

M = 48
N = M * M
NB = N // 128               # 18
TX = RX = 40
NL = TX * (RX - 1)          # 1560
LPAD = 1664
LB = LPAD // 128            # 13
RW = 256                    # [0:128]=Re plane, [128:256]=Im plane
CW = 256                    # Z-build column chunk
DOI = 3.0
WL = 0.125
K0 = 2.0 * math.pi / WL
IMP = 120.0 * math.pi
GRID_LEN = DOI / M
GRID_RADIUS = math.sqrt(GRID_LEN ** 2 / math.pi)
NOISE = 1e-6

def _j1s(x):
    t2 = (x / 3.0) ** 2
    return x * (0.5 - 0.56249985*t2 + 0.21093573*t2**2 - 0.03954289*t2**3
                + 0.00443319*t2**4 - 0.00031761*t2**5 + 0.00001109*t2**6)

def _y1s(x):
    t2 = (x / 3.0) ** 2
    p = (-0.6366198 + 0.2212091*t2 + 2.1682709*t2**2 - 1.3164827*t2**3
         + 0.3123951*t2**4 - 0.0400976*t2**5 + 0.0027873*t2**6)
    return ((2.0/math.pi) * x * math.log(0.5*x) * _j1s(x) + p) / x

X0C = K0 * GRID_RADIUS
GRID_AREA = 4.0*math.pi*GRID_RADIUS/(2.0*K0) * _j1s(X0C)
C1 = -IMP * math.pi * GRID_RADIUS / 2.0
C2 = _j1s(X0C)
C3R, C3I = _j1s(X0C), _y1s(X0C)
C1C2 = C1 * C2
ZD_RE = C1 * C3R
ZD_IM_C = C1 * C3I
SA = GRID_AREA * K0 * K0
TWO_PI = 2.0 * math.pi
INV_2PI = 1.0 / TWO_PI
LOG10E20 = 20.0 * math.log10(math.e)
CADD = 10.0 * math.log10(WL * WL / (4.0 * math.pi * IMP) / 1e-3)
C20L = 20.0 / math.log(10.0)

F0C = [0.79788456, -0.00000077, -0.00552740, -0.00009512,
       0.00137237, -0.00072805, 0.00014476]
THC = [-0.78539816, -0.04166397, -0.00003954, 0.00262573,
       -0.00054125, -0.00029333, 0.00013558]
F0CS = [c * (3.0 ** k) * C1C2 for k, c in enumerate(F0C)]
THCS = [c * (3.0 ** k) for k, c in enumerate(THC)]
# short-series Z build: th = x + 3*THC[1]/x ; amp = C1C2*(F0C0 + 3*F0C1/x)/sqrt(x)
TH1 = 3.0 * THC[1]
A0C = C1C2 * F0C[0]
A1C = 3.0 * C1C2 * F0C[1]
K0K0 = K0 * K0
PI4 = math.pi / 4.0
# short-series Z build: th = x - pi/4 + 3*THC[1]*rx ; amp = C1C2*(F0C0 + 3*F0C1*rx)/sqrt(x)
TH1 = 3.0 * THC[1]
A0C = C1C2 * F0C[0]
A1C = 3.0 * C1C2 * F0C[1]
K0K0 = K0 * K0
PI4 = math.pi / 4.0

NEWTON_Z = 12
NEWTON_SPD = 8


def _horner(nc, out_ap, s_ap, coeffs):
    cs = coeffs[::-1]
    nc.vector.tensor_scalar(out=out_ap, in0=s_ap, scalar1=float(cs[0]),
                            scalar2=float(cs[1]), op0=ALU.mult, op1=ALU.add)
    for c in cs[2:]:
        nc.vector.tensor_tensor(out=out_ap, in0=out_ap, in1=s_ap, op=ALU.mult)
        nc.vector.tensor_scalar(out=out_ap, in0=out_ap, scalar1=float(c),
                                scalar2=None, op0=ALU.add)


def _cmm(nc, pool, lhsT, rhs, n=RW):
    P1 = pool.tile([128, n], F32, tag="cmmp1")
    P2 = pool.tile([128, n], F32, tag="cmmp2")
    nc.tensor.matmul(P1[:], lhsT[:, 0:128], rhs, start=True, stop=True)
    nc.tensor.matmul(P2[:], lhsT[:, 128:256], rhs, start=True, stop=True)
    return P1, P2


def _combine_sub(nc, dst, P1, P2):
    nc.vector.tensor_tensor(out=dst[:, 0:256], in0=dst[:, 0:256],
                            in1=P1[:, 0:256], op=ALU.subtract)
    nc.vector.tensor_tensor(out=dst[:, 0:128], in0=dst[:, 0:128],
                            in1=P2[:, 128:256], op=ALU.add)
    nc.vector.tensor_tensor(out=dst[:, 128:256], in0=dst[:, 128:256],
                            in1=P2[:, 0:128], op=ALU.subtract)


def _combine_set(nc, dst, P1, P2):
    nc.vector.tensor_copy(dst[:, 0:256], P1[:, 0:256])
    nc.vector.tensor_tensor(out=dst[:, 0:128], in0=dst[:, 0:128],
                            in1=P2[:, 128:256], op=ALU.subtract)
    nc.vector.tensor_tensor(out=dst[:, 128:256], in0=dst[:, 128:256],
                            in1=P2[:, 0:128], op=ALU.add)


def _newton_scale(nc, work, pmisc, m, tag):
    """1/(colmax * rowmax) of m [128,128] -> [128,1] fp32 AP."""
    ones = work.tile([128, 1], F32, tag=f"nwo_{tag}")
    nc.vector.memset(ones[:], 1.0)
    pc = pmisc.tile([128, 1], F32, tag=f"nwpc_{tag}")
    nc.tensor.matmul(pc[:], m[:], ones[:], start=True, stop=True)
    pr = pmisc.tile([1, 128], F32, tag=f"nwpr_{tag}")
    nc.tensor.matmul(pr[:], ones[:], m[:], start=True, stop=True)
    cs = work.tile([128, 1], F32, tag=f"nwcs_{tag}")
    nc.vector.tensor_copy(cs[:], pc[:])
    rs = work.tile([1, 128], F32, tag=f"nwrs_{tag}")
    nc.vector.tensor_copy(rs[:], pr[:])
    nc.gpsimd.partition_all_reduce(cs[:], cs[:], 128, bass_isa.ReduceOp.max)
    rmax = work.tile([1, 1], F32, tag=f"nwrm_{tag}")
    nc.vector.tensor_reduce(rmax[:], rs[:], axis=AXX, op=ALU.max)
    rmax_b = work.tile([128, 1], F32, tag=f"nwrb_{tag}")
    nc.gpsimd.partition_broadcast(rmax_b[:], rmax[:])
    a = work.tile([128, 1], F32, tag=f"nwa_{tag}")
    nc.vector.tensor_tensor(out=a[:], in0=cs[:], in1=rmax_b[:], op=ALU.mult)
    nc.vector.reciprocal(a[:], a[:])
    return a


def _newton_cplx(nc, work, pmm, pmisc, D, Xout, id2_s, iters):
    """Swap-free Newton: maintains X=[Xr|Xi] and Xs=[-Xi|Xr].
    D@X via lhsT=Dr,rhs=X + lhsT=Di,rhs=Xs (both width-256, PSUM acc).
    """
    m = work.tile([128, 128], F32, tag="nw_m")
    m2 = work.tile([128, 128], F32, tag="nw_m2")
    nc.scalar.activation(m[:], D[:, 0:128], AF.Abs)
    nc.scalar.activation(m2[:], D[:, 128:256], AF.Abs)
    nc.vector.tensor_tensor(out=m[:], in0=m[:], in1=m2[:], op=ALU.max)
    a = _newton_scale(nc, work, pmisc, m, "c")
    nc.vector.tensor_scalar(out=Xout[:, 0:128], in0=D[:, 0:128], scalar1=a[:],
                            scalar2=None, op0=ALU.mult)
    Xs = work.tile([128, RW], F32R, tag="nw_Xs")
    nc.vector.tensor_scalar(out=Xs[:, 0:128], in0=D[:, 128:256], scalar1=a[:],
                            scalar2=None, op0=ALU.mult)
    # Xi = -Di*a = -(Xs lo)
    nc.vector.tensor_scalar(out=Xout[:, 128:256], in0=Xs[:, 0:128],
                            scalar1=-1.0, scalar2=None, op0=ALU.mult)
    nc.scalar.copy(Xs[:, 128:256], Xout[:, 0:128])
    R = work.tile([128, RW], F32R, tag="nw_R")
    Rs = work.tile([128, RW], F32R, tag="nw_Rs")
    for _ in range(iters):
        P = pmm.tile([128, RW], F32, tag="cmmp1")
        nc.tensor.matmul(P[:], D[:, 0:128], Xout[:, 0:RW], start=True,
                         stop=False)
        nc.tensor.matmul(P[:], D[:, 128:256], Xs[:, 0:RW], start=False,
                         stop=True)
        # R = [I|0] - P ;  Rs = [-Ri|Rr] = [Pi | Rr]
        nc.vector.tensor_tensor(out=R[:], in0=id2_s[:], in1=P[:],
                                op=ALU.subtract)
        nc.scalar.copy(Rs[:, 0:128], P[:, 128:256])
        nc.scalar.copy(Rs[:, 128:256], R[:, 0:128])
        Q = pmm.tile([128, RW], F32, tag="cmmp2")
        nc.tensor.matmul(Q[:], Xout[:, 0:128], R[:, 0:RW], start=True,
                         stop=False)
        nc.tensor.matmul(Q[:], Xout[:, 128:256], Rs[:, 0:RW], start=False,
                         stop=True)
        # X += Q ; Xs_lo -= Qi ; Xs_hi = new Xr
        nc.vector.tensor_tensor(out=Xout[:, 0:256], in0=Xout[:, 0:256],
                                in1=Q[:, 0:256], op=ALU.add)
        nc.vector.tensor_tensor(out=Xs[:, 0:128], in0=Xs[:, 0:128],
                                in1=Q[:, 128:256], op=ALU.subtract)
        nc.scalar.copy(Xs[:, 128:256], Xout[:, 0:128])


def _newton_real(nc, work, pmm, pmisc, D, Xout, id_s, iters):
    m = work.tile([128, 128], F32, tag="nw_m")
    nc.scalar.activation(m[:], D[:], AF.Abs)
    a = _newton_scale(nc, work, pmisc, m, "r")
    nc.vector.tensor_scalar(out=Xout[:], in0=D[:], scalar1=a[:], scalar2=None,
                            op0=ALU.mult)
    R = work.tile([128, 128], F32R, tag="nw_R")
    for _ in range(iters):
        P1 = pmm.tile([128, 128], F32, tag="cmmp1")
        nc.tensor.matmul(P1[:], D[:], Xout[:], start=True, stop=True)
        nc.vector.tensor_tensor(out=R[:], in0=id_s[:], in1=P1[:],
                                op=ALU.subtract)
        Q1 = pmm.tile([128, 128], F32, tag="cmmp2")
        nc.tensor.matmul(Q1[:], Xout[:], R[:], start=True, stop=True)
        nc.vector.tensor_tensor(out=Xout[:], in0=Xout[:], in1=Q1[:], op=ALU.add)


def build_program(link_groups, alpha):
    nc = bacc.Bacc("TRN2", target_bir_lowering=False, num_devices=8)
    din = {}
    def inp(name, shape, dtype=F32):
        din[name] = nc.dram_tensor(name, shape, dtype, kind="ExternalInput")
    inp("geomS", [4, N]); inp("geomR", [4, N]); inp("scat_t", [128, NB])
    inp("bpack", [N, RW]); inp("gscT", [N, 80]); inp("dfpack", [40, 80])
    inp("tpT", [40, RX - 1]); inp("id128", [128, 128]); inp("idu8", [128, 128], U8)
    out_chi = nc.dram_tensor("out_chi", [2 * N], F32, kind="ExternalOutput")
    xdbg = nc.dram_tensor("xdbg", [N, RW], F32, kind="ExternalOutput")
    tfdbg = nc.dram_tensor("tfdbg", [40, 80], F32, kind="ExternalOutput")
    ddbg = nc.dram_tensor("ddbg", [40, RX - 1], F32, kind="ExternalOutput")
    scr = {}
    scr["vdram"] = nc.dram_tensor("vdram", [NB * 128, RW], F32R, kind="Internal")
    scr["utdram"] = nc.dram_tensor("utdram", [N, 2 * N], F32R, kind="Internal")
    scr["htdram"] = nc.dram_tensor("htdram", [2 * N, LPAD], F32R, kind="Internal")
    scr["gramdram"] = nc.dram_tensor("gramdram", [LPAD, LPAD], F32, kind="Internal")
    scr["v2dram"] = nc.dram_tensor("v2dram", [LB * 128, 128], F32R, kind="Internal")
    scr["ut2dram"] = nc.dram_tensor("ut2dram", [LPAD, LPAD], F32R, kind="Internal")
    scr["sdram"] = nc.dram_tensor("sdram", [NL], F32, kind="Internal")
    scr["wdram"] = nc.dram_tensor("wdram", [2 * NL], F32, kind="Internal")
    scr["srowdram"] = nc.dram_tensor("srowdram", [LPAD], F32, kind="Internal")
    scr["yrowdram"] = nc.dram_tensor("yrowdram", [LPAD], F32, kind="Internal")

    with tile.TileContext(nc) as tc:
        _body(nc, tc, din, out_chi, xdbg, tfdbg, ddbg, scr, link_groups, alpha)
    nc.compile()
    return nc


def _body(nc, tc, din, out_chi, xdbg, tfdbg, ddbg, scr, link_groups, alpha):
    import contextlib
    ctx = contextlib.ExitStack()
    consts = ctx.enter_context(tc.tile_pool(name="consts", bufs=1))
    id_s = consts.tile([128, 128], F32)
    nc.sync.dma_start(id_s[:], din["id128"][:])
    idr_s = consts.tile([128, 128], F32R)
    nc.vector.tensor_copy(idr_s[:], id_s[:])
    idu_s = consts.tile([128, 128], U8)
    nc.sync.dma_start(idu_s[:], din["idu8"][:])
    id2_s = consts.tile([128, RW], F32)
    nc.vector.memset(id2_s[:], 0.0)
    nc.vector.tensor_copy(id2_s[:, 0:128], id_s[:])
    scat_s = consts.tile([128, NB], F32)
    nc.sync.dma_start(scat_s[:], din["scat_t"][:])

    zdi_s = consts.tile([128, NB], F32)
    fsc_s = consts.tile([128, NB], F32)
    t0 = consts.tile([128, NB], F32)
    nc.vector.tensor_scalar(out=t0[:], in0=scat_s[:], scalar1=-1.0,
                            scalar2=None, op0=ALU.add)
    nc.vector.reciprocal(t0[:], t0[:])
    nc.vector.tensor_scalar(out=fsc_s[:], in0=t0[:], scalar1=(IMP / K0),
                            scalar2=None, op0=ALU.mult)
    nc.vector.tensor_tensor(out=t0[:], in0=t0[:], in1=scat_s[:], op=ALU.mult)
    nc.vector.tensor_scalar(out=zdi_s[:], in0=t0[:], scalar1=-(IMP / K0),
                            scalar2=ZD_IM_C, op0=ALU.mult, op1=ALU.add)
    zdr_c = consts.tile([128, 1], F32)
    nc.vector.memset(zdr_c[:], float(ZD_RE))

    bf_pool = ctx.enter_context(tc.tile_pool(name="bf", bufs=1))
    BF = [bf_pool.tile([128, RW], F32R, tag=f"bf{i}", name=f"bf{i}") for i in range(NB)]

    with tc.tile_pool(name="tri", bufs=1) as tri:
        ZT = {}
        for i in range(NB):
            for j in range(i, NB):
                ZT[(i, j)] = tri.tile([128, RW], F32R, tag=f"z{i}_{j}", name=f"z{i}_{j}")

        # ---------------- P1: Z build ----------------
        with (
            tc.tile_pool(name="zb_geom", bufs=2) as gpool,
            tc.tile_pool(name="zb_work", bufs=1) as work,
            tc.tile_pool(name="zb_psum", bufs=2, space="PSUM") as pz,
        ):
            for k in range(NB):
                r0 = 128 * k
                gS = gpool.tile([4, 128], F32, tag="gS", name="gS")
                nc.sync.dma_start(gS[:], din["geomS"][:, r0:r0+128])
                chunks = []
                j = k
                while j < NB:
                    w = 256 if j + 1 < NB else 128
                    chunks.append((j, w))
                    j += w // 128

                def stage_aps(j, w):
                    # r1/r2 staged in the ZT tiles themselves (scratch reuse)
                    if w == 256:
                        return ZT[(k, j)][:, 0:256], ZT[(k, j + 1)][:, 0:256]
                    return ZT[(k, j)][:, 0:128], ZT[(k, j)][:, 128:256]

                for b0 in range(0, len(chunks), 5):
                    batch = chunks[b0:b0+5]
                    amps = []
                    # pass A: sqrt act-table (+Copy for int round-trips)
                    for ci, (j, w) in enumerate(batch):
                        c0 = 128 * j
                        gR = gpool.tile([4, CW], F32, tag="gR", name="gR")
                        nc.sync.dma_start(gR[:, 0:w], din["geomR"][:, c0:c0+w])
                        pd = pz.tile([128, CW], F32, tag="zb_pd")
                        nc.tensor.matmul(pd[:, 0:w], gS[:], gR[:, 0:w],
                                         start=True, stop=True)
                        ts1 = work.tile([128, CW], F32, tag="zb_ts1")
                        nc.vector.tensor_scalar(out=ts1[:, 0:w], in0=pd[:, 0:w],
                                                scalar1=0.002,
                                                scalar2=float(K0K0),
                                                op0=ALU.max, op1=ALU.mult)
                        xf = work.tile([128, CW], F32, tag="zb_xf")
                        nc.scalar.activation(xf[:, 0:w], ts1[:, 0:w], AF.Sqrt)
                        rx = work.tile([128, CW], F32, tag="zb_rx")
                        nc.vector.reciprocal(rx[:, 0:w], xf[:, 0:w])
                        th = work.tile([128, CW], F32, tag="zb_ts1")
                        nc.vector.scalar_tensor_tensor(
                            out=th[:, 0:w], in0=rx[:, 0:w], scalar=float(TH1),
                            in1=xf[:, 0:w], op0=ALU.mult, op1=ALU.add)
                        srx = work.tile([128, CW], F32, tag="zb_xf")
                        nc.scalar.activation(srx[:, 0:w], rx[:, 0:w], AF.Sqrt)
                        r1ap, r2ap = stage_aps(j, w)
                        ki = work.tile([128, CW], mybir.dt.int32, tag="zb_ki")
                        mf = work.tile([128, CW], F32, tag="zb_mf")
                        # r1 = (th - pi/4) - 2pi*round((th - pi/4)/2pi)
                        nc.scalar.activation(ki[:, 0:w], th[:, 0:w], AF.Copy,
                                             scale=float(INV_2PI),
                                             bias=-0.125)
                        nc.scalar.activation(mf[:, 0:w], ki[:, 0:w], AF.Copy,
                                             bias=0.125)
                        nc.vector.scalar_tensor_tensor(
                            out=r1ap, in0=mf[:, 0:w],
                            scalar=float(-TWO_PI), in1=th[:, 0:w],
                            op0=ALU.mult, op1=ALU.add)
                        # r2 = (th + pi/4) - 2pi*round((th + pi/4)/2pi)
                        nc.scalar.activation(ki[:, 0:w], th[:, 0:w], AF.Copy,
                                             scale=float(INV_2PI),
                                             bias=0.125)
                        nc.scalar.activation(mf[:, 0:w], ki[:, 0:w], AF.Copy,
                                             bias=-0.125)
                        nc.vector.scalar_tensor_tensor(
                            out=r2ap, in0=mf[:, 0:w],
                            scalar=float(-TWO_PI), in1=th[:, 0:w],
                            op0=ALU.mult, op1=ALU.add)
                        f0t = work.tile([128, CW], F32, tag="zb_ts1")
                        nc.vector.tensor_scalar(out=f0t[:, 0:w],
                                                in0=rx[:, 0:w],
                                                scalar1=float(A1C),
                                                scalar2=float(A0C),
                                                op0=ALU.mult, op1=ALU.add)
                        amp = work.tile([128, CW], F32, tag=f"zb_amp{ci}",
                                        name=f"zb_amp{ci}")
                        nc.vector.tensor_tensor(out=amp[:, 0:w],
                                                in0=f0t[:, 0:w],
                                                in1=srx[:, 0:w], op=ALU.mult)
                        amps.append(amp)
                    # pass B: trig act-table
                    for ci, (j, w) in enumerate(batch):
                        amp = amps[ci]
                        r1ap, r2ap = stage_aps(j, w)
                        sinr = work.tile([128, CW], F32, tag="zb_rx")
                        nc.scalar.activation(sinr[:, 0:w], r1ap, AF.Sin)
                        cosr = work.tile([128, CW], F32, tag="zb_xf")
                        nc.scalar.activation(cosr[:, 0:w], r2ap, AF.Sin)
                        if j == k:
                            # diag block: predicate on F32 staging, then copy
                            stg = work.tile([128, CW], F32, tag="zb_mf")
                            nc.vector.tensor_tensor(
                                out=stg[:, 0:128], in0=amp[:, 0:128],
                                in1=cosr[:, 0:128], op=ALU.mult)
                            nc.vector.tensor_tensor(
                                out=stg[:, 128:256], in0=amp[:, 0:128],
                                in1=sinr[:, 0:128], op=ALU.mult)
                            nc.vector.copy_predicated(
                                stg[:, 0:128], idu_s[:],
                                zdr_c[:].broadcast_to([128, 128]))
                            nc.vector.copy_predicated(
                                stg[:, 128:256], idu_s[:],
                                zdi_s[:, k:k+1].broadcast_to([128, 128]))
                            nc.vector.tensor_copy(ZT[(k, k)][:, 0:256],
                                                  stg[:, 0:256])
                            bstart = 1
                        else:
                            bstart = 0
                        for b in range(bstart, w // 128):
                            sl = slice(128 * b, 128 * b + 128)
                            nc.vector.tensor_tensor(
                                out=ZT[(k, j + b)][:, 0:128],
                                in0=amp[:, sl], in1=cosr[:, sl], op=ALU.mult)
                            nc.vector.tensor_tensor(
                                out=ZT[(k, j + b)][:, 128:256],
                                in0=amp[:, sl], in1=sinr[:, sl], op=ALU.mult)

        # ---------------- P2: block LDL^T ----------------
        with (
            tc.tile_pool(name="lu_big", bufs=1) as work,
            tc.tile_pool(name="lu_sm", bufs=2) as wsm,
            tc.tile_pool(name="lu_pmm", bufs=1, space="PSUM") as pmm,
            tc.tile_pool(name="lu_pmisc", bufs=1, space="PSUM") as pmisc,
        ):
            ldtmp0 = wsm.tile([128, RW], F32, tag="ldtmp")
            for i in range(NB):
                nc.sync.dma_start(ldtmp0[:], din["bpack"][128*i:128*(i+1), :])
                nc.vector.tensor_copy(BF[i][:], ldtmp0[:])
                ldtmp0 = wsm.tile([128, RW], F32, tag="ldtmp")
            IC = 3
            for k in range(NB):
                V = work.tile([128, RW], F32R, tag="lu_V")
                _newton_cplx(nc, work, pmm, pmisc, ZT[(k, k)], V, id2_s,
                             NEWTON_Z)
                nc.sync.dma_start(scr["vdram"][128*k:128*(k+1), :], V[:])
                for i in range(k + 1, NB):
                    ptr = pmisc.tile([128, 128], F32R, tag="lu_ptr")
                    nc.tensor.transpose(ptr[:], ZT[(k, i)][:, 0:128], idr_s[:])
                    utt = wsm.tile([128, RW], F32R, tag="lu_utt")
                    nc.vector.tensor_copy(utt[:, 0:128], ptr[:])
                    pti = pmisc.tile([128, 128], F32R, tag="lu_pti")
                    nc.tensor.transpose(pti[:], ZT[(k, i)][:, 128:256], idr_s[:])
                    nc.vector.tensor_copy(utt[:, 128:256], pti[:])
                    nc.sync.dma_start(
                        scr["utdram"][128*i:128*(i+1), 256*k:256*(k+1)], utt[:])
                if k == NB - 1:
                    continue
                bswap = work.tile([128, RW], F32R, tag="lu_bs")
                nc.scalar.mul(bswap[:, 0:128], BF[k][:, 128:256], -1.0)
                nc.scalar.copy(bswap[:, 128:256], BF[k][:, 0:128])
                for a in range(k + 1, NB, IC):
                    b = min(a + IC, NB)
                    LTs = {}
                    for j in range(a, NB):
                        zsw = wsm.tile([128, RW], F32R, tag="lu_zsw")
                        nc.scalar.mul(zsw[:, 0:128],
                                      ZT[(k, j)][:, 128:256], -1.0)
                        nc.scalar.copy(zsw[:, 128:256], ZT[(k, j)][:, 0:128])
                        if j < b:
                            pl = pmm.tile([128, RW], F32, tag="cmmp1")
                            nc.tensor.matmul(pl[:], V[:, 0:128],
                                             ZT[(k, j)][:, 0:RW],
                                             start=True, stop=False)
                            nc.tensor.matmul(pl[:], V[:, 128:256],
                                             zsw[:, 0:RW],
                                             start=False, stop=True)
                            LT = work.tile([128, RW], F32R,
                                           tag=f"lu_LT{j - a}",
                                           name=f"lu_LT{j - a}")
                            nc.scalar.copy(LT[:], pl[:])
                            LTs[j] = LT
                            pb = pmm.tile([128, RW], F32, tag="cmmp2")
                            nc.tensor.matmul(pb[:], LT[:, 0:128],
                                             BF[k][:, 0:RW],
                                             start=True, stop=False)
                            nc.tensor.matmul(pb[:], LT[:, 128:256],
                                             bswap[:, 0:RW],
                                             start=False, stop=True)
                            nc.vector.tensor_tensor(out=BF[j][:, 0:256],
                                                    in0=BF[j][:, 0:256],
                                                    in1=pb[:, 0:256],
                                                    op=ALU.subtract)
                        for i in range(a, min(b, j + 1)):
                            pu = pmm.tile([128, RW], F32,
                                          tag=f"updp{(i - a) % 2}",
                                          name=f"updp{(i - a) % 2}")
                            nc.tensor.matmul(pu[:], LTs[i][:, 0:128],
                                             ZT[(k, j)][:, 0:RW],
                                             start=True, stop=False)
                            nc.tensor.matmul(pu[:], LTs[i][:, 128:256],
                                             zsw[:, 0:RW],
                                             start=False, stop=True)
                            nc.vector.tensor_tensor(out=ZT[(i, j)][:, 0:256],
                                                    in0=ZT[(i, j)][:, 0:256],
                                                    in1=pu[:, 0:256],
                                                    op=ALU.subtract)

    # ---------------- P3: back-substitution ----------------
    with (
        tc.tile_pool(name="bs_work", bufs=3) as work,
        tc.tile_pool(name="bs_pacc", bufs=1, space="PSUM") as pacc,
        tc.tile_pool(name="bs_pmm", bufs=2, space="PSUM") as pmm,
    ):
        for k in range(NB - 1, -1, -1):
            W = work.tile([128, RW], F32R, tag="bs_W")
            nc.vector.tensor_copy(W[:], BF[k][:])
            if k < NB - 1:
                P1 = pacc.tile([128, RW], F32, tag="bs_p1")
                P2 = pacc.tile([128, RW], F32, tag="bs_p2")
                for idx, j in enumerate(range(k + 1, NB)):
                    utt = work.tile([128, RW], F32R, tag="bs_utt")
                    nc.sync.dma_start(
                        utt[:], scr["utdram"][128*j:128*(j+1), 256*k:256*(k+1)])
                    st = (idx == 0); sp_ = (j == NB - 1)
                    nc.tensor.matmul(P1[:], utt[:, 0:128], BF[j][:, 0:RW],
                                     start=st, stop=sp_)
                    nc.tensor.matmul(P2[:], utt[:, 128:256], BF[j][:, 0:RW],
                                     start=st, stop=sp_)
                _combine_sub(nc, W, P1, P2)
            Vk = work.tile([128, RW], F32R, tag="bs_V")
            nc.sync.dma_start(Vk[:], scr["vdram"][128*k:128*(k+1), :])
            P1, P2 = _cmm(nc, pmm, Vk, W[:, 0:RW])
            _combine_set(nc, BF[k], P1, P2)
            nc.sync.dma_start(xdbg[128*k:128*(k+1), :], BF[k][:].bitcast(F32))

    # ---------------- P4: tf + data vector ----------------
    late = ctx.enter_context(tc.tile_pool(name="late", bufs=1))
    dvec = late.tile([128, LB], F32)
    drep = late.tile([128, LPAD], F32)
    wrep_r = late.tile([128, NL], F32)
    wrep_i = late.tile([128, NL], F32)
    wneg_r = late.tile([128, NL], F32)
    wneg_i = late.tile([128, NL], F32)
    with (
        tc.tile_pool(name="p4_work", bufs=2) as work,
        tc.tile_pool(name="p4_pacc", bufs=1, space="PSUM") as pacc,
        tc.tile_pool(name="p4_pmisc", bufs=1, space="PSUM") as pmisc,
    ):
        Ptf1 = pacc.tile([40, RW], F32, tag="tf_p1")
        Ptf2 = pacc.tile([40, RW], F32, tag="tf_p2")
        for i in range(NB):
            gt = work.tile([128, 80], F32, tag="tf_g")
            nc.sync.dma_start(gt[:], din["gscT"][128*i:128*(i+1), :])
            gtr = work.tile([128, 80], F32R, tag="tf_gr")
            nc.vector.tensor_copy(gtr[:], gt[:])
            st = (i == 0); sp_ = (i == NB - 1)
            nc.tensor.matmul(Ptf1[:], gtr[:, 0:40], BF[i][:, 0:RW],
                             start=st, stop=sp_)
            nc.tensor.matmul(Ptf2[:], gtr[:, 40:80], BF[i][:, 0:RW],
                             start=st, stop=sp_)
        df = work.tile([40, 80], F32, tag="tf_df")
        nc.sync.dma_start(df[:], din["dfpack"][:])
        tfr = work.tile([40, 40], F32, tag="tfr")
        tfi = work.tile([40, 40], F32, tag="tfi")
        nc.vector.tensor_tensor(out=tfr[:], in0=df[:, 0:40],
                                in1=Ptf1[:, 0:40], op=ALU.add)
        nc.vector.tensor_tensor(out=tfr[:], in0=tfr[:],
                                in1=Ptf2[:, 128:168], op=ALU.subtract)
        nc.vector.tensor_tensor(out=tfi[:], in0=df[:, 40:80],
                                in1=Ptf1[:, 128:168], op=ALU.add)
        nc.vector.tensor_tensor(out=tfi[:], in0=tfi[:],
                                in1=Ptf2[:, 0:40], op=ALU.add)
        tfd = work.tile([40, 80], F32, tag="tf_out")
        nc.vector.tensor_copy(tfd[:, 0:40], tfr[:])
        nc.vector.tensor_copy(tfd[:, 40:80], tfi[:])
        nc.sync.dma_start(tfdbg[:], tfd[:])

        pw = work.tile([40, 40], F32, tag="pw")
        nc.vector.tensor_tensor(out=pw[:], in0=tfr[:], in1=tfr[:], op=ALU.mult)
        t1 = work.tile([40, 40], F32, tag="pw_t")
        nc.vector.tensor_tensor(out=t1[:], in0=tfi[:], in1=tfi[:], op=ALU.mult)
        nc.vector.tensor_tensor(out=pw[:], in0=pw[:], in1=t1[:], op=ALU.add)
        amp = work.tile([40, 40], F32, tag="amp")
        nc.scalar.activation(amp[:], pw[:], AF.Sqrt)
        nc.vector.tensor_scalar(out=amp[:], in0=amp[:], scalar1=NOISE,
                                scalar2=None, op0=ALU.add)
        nc.scalar.activation(amp[:], amp[:], AF.Ln)
        tpi = work.tile([40, 40], F32, tag="tpi")
        nc.vector.tensor_scalar(out=tpi[:], in0=amp[:], scalar1=C20L,
                                scalar2=CADD, op0=ALU.mult, op1=ALU.add)
        rec = work.tile([40, 40], F32, tag="rec")
        nc.vector.reciprocal(rec[:], pw[:])
        wr = work.tile([40, 40], F32, tag="wr")
        nc.vector.tensor_tensor(out=wr[:], in0=tfr[:], in1=rec[:], op=ALU.mult)
        nc.vector.tensor_scalar(out=wr[:], in0=wr[:], scalar1=SA, scalar2=None,
                                op0=ALU.mult)
        wi = work.tile([40, 40], F32, tag="wi")
        nc.vector.tensor_tensor(out=wi[:], in0=tfi[:], in1=rec[:], op=ALU.mult)
        nc.vector.tensor_scalar(out=wi[:], in0=wi[:], scalar1=-SA, scalar2=None,
                                op0=ALU.mult)

        def t40(src, name):
            pt = pmisc.tile([40, 40], F32, tag=f"t40p_{name}")
            nc.tensor.matmul(pt[:], src[:], id_s[0:40, 0:40], start=True,
                             stop=True)
            d = work.tile([40, 40], F32, tag=f"t40_{name}")
            nc.vector.tensor_copy(d[:], pt[:])
            return d
        tpiT = t40(tpi, "tpi"); wrT = t40(wr, "wr"); wiT = t40(wi, "wi")

        pack = work.tile([40, 120], F32, tag="pack")
        nc.vector.tensor_copy(pack[:, 0:40], tpiT[:])
        nc.vector.tensor_copy(pack[:, 40:80], wrT[:])
        nc.vector.tensor_copy(pack[:, 80:120], wiT[:])
        kept3 = work.tile([1, 3 * NL], F32, tag="kept3")
        pack3d = pack[:].rearrange("p (a b) -> p a b", a=3)
        kept3d = kept3[:].rearrange("p (a b) -> p a b", a=3)
        for (t, rs_list) in link_groups:
            o = _GBASE[t]
            for (s0, ln) in _contig_segments(rs_list):
                nc.sync.dma_start(kept3d[0:1, :, o:o+ln],
                                  pack3d[t:t+1, :, s0:s0+ln])
                o += ln
        # data = (tpT - tpi_kept)/LOG10E20 on the packed row
        tprow = work.tile([1, NL], F32, tag="tprow")
        nc.sync.dma_start(tprow[:], bass.AP(din["tpT"], 0, [[1, NL]]))
        nc.vector.tensor_tensor(out=kept3[0:1, 0:NL], in0=tprow[:],
                                in1=kept3[0:1, 0:NL], op=ALU.subtract)
        nc.vector.tensor_scalar(out=kept3[0:1, 0:NL], in0=kept3[0:1, 0:NL],
                                scalar1=1.0 / LOG10E20, scalar2=None,
                                op0=ALU.mult)
        nc.sync.dma_start(bass.AP(ddbg, 0, [[1, NL]]), kept3[0:1, 0:NL])
        nc.sync.dma_start(bass.AP(scr["sdram"], 0, [[1, NL]]), kept3[0:1, 0:NL])

        nc.vector.memset(dvec[:], 0.0)
        nc.sync.dma_start(dvec[:, 0:12],
                          bass.AP(scr["sdram"], 0, [[1, 128], [128, 12]]))
        nc.sync.dma_start(dvec[0:24, 12:13],
                          bass.AP(scr["sdram"], 1536, [[1, 24]]))
        nc.vector.memset(drep[:], 0.0)
        nc.gpsimd.partition_broadcast(drep[:, 0:NL], kept3[0:1, 0:NL])
        nc.gpsimd.partition_broadcast(wrep_r[:], kept3[0:1, NL:2*NL])
        nc.gpsimd.partition_broadcast(wrep_i[:], kept3[0:1, 2*NL:3*NL])
        nc.scalar.mul(wneg_r[:], wrep_r[:], -1.0)
        nc.scalar.mul(wneg_i[:], wrep_i[:], -1.0)

    # ---------------- P5: Ht build + v = Ht d ----------------
    vsum = late.tile([128, 2 * NB], F32)
    lam = late.tile([128, 1], F32)
    with tc.tile_pool(name="p5_work", bufs=2) as work:
        nc.vector.memset(vsum[:], 0.0)
        for i in range(NB):
            Gq = work.tile([128, 80], F32, tag="h_gq")
            Iq = work.tile([128, 80], F32, tag="h_iq")
            f_ap = fsc_s[:, i:i+1]
            nc.vector.tensor_scalar(out=Gq[:, 0:40], in0=BF[i][:, 168:208],
                                    scalar1=f_ap, scalar2=None, op0=ALU.mult)
            nc.vector.tensor_scalar(out=Gq[:, 0:40], in0=Gq[:, 0:40],
                                    scalar1=-1.0, scalar2=None, op0=ALU.mult)
            nc.vector.tensor_scalar(out=Gq[:, 40:80], in0=BF[i][:, 40:80],
                                    scalar1=f_ap, scalar2=None, op0=ALU.mult)
            nc.vector.tensor_scalar(out=Iq[:, 0:40], in0=BF[i][:, 128:168],
                                    scalar1=f_ap, scalar2=None, op0=ALU.mult)
            nc.vector.tensor_scalar(out=Iq[:, 0:40], in0=Iq[:, 0:40],
                                    scalar1=-1.0, scalar2=None, op0=ALU.mult)
            nc.vector.tensor_scalar(out=Iq[:, 40:80], in0=BF[i][:, 0:40],
                                    scalar1=f_ap, scalar2=None, op0=ALU.mult)
            Gg_r = work.tile([128, NL], F32, tag="h_ggr")
            Gg_i = work.tile([128, NL], F32, tag="h_ggi")
            qr = work.tile([128, NL], F32, tag="h_qr")
            qi = work.tile([128, NL], F32, tag="h_qi")
            base = 0
            for (t, rs_list) in link_groups:
                o = base
                for (s0, ln) in _contig_segments(rs_list):
                    nc.vector.tensor_copy(Gg_r[:, o:o+ln], Gq[:, s0:s0+ln])
                    nc.vector.tensor_copy(Gg_i[:, o:o+ln], Gq[:, 40+s0:40+s0+ln])
                    o += ln
                base += len(rs_list)
            uniform = (len(link_groups) == 40
                       and all(len(rs) == 39 for _, rs in link_groups))
            if uniform:
                # full-width inc multiply via 0-stride replicated APs
                IncR = Iq[:, 0:40].rearrange("p (t o) -> p t o", o=1
                                             ).broadcast_to([128, 40, 39])
                IncI = Iq[:, 40:80].rearrange("p (t o) -> p t o", o=1
                                              ).broadcast_to([128, 40, 39])
                Gg_r3 = Gg_r[:].rearrange("p (t j) -> p t j", t=40)
                Gg_i3 = Gg_i[:].rearrange("p (t j) -> p t j", t=40)
                qr3 = qr[:].rearrange("p (t j) -> p t j", t=40)
                qi3 = qi[:].rearrange("p (t j) -> p t j", t=40)
                nc.vector.tensor_tensor(out=qr3, in0=Gg_r3, in1=IncR,
                                        op=ALU.mult)
                nc.vector.tensor_tensor(out=qi3, in0=Gg_i3, in1=IncR,
                                        op=ALU.mult)
                nc.vector.tensor_tensor(out=Gg_i3, in0=Gg_i3, in1=IncI,
                                        op=ALU.mult)
                nc.vector.tensor_tensor(out=Gg_r3, in0=Gg_r3, in1=IncI,
                                        op=ALU.mult)
            else:
                base = 0
                for (t, rs_list) in link_groups:
                    sl = slice(base, base + len(rs_list))
                    nc.vector.tensor_scalar(out=qr[:, sl], in0=Gg_r[:, sl],
                                            scalar1=Iq[:, t:t+1], scalar2=None,
                                            op0=ALU.mult)
                    nc.vector.tensor_scalar(out=qi[:, sl], in0=Gg_i[:, sl],
                                            scalar1=Iq[:, t:t+1], scalar2=None,
                                            op0=ALU.mult)
                    nc.vector.tensor_scalar(out=Gg_i[:, sl], in0=Gg_i[:, sl],
                                            scalar1=Iq[:, 40+t:40+t+1],
                                            scalar2=None, op0=ALU.mult)
                    nc.vector.tensor_scalar(out=Gg_r[:, sl], in0=Gg_r[:, sl],
                                            scalar1=Iq[:, 40+t:40+t+1],
                                            scalar2=None, op0=ALU.mult)
                    base += len(rs_list)
            nc.vector.tensor_tensor(out=qr[:], in0=qr[:], in1=Gg_i[:],
                                    op=ALU.subtract)
            nc.vector.tensor_tensor(out=qi[:], in0=qi[:], in1=Gg_r[:],
                                    op=ALU.add)
            hr = work.tile([128, LPAD], F32R, tag="h_hr")
            hi = work.tile([128, LPAD], F32R, tag="h_hi")
            t2 = work.tile([128, NL], F32, tag="h_t2")
            nc.vector.memset(hr[:, NL:LPAD].bitcast(F32), 0.0)
            nc.vector.memset(hi[:, NL:LPAD].bitcast(F32), 0.0)
            nc.vector.tensor_tensor(out=hr[:, 0:NL], in0=qr[:], in1=wrep_r[:],
                                    op=ALU.mult)
            nc.vector.tensor_tensor(out=t2[:], in0=qi[:], in1=wrep_i[:],
                                    op=ALU.mult)
            nc.vector.tensor_tensor(out=hr[:, 0:NL], in0=hr[:, 0:NL], in1=t2[:],
                                    op=ALU.subtract)
            nc.vector.tensor_tensor(out=hi[:, 0:NL], in0=qr[:], in1=wneg_i[:],
                                    op=ALU.mult)
            nc.vector.tensor_tensor(out=t2[:], in0=qi[:], in1=wneg_r[:],
                                    op=ALU.mult)
            nc.vector.tensor_tensor(out=hi[:, 0:NL], in0=hi[:, 0:NL], in1=t2[:],
                                    op=ALU.add)
            nc.sync.dma_start(scr["htdram"][128*i:128*(i+1), :], hr[:])
            nc.sync.dma_start(scr["htdram"][N+128*i:N+128*(i+1), :], hi[:])
            nc.vector.tensor_tensor(out=t2[:], in0=hr[:, 0:NL],
                                    in1=drep[:, 0:NL], op=ALU.mult)
            nc.vector.tensor_reduce(vsum[:, i:i+1], t2[:], axis=AXX, op=ALU.add)
            nc.vector.tensor_tensor(out=t2[:], in0=hi[:, 0:NL],
                                    in1=drep[:, 0:NL], op=ALU.mult)
            nc.vector.tensor_reduce(vsum[:, NB+i:NB+i+1], t2[:], axis=AXX,
                                    op=ALU.add)
        vsq = work.tile([128, 2 * NB], F32, tag="vsq")
        nc.vector.tensor_tensor(out=vsq[:], in0=vsum[:], in1=vsum[:],
                                op=ALU.mult)
        vred = work.tile([128, 1], F32, tag="vred")
        nc.vector.tensor_reduce(vred[:], vsq[:], axis=AXX, op=ALU.add)
        nc.gpsimd.partition_all_reduce(vred[:], vred[:], 128,
                                       bass_isa.ReduceOp.add)
        nc.scalar.activation(lam[:], vred[:], AF.Sqrt)
        nc.vector.tensor_scalar(out=lam[:], in0=lam[:], scalar1=float(alpha),
                                scalar2=None, op0=ALU.mult)

    # ---------------- P7: Gram ----------------
    st_ = late.tile([128, LB], F32)
    srep = late.tile([128, LPAD], F32)
    with (
        tc.tile_pool(name="g_acc", bufs=1) as gacc,
        tc.tile_pool(name="g_work", bufs=1) as work,
        tc.tile_pool(name="g_psum", bufs=4, space="PSUM") as pg,
    ):
        GA = [gacc.tile([128, LPAD], F32, tag=f"ga{l}", name=f"ga{l}") for l in range(LB)]
        GRP = 6
        for g0 in range(0, 2 * NB, GRP):
            htrs = []
            for gi in range(GRP):
                ch = g0 + gi
                htr = work.tile([128, LPAD], F32R, tag=f"g_htr{gi}",
                                name=f"g_htr{gi}")
                nc.sync.dma_start(htr[:], scr["htdram"][128*ch:128*(ch+1), :])
                htrs.append(htr)
            for l in range(LB):
                c0 = 128 * l
                for cc in range(c0, LPAD, 416):
                    cw = min(416, LPAD - cc)
                    pgt = pg.tile([128, 416], F32, tag="g_pg")
                    for gi in range(GRP):
                        nc.tensor.matmul(pgt[:, 0:cw],
                                         htrs[gi][:, c0:c0+128],
                                         htrs[gi][:, cc:cc+cw],
                                         start=(gi == 0), stop=(gi == GRP - 1))
                    if g0 == 0:
                        nc.vector.tensor_copy(GA[l][:, cc:cc+cw], pgt[:, 0:cw])
                    else:
                        nc.vector.tensor_tensor(out=GA[l][:, cc:cc+cw],
                                                in0=GA[l][:, cc:cc+cw],
                                                in1=pgt[:, 0:cw], op=ALU.add)
        for l in range(LB):
            nc.sync.dma_start(scr["gramdram"][128*l:128*(l+1), :], GA[l][:])
        gd = work.tile([128, LB], F32, tag="gd")
        nc.sync.dma_start(gd[:], bass.AP(scr["gramdram"], 0,
                                         [[LPAD + 1, 128],
                                          [128 * (LPAD + 1), LB]]))
        nc.vector.tensor_scalar(out=gd[:], in0=gd[:], scalar1=lam[:],
                                scalar2=None, op0=ALU.add)
        nc.scalar.activation(st_[:], gd[:], AF.Sqrt)
        nc.vector.reciprocal(st_[:], st_[:])
        ps_ = pg.tile([LB, 128], F32, tag="s_ps")
        nc.tensor.matmul(ps_[:], st_[:], id_s[:], start=True, stop=True)
        s13 = work.tile([LB, 128], F32, tag="s13")
        nc.vector.tensor_copy(s13[:], ps_[:])
        nc.sync.dma_start(bass.AP(scr["srowdram"], 0, [[1, LPAD]]), s13[:])
        srow = work.tile([1, LPAD], F32, tag="srow")
        nc.sync.dma_start(srow[:], bass.AP(scr["srowdram"], 0, [[1, LPAD]]))
        nc.gpsimd.partition_broadcast(srep[:], srow[:])

    # ---------------- P8: scaled SPD solve ----------------
    bf2_pool = ctx.enter_context(tc.tile_pool(name="bf2", bufs=1))
    BF2 = [bf2_pool.tile([128, 128], F32R, tag=f"bf2_{l}", name=f"bf2_{l}") for l in range(LB)]
    with (
        tc.tile_pool(name="s_tri", bufs=1) as tri2,
        tc.tile_pool(name="s_work", bufs=2) as work,
        tc.tile_pool(name="s_pmm", bufs=2, space="PSUM") as pmm,
        tc.tile_pool(name="s_pmisc", bufs=1, space="PSUM") as pmisc,
    ):
        dsc = work.tile([128, LB], F32, tag="dsc")
        nc.vector.tensor_tensor(out=dsc[:], in0=dvec[:], in1=st_[:], op=ALU.mult)
        zz = work.tile([128, 128], F32, tag="zz")
        nc.vector.memset(zz[:], 0.0)
        for l in range(LB):
            nc.vector.tensor_copy(BF2[l][:], zz[:])
            nc.vector.tensor_copy(BF2[l][:, 0:1], dsc[:, l:l+1])
        GT = {}
        for i in range(LB):
            for j in range(i, LB):
                GT[(i, j)] = tri2.tile([128, 128], F32R, tag=f"g{i}_{j}", name=f"g{i}_{j}")
                gload = work.tile([128, 128], F32, tag="g_load")
                nc.sync.dma_start(gload[:],
                                  scr["gramdram"][128*i:128*(i+1),
                                                  128*j:128*(j+1)])
                nc.vector.tensor_scalar(out=gload[:], in0=gload[:],
                                        scalar1=st_[:, i:i+1], scalar2=None,
                                        op0=ALU.mult)
                nc.vector.tensor_tensor(out=gload[:], in0=gload[:],
                                        in1=srep[:, 128*j:128*(j+1)],
                                        op=ALU.mult)
                if i == j:
                    ones1 = work.tile([128, 1], F32, tag="diag1")
                    nc.vector.memset(ones1[:], 1.0)
                    nc.vector.copy_predicated(gload[:], idu_s[:],
                                              ones1[:].broadcast_to([128, 128]))
                nc.vector.tensor_copy(GT[(i, j)][:], gload[:])
        for k in range(LB):
            V = work.tile([128, 128], F32R, tag="lu2_V")
            _newton_real(nc, work, pmm, pmisc, GT[(k, k)], V, id_s, NEWTON_SPD)
            nc.sync.dma_start(scr["v2dram"][128*k:128*(k+1), :], V[:])
            for i in range(k + 1, LB):
                ptr = pmisc.tile([128, 128], F32R, tag="lu2_ptr")
                nc.tensor.transpose(ptr[:], GT[(k, i)][:], idr_s[:])
                utt = work.tile([128, 128], F32R, tag="lu2_utt")
                nc.vector.tensor_copy(utt[:], ptr[:])
                nc.sync.dma_start(
                    scr["ut2dram"][128*i:128*(i+1), 128*k:128*(k+1)], utt[:])
            for i in range(k + 1, LB):
                pl = pmm.tile([128, 128], F32, tag="cmmp1")
                nc.tensor.matmul(pl[:], V[:], GT[(k, i)][:], start=True,
                                 stop=True)
                LT = work.tile([128, 128], F32R, tag="lu2_LT")
                nc.vector.tensor_copy(LT[:], pl[:])
                pb = pmm.tile([128, 128], F32, tag="cmmp2")
                nc.tensor.matmul(pb[:], LT[:], BF2[k][:], start=True, stop=True)
                nc.vector.tensor_tensor(out=BF2[i][:], in0=BF2[i][:],
                                        in1=pb[:], op=ALU.subtract)
                for j in range(i, LB):
                    pt_ = pmm.tile([128, 128], F32, tag="cmmp1")
                    nc.tensor.matmul(pt_[:], LT[:], GT[(k, j)][:], start=True,
                                     stop=True)
                    nc.vector.tensor_tensor(out=GT[(i, j)][:],
                                            in0=GT[(i, j)][:], in1=pt_[:],
                                            op=ALU.subtract)

    ys = late.tile([128, LB], F32)
    yrep = late.tile([128, LPAD], F32)
    with (
        tc.tile_pool(name="b2_work", bufs=3) as work,
        tc.tile_pool(name="b2_pacc", bufs=1, space="PSUM") as pacc,
        tc.tile_pool(name="b2_pmm", bufs=2, space="PSUM") as pmm,
    ):
        for k in range(LB - 1, -1, -1):
            W = work.tile([128, 128], F32R, tag="bs2_W")
            nc.vector.tensor_copy(W[:], BF2[k][:])
            if k < LB - 1:
                P1 = pacc.tile([128, 128], F32, tag="bs2_p1")
                for idx, j in enumerate(range(k + 1, LB)):
                    utt = work.tile([128, 128], F32R, tag="bs2_utt")
                    nc.sync.dma_start(
                        utt[:], scr["ut2dram"][128*j:128*(j+1),
                                               128*k:128*(k+1)])
                    nc.tensor.matmul(P1[:], utt[:], BF2[j][:],
                                     start=(idx == 0), stop=(j == LB - 1))
                nc.vector.tensor_tensor(out=W[:], in0=W[:], in1=P1[:],
                                        op=ALU.subtract)
            Vk = work.tile([128, 128], F32R, tag="bs2_V")
            nc.sync.dma_start(Vk[:], scr["v2dram"][128*k:128*(k+1), :])
            Pf = pmm.tile([128, 128], F32, tag="bs2_pf")
            nc.tensor.matmul(Pf[:], Vk[:], W[:], start=True, stop=True)
            nc.vector.tensor_copy(BF2[k][:], Pf[:])
        for l in range(LB):
            nc.vector.tensor_copy(ys[:, l:l+1], BF2[l][:, 0:1])
        nc.vector.tensor_tensor(out=ys[:], in0=ys[:], in1=st_[:], op=ALU.mult)
        psy = pmm.tile([LB, 128], F32, tag="y_ps")
        nc.tensor.matmul(psy[:], ys[:], id_s[:], start=True, stop=True)
        y13 = work.tile([LB, 128], F32, tag="y13")
        nc.vector.tensor_copy(y13[:], psy[:])
        nc.sync.dma_start(bass.AP(scr["yrowdram"], 0, [[1, LPAD]]), y13[:])
        yrow = work.tile([1, LPAD], F32, tag="yrow")
        nc.sync.dma_start(yrow[:], bass.AP(scr["yrowdram"], 0, [[1, LPAD]]))
        nc.gpsimd.partition_broadcast(yrep[:], yrow[:])

    # ---------------- P9: chi = Ht y ----------------
    with tc.tile_pool(name="p9_work", bufs=2) as work:
        chi = late.tile([128, 2 * NB], F32)
        for ch in range(2 * NB):
            htc = work.tile([128, LPAD], F32R, tag="c_htc")
            nc.sync.dma_start(htc[:], scr["htdram"][128*ch:128*(ch+1), :])
            tm = work.tile([128, LPAD], F32, tag="c_tm")
            nc.vector.tensor_tensor(out=tm[:], in0=htc[:], in1=yrep[:],
                                    op=ALU.mult)
            nc.vector.tensor_reduce(chi[:, ch:ch+1], tm[:], axis=AXX,
                                    op=ALU.add)
        nc.sync.dma_start(bass.AP(out_chi, 0, [[1, 128], [128, 2 * NB]]),
                          chi[:])
    ctx.close()


_GBASE = {}

def _contig_segments(rs_list):
    segs = []
    s = rs_list[0]; prev = s
    for r in rs_list[1:]:
        if r == prev + 1:
            prev = r
        else:
            segs.append((s, prev - s + 1)); s = r; prev = r
    segs.append((s, prev - s + 1))
    return segs


_CACHED = {}


def kernel(epsilon_r_iter, chi_iter, total_power, alpha, grid_x, grid_y,
           direct_field, incident_field, G_freespace, G_freespace_scaled,
           sensor_links):
    eps = np.asarray(epsilon_r_iter)
    chi_it = np.asarray(chi_iter)
    tp = np.asarray(total_power, dtype=np.float32)
    alpha_f = float(np.asarray(alpha))
    gx = np.asarray(grid_x, dtype=np.float32)
    gy = np.asarray(grid_y, dtype=np.float32)
    df = np.asarray(direct_field)
    einc = np.asarray(incident_field)
    gfs = np.asarray(G_freespace)
    gsc = np.asarray(G_freespace_scaled)
    links = np.asarray(sensor_links)

    x = gx.T.reshape(N).astype(np.float32)
    y = gy.T.reshape(N).astype(np.float32)
    scat = np.real(eps.T.reshape(N)).astype(np.float32)

    geomS = np.stack([np.ones(N, np.float32), -2.0*x, -2.0*y,
                      (x*x + y*y)]).astype(np.float32)
    geomR = np.stack([(x*x + y*y), x, y,
                      np.ones(N, np.float32)]).astype(np.float32)
    scat_t = scat.reshape(NB, 128).T.copy()

    bpack = np.zeros((N, RW), np.float32)
    bpack[:, 0:40] = -einc.real; bpack[:, 40:80] = -gfs.real
    bpack[:, 128:168] = -einc.imag; bpack[:, 168:208] = -gfs.imag
    gscT = np.concatenate([gsc.real.T, gsc.imag.T], axis=1).astype(np.float32)
    dfpack = np.concatenate([df.real, df.imag], axis=1).astype(np.float32)
    tpT = tp.T.copy().astype(np.float32)

    groups = []
    i = 0
    while i < len(links):
        t = int(links[i, 0])
        rs_list = []
        while i < len(links) and int(links[i, 0]) == t:
            rs_list.append(int(links[i, 1]))
            i += 1
        groups.append((t, rs_list))

    _GBASE.clear()
    o = 0
    for (t, rs_list) in groups:
        _GBASE[t] = o
        o += len(rs_list)
    key = (hash(links.tobytes()), alpha_f)
    if key not in _CACHED:
        _CACHED[key] = build_program(groups, alpha_f)
    nc = _CACHED[key]

    id128 = np.eye(128, dtype=np.float32)
    im = {
        "geomS": geomS, "geomR": geomR, "scat_t": scat_t, "bpack": bpack,
        "gscT": gscT, "dfpack": dfpack, "tpT": tpT,
        "id128": id128, "idu8": id128.astype(np.uint8),
    }
    import os as _os
    _tr = _os.environ.get("KTRACE", "0") == "1"
    res = run_bass_kernel_spmd(nc, [im] * 8, core_ids=list(range(8)), trace=_tr)
    out = res.results[0]
    _CACHED["last"] = (res, out)

    chi = out["out_chi"]
    dchi_r = chi[:N].reshape(M, M).T
    dchi_i = chi[N:].reshape(M, M).T
    chi_new = (chi_it + (dchi_r + 1j * dchi_i)).astype(np.complex64)
    return chi_new + 1.0, chi_new

